# revision 23
# baseline (speedup 1.0000x reference)
# Trainium2 Bass kernel for nn_BAKTTime (dense_transformer).
# Self-contained: hardcodes shapes B=2, L=2048, D=256, H=8, dk=32.
#
# Sharding: 8 cores, SPMD program. core i handles batch (i & 1) and query
# slab (i // 2); slab j computes the position-local pipeline (folded
# 5-tap conv + layernorm + capsule routing + banded time attention + fusion
# + qkv) over the causal prefix [0, 512*(j+1)), then flash MHA over q in
# [512*j, 512*(j+1)).  The wall-clock of this problem is dominated by
# host<->device transfer over the axon tunnel, so all inputs are packed
# into one fp16 [128, C] tensor per core; with USE_AG each core ships only
# a 1/8 column slice and an on-device AllGather reconstructs the full
# pack.  Constant matrices (identity, causal mask, decay bias, bias rows,
# depthwise-diag) are built on device.  Output is a per-core fp16
# (512, 256) slab; the host stitches the 8 slabs.
import numpy as np

B, L, D = 2, 2048, 256
H, DK = 8, 32
DECAY = 0.2
EPS_LN = 1e-12
NEG = -1e30
CW = 512
ABOUNDS = (0, 512, 1024, 1536, 2048)
SLAB = 512

USE_AG = True  # AllGather weights+x on device (ship 1/8 per core)

# ---- fp16 pack column layout (single source of truth) ----
C_WTT = 0                      # [2(k),5(t),2(o),128]
C_FWT = C_WTT + 2560           # [4(k),2(o),128]
C_WQT = C_FWT + 1024           # [2(k),2(o),128]
C_WKT = C_WQT + 512
C_WVT = C_WKT + 512            # [2(k),256]
C_WO = C_WVT + 512             # head h at partitions [32*(h%4),+32), col (h//4)*256
C_EYE = C_WO + 512             # [128,128] identity
C_DWC = C_EYE + 128            # [2(pd),5(t),2(o)] depthwise tap coefs
C_IOTA = C_DWC + 20            # [128,1] iota
C_BVC = C_IOTA + 1             # [2] bv col layout
C_BOC = C_BVC + 2              # [2] bo col layout
C_P32HI = C_BOC + 2            # [33] fp16 hi half of the f32 pack
C_P32LO = C_P32HI + 33         # [33] fp16 lo half (v - f32(hi))
W_COLS = 5888                  # C_P32LO+33 = 5851, padded to 5888
X_COLS = 2 * (4 + L)           # 4104
PK_COLS = W_COLS + X_COLS      # 9896 (per-core pack: weights + my batch xT)
AG_COLS = W_COLS + 2 * X_COLS  # 14000 (global pack: weights + both batches)
AG_SL = AG_COLS // 8           # 1750

# pack32 f32 [128, 33]
P32_BEFF, P32_LNW, P32_LNB, P32_FB, P32_BQ, P32_BK, P32_IOTA, P32_DWC = 0, 2, 4, 6, 8, 10, 12, 13
P32_COLS = 33


def _host_prep(inp):
    f32, f16 = np.float32, np.float16
    x = np.asarray(inp["x"], f32)
    c3w, c3b = np.asarray(inp["conv3_w"], f32), np.asarray(inp["conv3_b"], f32)
    c5w, c5b = np.asarray(inp["conv5_w"], f32), np.asarray(inp["conv5_b"], f32)
    b3 = np.asarray(inp["beta3"], f32).reshape(D)
    b5 = np.asarray(inp["beta5"], f32).reshape(D)
    cw = np.asarray(inp["combine_w"], f32)
    cwt = np.exp(cw - cw.max())
    cwt = (cwt / cwt.sum()).astype(f32)
    g3 = (cwt[0] * (1.0 - b3 ** 2)).astype(f32)
    g5 = (cwt[1] * (1.0 - b5 ** 2)).astype(f32)
    dscale = (1.0 + cwt[0] * b3 ** 2 + cwt[1] * b5 ** 2).astype(f32)
    W = np.zeros((5, D, D), f32)
    W[0] = g3[:, None] * c3w[:, :, 2] + g5[:, None] * c5w[:, :, 4] + np.diag(dscale)
    W[1] = g3[:, None] * c3w[:, :, 1] + g5[:, None] * c5w[:, :, 3]
    W[2] = g3[:, None] * c3w[:, :, 0] + g5[:, None] * c5w[:, :, 2]
    W[3] = g5[:, None] * c5w[:, :, 1]
    W[4] = g5[:, None] * c5w[:, :, 0]
    # lhsT layout [din_par, din_ch(k), tap, o_ch, dout]
    wtT = np.transpose(W, (0, 2, 1)).reshape(5, 2, 128, 2, 128)
    wtT = np.ascontiguousarray(np.transpose(wtT, (2, 1, 0, 3, 4)))  # [128,2,5,2,128]
    beff = (g3 * c3b + g5 * c5b).reshape(2, 128).T.copy()           # [128, 2]
    dw3 = np.asarray(inp["dw3_w"], f32)[:, 0, :]
    dw5 = np.asarray(inp["dw5_w"], f32)[:, 0, :]
    c3l = np.zeros((5, D), f32)
    c5l = np.zeros((5, D), f32)
    for l in range(3):
        c3l[l] = dw3[:, 2 - l]
    for l in range(5):
        c5l[l] = dw5[:, 4 - l]
    pco, dco = c3l + c5l, c3l - c5l                                  # [5, 256]
    # dwc [128, 2(pd), 5(t), 2(o)]
    dwc = np.zeros((128, 2, 5, 2), f32)
    for t in range(5):
        for o in range(2):
            dwc[:, 0, t, o] = pco[t, o * 128:(o + 1) * 128]
            dwc[:, 1, t, o] = dco[t, o * 128:(o + 1) * 128]
    col = lambda v: np.asarray(v, f32).reshape(2, 128).T.copy()      # [128, 2]
    fwT = np.asarray(inp["fusion_w"], f32).T.reshape(4, 128, 2, 128)
    fwT = np.ascontiguousarray(np.transpose(fwT, (1, 0, 2, 3)))      # [128,4,2,128]
    s = 1.0 / np.sqrt(DK)

    def sqT(w):
        t = np.asarray(w, f32).T.reshape(2, 128, 2, 128)
        return np.ascontiguousarray(np.transpose(t, (1, 0, 2, 3)))   # [128,2,2,128]

    def hvT(w):
        t = np.asarray(w, f32).T.reshape(2, 128, 256)
        return np.ascontiguousarray(np.transpose(t, (1, 0, 2)))      # [128,2,256]

    # wo pack [128, 512]: head h tile (32,256) at partitions [32*(h%4),+32),
    # cols [(h//4)*256,+256)
    woT = np.asarray(inp["wo"], f32).T.reshape(8, 32, 256).transpose(1, 0, 2)  # [32,8,256]
    wop = np.zeros((128, 512), f32)
    for h in range(8):
        wop[32 * (h % 4):32 * (h % 4) + 32, (h // 4) * 256:(h // 4) * 256 + 256] = woT[:, h, :]

    Wpk = np.zeros((128, W_COLS), f32)
    Wpk[:, C_WTT:C_WTT + 2560] = wtT.reshape(128, -1)
    Wpk[:, C_FWT:C_FWT + 1024] = fwT.reshape(128, -1)
    Wpk[:, C_WQT:C_WQT + 512] = sqT(np.asarray(inp["wq"], f32) * s).reshape(128, -1)
    Wpk[:, C_WKT:C_WKT + 512] = sqT(inp["wk"]).reshape(128, -1)
    Wpk[:, C_WVT:C_WVT + 512] = hvT(inp["wv"]).reshape(128, -1)
    Wpk[:, C_WO:C_WO + 512] = wop
    Wpk[:, C_EYE:C_EYE + 128] = np.eye(128, dtype=f32)
    Wpk[:, C_DWC:C_DWC + 20] = dwc.reshape(128, -1)
    Wpk[:, C_IOTA:C_IOTA + 1] = np.arange(128, dtype=f32)[:, None]
    Wpk[:, C_BVC:C_BVC + 2] = col(inp["bv"])
    Wpk[:, C_BOC:C_BOC + 2] = col(inp["bo"])

    # xT [B, 128, 2, 4+L]: 4 leading zero cols per o-half for the conv halo
    xT = np.zeros((B, 128, 2, 4 + L), f32)
    xt_full = np.transpose(x, (0, 2, 1)).reshape(B, 2, 128, L)
    xT[:, :, :, 4:] = np.transpose(xt_full, (0, 2, 1, 3))
    xT16 = xT.reshape(B, 128, X_COLS).astype(f16)

    pk32 = np.zeros((128, P32_COLS), f32)
    pk32[:, P32_BEFF:P32_BEFF + 2] = beff
    pk32[:, P32_LNW:P32_LNW + 2] = col(inp["ln_w"])
    pk32[:, P32_LNB:P32_LNB + 2] = col(inp["ln_b"])
    pk32[:, P32_FB:P32_FB + 2] = col(inp["fusion_b"])
    pk32[:, P32_BQ:P32_BQ + 2] = col(np.asarray(inp["bq"], f32) * s)
    pk32[:, P32_BK:P32_BK + 2] = col(inp["bk"])
    pk32[:, P32_IOTA:P32_IOTA + 1] = np.arange(128, dtype=f32)[:, None]
    pk32[:, P32_DWC:P32_DWC + 20] = dwc.reshape(128, -1)
    hi = pk32.astype(f16)
    lo = (pk32 - hi.astype(f32)).astype(f16)
    Wpk[:, C_P32HI:C_P32HI + P32_COLS] = hi.astype(f32)
    Wpk[:, C_P32LO:C_P32LO + P32_COLS] = lo.astype(f32)
    Wpk16 = Wpk.astype(f16)
    return Wpk16, xT16


def build_in_maps(inputs):
    Wpk16, xT16 = _host_prep(inputs)
    in_maps = []
    if USE_AG:
        gpack = np.concatenate([Wpk16, xT16[0], xT16[1]], axis=1)  # [128, AG_COLS]
        for core in range(8):
            in_maps.append(dict(
                pksl=np.ascontiguousarray(gpack[:, AG_SL * core:AG_SL * (core + 1)])))
    else:
        pk_b = [np.ascontiguousarray(np.concatenate([Wpk16, xT16[b]], axis=1))
                for b in range(B)]
        for core in range(8):
            in_maps.append(dict(pk16=pk_b[core & 1]))
    return in_maps


def _build(force_variant=None, use_ag=None):
    import concourse.mybir as mybir
    import concourse.tile as tile
    from concourse import bacc

    F32, F32R, F16 = mybir.dt.float32, mybir.dt.float32r, mybir.dt.float16
    U32 = mybir.dt.uint32
    AF = mybir.ActivationFunctionType
    ALU = mybir.AluOpType
    # 2*0x5f3759df + 1 as signed int32, for rsqrt seed (C2 + ~i) >> 1
    RSQRT_C2 = 0xBE6EB3BF - (1 << 32)

    use_ag = USE_AG if use_ag is None else use_ag

    nc = bacc.Bacc(num_devices=8) if use_ag else bacc.Bacc()

    if use_ag:
        pksl_d = nc.dram_tensor("pksl", [128, AG_SL], F16, kind="ExternalInput")
        ag_in = nc.dram_tensor("ag_in", [128, AG_SL], F16)
        ag_out = nc.dram_tensor("ag_out", [8, 128, AG_SL], F16, addr_space="Shared")
    else:
        pk16_d = nc.dram_tensor("pk16", [128, PK_COLS], F16, kind="ExternalInput")
    ag2_in = nc.dram_tensor("ag2_in", [128, 2048], F16)
    ag2_out = nc.dram_tensor("ag2_out", [8, 128, 2048], F16, addr_space="Shared")
    outd = nc.dram_tensor("out", [SLAB, D], F16, kind="ExternalOutput")

    X0 = W_COLS  # my-batch xT offset within pk

    with tile.TileContext(nc) as tc:
        pid = nc.partition_id() if force_variant is None else None
        with tc.tile_pool(name="wpool", bufs=1) as wpool, \
             tc.tile_pool(name="ppool", bufs=1) as ppool, \
             tc.tile_pool(name="work", bufs=1) as work, \
             tc.tile_pool(name="wk3", bufs=4) as wk3, \
             tc.tile_pool(name="pspool", bufs=1, space="PSUM") as pspool:
            psO = pspool

            pk = wpool.tile([128, PK_COLS], F16, name="pk")
            pk32 = wpool.tile([128, P32_COLS], F32, name="pk32s")
            if use_ag:
                nc.sync.dma_start(ag_in[:], pksl_d[:])
                nc.gpsimd.collective_compute(
                    "AllGather", ALU.bypass,
                    replica_groups=[list(range(8))],
                    ins=[ag_in[:]], outs=[ag_out[:]])

                def load_cols(dst_c0, g_c0, g_c1):
                    # copy global pack cols [g_c0,g_c1) into pk[:, dst_c0...]
                    for blk in range(8):
                        b0, b1 = blk * AG_SL, (blk + 1) * AG_SL
                        lo, hi = max(g_c0, b0), min(g_c1, b1)
                        if lo < hi:
                            nc.sync.dma_start(
                                pk[:, dst_c0 + lo - g_c0:dst_c0 + hi - g_c0],
                                ag_out[blk, :, lo - b0:hi - b0])

                load_cols(0, 0, W_COLS)

                # my batch's xT: binary branch tree on pid (batch = pid & 1)
                def xt_tree(lo_pid, hi_pid):
                    if hi_pid - lo_pid == 1:
                        bsel = lo_pid & 1
                        load_cols(X0, W_COLS + bsel * X_COLS,
                                  W_COLS + (bsel + 1) * X_COLS)
                        return
                    mid = (lo_pid + hi_pid) // 2
                    with tc.If(pid < mid) as cc:
                        xt_tree(lo_pid, mid)
                    with cc.Else():
                        xt_tree(mid, hi_pid)

                xt_tree(0, 8)
            else:
                nc.sync.dma_start(pk[:], pk16_d[:])

            # reconstruct the f32 side-pack from fp16 hi/lo halves
            p32lo = wpool.tile([128, P32_COLS], F32, name="p32lo")
            nc.vector.tensor_copy(out=pk32[:], in_=pk[:, C_P32HI:C_P32HI + P32_COLS])
            nc.vector.tensor_copy(out=p32lo[:], in_=pk[:, C_P32LO:C_P32LO + P32_COLS])
            nc.vector.tensor_add(out=pk32[:], in0=pk32[:], in1=p32lo[:])

            # ---- on-device constants ----
            ones32 = wpool.tile([128, 512], F32R, name="ones32")
            nc.vector.memset(ones32[:].bitcast(F32), 1.0)
            ones16 = wpool.tile([128, 512], F16, name="ones16")
            nc.vector.tensor_copy(out=ones16[:], in_=ones32[:])
            zeros8 = wpool.tile([128, 8], F32, name="zeros8")
            nc.vector.memset(zeros8[:], 0.0)
            eps_sb = wpool.tile([128, 1], F32, name="eps_sb")
            nc.vector.memset(eps_sb[:], EPS_LN)
            dsign_sb = wpool.tile([128, 4, 2], F32, name="dsign_sb")
            nc.vector.memset(dsign_sb[:, :, 0:1], -1.0)
            nc.vector.memset(dsign_sb[:, :, 1:2], 1.0)

            eye16 = pk[:, C_EYE:C_EYE + 128]
            eye32 = wpool.tile([128, 128], F32R, name="eye32")
            nc.vector.tensor_copy(out=eye32[:], in_=eye16)

            # iota row via M=1 matmul: out[0,j] = sum_k iota[k]*eye[k,j]
            rowp = pspool.tile([128, 128], F32, tag="E1", name="rowp")
            nc.tensor.matmul(rowp[0:1, 0:128], pk[:, C_IOTA:C_IOTA + 1], eye16,
                             start=True, stop=True)
            iota_r16 = wpool.tile([1, 128], F16, name="iota_r16")
            nc.vector.tensor_copy(out=iota_r16[:], in_=rowp[0:1, 0:128])
            iota_r32 = wpool.tile([1, 128], F32, name="iota_r32")
            nc.vector.tensor_copy(out=iota_r32[:], in_=rowp[0:1, 0:128])

            # decay bias row [1, 512]: dbias[bb*128+i] = DECAY*(i + 128*(bb-2))
            dbias16 = wpool.tile([1, 512], F16, name="dbias16")
            for bb in range(4):
                nc.vector.tensor_scalar(
                    out=dbias16[0:1, bb * 128:(bb + 1) * 128], in0=iota_r32[:],
                    scalar1=DECAY, scalar2=DECAY * 128.0 * (bb - 2),
                    op0=ALU.mult, op1=ALU.add)

            # bias rows [1, 256] from col layout via M=1 matmuls
            bv_row = wpool.tile([1, 256], F16, name="bv_row")
            bo_row = wpool.tile([1, 256], F16, name="bo_row")
            for dst, c0 in ((bv_row, C_BVC), (bo_row, C_BOC)):
                for o in range(2):
                    rp = pspool.tile([128, 128], F32, tag="E1", name="rowp2")
                    nc.tensor.matmul(rp[0:1, 0:128], pk[:, c0 + o:c0 + o + 1],
                                     eye16, start=True, stop=True)
                    nc.vector.tensor_copy(out=dst[0:1, o * 128:(o + 1) * 128],
                                          in_=rp[0:1, 0:128])

            # causal band mask T32[r, c] = NEG where c < r else 0
            Jps = pspool.tile([128, 128], F32, tag="E2", name="Jps")
            nc.tensor.matmul(Jps[:, 0:128], ones16[0:1, 0:128], iota_r16[0:1, 0:128],
                             start=True, stop=True)
            T32 = wpool.tile([128, 128], F32, name="T32")
            nc.vector.tensor_scalar(out=T32[:], in0=Jps[:, 0:128],
                                    scalar1=pk32[:, P32_IOTA:P32_IOTA + 1],
                                    scalar2=NEG, op0=ALU.is_lt, op1=ALU.mult)


            # wo tiles at partitions 0-31: [32, 8, 256]
            woT_sb = wpool.tile([32, 8, 256], F16, name="woT_sb")
            for h in range(8):
                nc.sync.dma_start(
                    woT_sb[0:32, h, :],
                    pk[32 * (h % 4):32 * (h % 4) + 32,
                       C_WO + (h // 4) * 256:C_WO + (h // 4) * 256 + 256])

            hT32 = ppool.tile([128, 2, 2, 4 + CW], F32, name="hT32")
            vT_s = ppool.tile([128, 2, 6 * 128], F16, name="vT_s")
            vT_t = ppool.tile([128, 2, 6 * 128], F16, name="vT_t")
            vn_s = ppool.tile([128, 6, 256], F32R, name="vn_s")
            vn_t = ppool.tile([128, 6, 256], F32R, name="vn_t")
            v_all = ppool.tile([128, L // 128, 256], F32R, name="v_all")
            v16 = ppool.tile([128, L // 128, 256], F16, name="v16")
            qT_all = ppool.tile([128, 2, L], F16, name="qT_all")
            kT_all = ppool.tile([128, 2, L], F16, name="kT_all")
            for _o in range(2):
                for _p in range(2):
                    nc.vector.memset(hT32[:, _o, _p, 0:4], 0.0)
            vT = {0: vT_s, 1: vT_t}
            vn = {0: vn_s, 1: vn_t}

            def emit_rsqrt(eng, y, x, tmp, niter=2):
                # y <- 1/sqrt(x) elementwise; x must be > 0 (pre-clamped).
                yi, xi, ti = y.bitcast(U32), x.bitcast(U32), tmp.bitcast(U32)
                nc.vector.tensor_scalar(out=ti, in0=xi, scalar1=0, scalar2=None,
                                        op0=ALU.bitwise_not)
                nc.vector.tensor_scalar(out=ti, in0=ti, scalar1=RSQRT_C2,
                                        scalar2=None, op0=ALU.add)
                nc.vector.tensor_scalar(out=yi, in0=ti, scalar1=1, scalar2=None,
                                        op0=ALU.logical_shift_right)
                for _ in range(niter):
                    eng.tensor_mul(out=tmp, in0=y, in1=y)
                    eng.tensor_mul(out=tmp, in0=tmp, in1=x)
                    eng.tensor_scalar(out=tmp, in0=tmp, scalar1=-0.5,
                                      scalar2=1.5, op0=ALU.mult, op1=ALU.add)
                    eng.tensor_mul(out=y, in0=y, in1=tmp)

            def emit_s1f(l0, w):
                # folded conv + y/sq activations (x read from resident pack)
                y = work.tile([128, 2, CW], F32R, tag="y_sb")
                sq = work.tile([128, 2, CW], F16, tag="sq_sb")
                for o in range(2):
                    ps = pspool.tile([128, CW], F32, tag="E1")
                    for t in range(5):
                        for k in range(2):
                            xc = X0 + k * (4 + L) + l0 + 4 - t
                            nc.tensor.matmul(ps[:, 0:w],
                                             pk[:, C_WTT + ((k * 5 + t) * 2 + o) * 128:
                                                C_WTT + ((k * 5 + t) * 2 + o) * 128 + 128],
                                             pk[:, xc:xc + w],
                                             start=(t == 0 and k == 0),
                                             stop=(t == 4 and k == 1))
                    nc.scalar.activation(y[:, o, 0:w], ps[:, 0:w], AF.Identity,
                                         bias=pk32[:, P32_BEFF + o:P32_BEFF + o + 1],
                                         scale=1.0)
                    nc.scalar.activation(sq[:, o, 0:w], ps[:, 0:w], AF.Square,
                                         bias=pk32[:, P32_BEFF + o:P32_BEFF + o + 1],
                                         scale=1.0)
                return y, sq

            def emit_s1b1(l0, w, y, sq):
                # layernorm moments via replicated-moment matmuls
                mps = pspool.tile([128, 2, CW], F32, tag="E2")
                for o in range(2):
                    nc.tensor.matmul(mps[:, 0, 0:w], ones32[:, 0:128], y[:, o, 0:w],
                                     start=(o == 0), stop=(o == 1))
                    nc.tensor.matmul(mps[:, 1, 0:w], ones16[:, 0:128], sq[:, o, 0:w],
                                     start=(o == 0), stop=(o == 1))
                mu = work.tile([128, CW], F32, tag="mu")
                nc.vector.tensor_scalar_mul(out=mu[:, 0:w], in0=mps[:, 0, 0:w], scalar1=1.0 / D)
                mu2 = work.tile([128, CW], F32, tag="tmpA")
                nc.scalar.activation(mu2[:, 0:w], mu[:, 0:w], AF.Square)
                var = work.tile([128, CW], F32, tag="tmpB")
                nc.vector.scalar_tensor_tensor(out=var[:, 0:w], in0=mps[:, 1, 0:w],
                                               scalar=1.0 / D, in1=mu2[:, 0:w],
                                               op0=ALU.mult, op1=ALU.subtract)
                return mu, var

            def emit_s1b2(l0, w, y, mu, var):
                par = (l0 // CW) & 1
                c0 = l0 % CW
                lnv = work.tile([128, CW], F32, tag="tmpA")
                nc.scalar.activation(lnv[:, 0:w], var[:, 0:w], AF.Ln, bias=eps_sb[:])
                rstd = work.tile([128, CW], F32, tag="tmpB")
                nc.scalar.activation(rstd[:, 0:w], lnv[:, 0:w], AF.Exp, scale=-0.5)
                for o in range(2):
                    t1 = work.tile([128, CW], F32, tag="tmpA" if o else "tmpC")
                    nc.vector.tensor_sub(out=t1[:, 0:w], in0=y[:, o, 0:w], in1=mu[:, 0:w])
                    nc.vector.tensor_mul(out=t1[:, 0:w], in0=t1[:, 0:w], in1=rstd[:, 0:w])
                    nc.vector.tensor_scalar(out=hT32[:, o, par, 4 + c0:4 + c0 + w],
                                            in0=t1[:, 0:w],
                                            scalar1=pk32[:, P32_LNW + o:P32_LNW + o + 1],
                                            scalar2=pk32[:, P32_LNB + o:P32_LNB + o + 1],
                                            op0=ALU.mult, op1=ALU.add)
                if c0 + w == CW:
                    nc.vector.tensor_copy(out=hT32[:, :, 1 - par, 0:4],
                                          in_=hT32[:, :, par, CW:CW + 4])

            def emit_s2(l0, w):
                par = (l0 // CW) & 1
                c0 = l0 % CW
                nblk = w // 128
                blk6 = lambda b: (l0 // 128 + b) % 6
                # ---- trend taps: P = t3+t5, Dt = t3-t5 (per-channel shift
                # chains on DVE/GpSimd; beats diag matmuls in this runtime) ----
                wsT = work.tile([128, 2, CW], F32R, tag="bufA")
                wtTt = work.tile([128, 2, CW], F32R, tag="bufB")
                dnT = work.tile([128, 2, CW], F32R, tag="bufC")
                for o in range(2):
                    for t in range(5):
                        src = hT32[:, o, par, 4 + c0 - t:4 + c0 - t + w]
                        cP = pk32[:, P32_DWC + t * 2 + o:P32_DWC + t * 2 + o + 1]
                        cD = pk32[:, P32_DWC + 10 + t * 2 + o:P32_DWC + 10 + t * 2 + o + 1]
                        if t == 0:
                            nc.vector.tensor_scalar(out=wtTt[:, o, 0:w], in0=src,
                                                    scalar1=cP, scalar2=None,
                                                    op0=ALU.mult)
                            nc.vector.tensor_scalar(out=dnT[:, o, 0:w], in0=src,
                                                    scalar1=cD, scalar2=None,
                                                    op0=ALU.mult)
                        else:
                            nc.vector.scalar_tensor_tensor(
                                out=wtTt[:, o, 0:w], in0=src, scalar=cP,
                                in1=wtTt[:, o, 0:w], op0=ALU.mult, op1=ALU.add)
                            nc.vector.scalar_tensor_tensor(
                                out=dnT[:, o, 0:w], in0=src, scalar=cD,
                                in1=dnT[:, o, 0:w], op0=ALU.mult, op1=ALU.add)
                    nc.vector.scalar_tensor_tensor(out=wsT[:, o, 0:w],
                                                   in0=hT32[:, o, par, 4 + c0:4 + c0 + w],
                                                   scalar=2.0, in1=wtTt[:, o, 0:w],
                                                   op0=ALU.mult, op1=ALU.subtract)
                # ---- transpose routing inputs to [l, c] ----
                wsn = work.tile([128, 4, 256], F32, tag="wsn")
                wtn = work.tile([128, 4, 256], F32, tag="wtn")
                dnn = work.tile([128, 4, 256], F32, tag="dnn")
                for srct, dst, use_act in ((wsT, wsn, False), (wtTt, wtn, True),
                                           (dnT, dnn, True)):
                    for bi in range(nblk):
                        pst = pspool.tile([128, 2, 128], F32R, tag="E1")
                        for o in range(2):
                            nc.tensor.transpose(pst[:, o, :], srct[:, o, bi * 128:(bi + 1) * 128], eye32[:])
                        if use_act:
                            nc.scalar.activation(dst[:, bi, :], pst[:],
                                                 AF.Copy, bias=0.0, scale=1.0)
                        else:
                            nc.vector.tensor_copy(out=dst[:, bi, :], in_=pst[:])
                yield
                # ---- routing invariants (st = wn + w*dnn):
                #   A_p = sum wn_p^2, B_p = sum dnn*wn_p, C = sum dnn^2
                # then per-iteration sums are analytic:
                #   S(w) = A + w*(B + D(w)),  D(w) = B + w*C.
                g = work.tile([128, 16, 4, 2], F32, tag="g")
                diff = work.tile([128, 4, 2], F32, tag="diff")
                scrA = work.tile([128, 4, 256], F32, tag="bufB", name="scrA")
                scrB = work.tile([128, 4, 256], F32, tag="bufA", name="scrB")
                nb = nblk
                nc.scalar.activation(scrA[:, 0:nb, :], wsn[:, 0:nb, :], AF.Square)
                nc.vector.tensor_reduce(out=g[:, 0, 0:nb, 0:1], in_=scrA[:, 0:nb, :],
                                        axis=mybir.AxisListType.X, op=ALU.add)
                nc.scalar.activation(scrB[:, 0:nb, :], wtn[:, 0:nb, :], AF.Square)
                nc.vector.tensor_reduce(out=g[:, 0, 0:nb, 1:2], in_=scrB[:, 0:nb, :],
                                        axis=mybir.AxisListType.X, op=ALU.add)
                nc.gpsimd.tensor_mul(out=scrA[:, 0:nb, :], in0=dnn[:, 0:nb, :],
                                     in1=dnn[:, 0:nb, :])
                nc.vector.tensor_reduce(out=g[:, 2, 0:nb, 0:1], in_=scrA[:, 0:nb, :],
                                        axis=mybir.AxisListType.X, op=ALU.add)
                nc.vector.tensor_copy(out=g[:, 2, 0:nb, 1:2], in_=g[:, 2, 0:nb, 0:1])
                nc.gpsimd.tensor_mul(out=scrB[:, 0:nb, :], in0=dnn[:, 0:nb, :],
                                     in1=wsn[:, 0:nb, :])
                nc.vector.tensor_reduce(out=g[:, 1, 0:nb, 0:1], in_=scrB[:, 0:nb, :],
                                        axis=mybir.AxisListType.X, op=ALU.add)
                nc.gpsimd.tensor_mul(out=scrA[:, 0:nb, :], in0=dnn[:, 0:nb, :],
                                     in1=wtn[:, 0:nb, :])
                nc.vector.tensor_reduce(out=g[:, 1, 0:nb, 1:2], in_=scrA[:, 0:nb, :],
                                        axis=mybir.AxisListType.X, op=ALU.add)
                yield
                for it in range(3):
                    if it == 0:
                        S, Dr = 0, 1
                    else:
                        # D = B + w*C ; S = A + w*(B + D)
                        nc.vector.tensor_mul(out=g[:, 4, 0:nb, :], in0=g[:, 15, 0:nb, :], in1=g[:, 2, 0:nb, :])
                        nc.vector.tensor_add(out=g[:, 4, 0:nb, :], in0=g[:, 1, 0:nb, :], in1=g[:, 4, 0:nb, :])
                        nc.vector.tensor_add(out=g[:, 5, 0:nb, :], in0=g[:, 1, 0:nb, :], in1=g[:, 4, 0:nb, :])
                        nc.vector.tensor_mul(out=g[:, 5, 0:nb, :], in0=g[:, 15, 0:nb, :], in1=g[:, 5, 0:nb, :])
                        nc.vector.tensor_add(out=g[:, 3, 0:nb, :], in0=g[:, 0, 0:nb, :], in1=g[:, 5, 0:nb, :])
                        S, Dr = 3, 4
                    # squash scale from S: g10 = 0.125*S / ((0.25*S+1)*(0.5*sqrt(S)+1e-9))
                    nc.vector.tensor_scalar_max(out=g[:, 6, 0:nb, :], in0=g[:, S, 0:nb, :],
                                                scalar1=1e-30)
                    emit_rsqrt(nc.vector, g[:, 7, 0:nb, :], g[:, 6, 0:nb, :], g[:, 8, 0:nb, :], niter=1)
                    nc.vector.tensor_mul(out=g[:, 8, 0:nb, :], in0=g[:, 6, 0:nb, :], in1=g[:, 7, 0:nb, :])
                    nc.vector.tensor_scalar(out=g[:, 9, 0:nb, :], in0=g[:, 6, 0:nb, :],
                                            scalar1=0.25, scalar2=1.0,
                                            op0=ALU.mult, op1=ALU.add)
                    nc.vector.tensor_scalar(out=g[:, 8, 0:nb, :], in0=g[:, 8, 0:nb, :],
                                            scalar1=0.5, scalar2=1e-9,
                                            op0=ALU.mult, op1=ALU.add)
                    nc.vector.tensor_mul(out=g[:, 9, 0:nb, :], in0=g[:, 9, 0:nb, :], in1=g[:, 8, 0:nb, :])
                    nc.vector.reciprocal_approx_fast(out=g[:, 13, 0:nb, :], in_=g[:, 9, 0:nb, :])
                    nc.vector.scalar_tensor_tensor(out=g[:, 10, 0:nb, :], in0=g[:, 6, 0:nb, :],
                                                   scalar=0.125, in1=g[:, 13, 0:nb, :],
                                                   op0=ALU.mult, op1=ALU.mult)
                    if it < 2:
                        # logit update u = D*scale ; diff += dsign*u ; w = dsign*tanh(diff/2)
                        nc.vector.tensor_mul(out=g[:, 5, 0:nb, :], in0=g[:, Dr, 0:nb, :], in1=g[:, 10, 0:nb, :])
                        if it == 0:
                            nc.vector.tensor_mul(out=diff[:, 0:nb, :], in0=g[:, 5, 0:nb, :], in1=dsign_sb[:, 0:nb, :])
                        else:
                            nc.vector.tensor_mul(out=g[:, 14, 0:nb, :], in0=g[:, 5, 0:nb, :], in1=dsign_sb[:, 0:nb, :])
                            nc.vector.tensor_add(out=diff[:, 0:nb, :], in0=diff[:, 0:nb, :], in1=g[:, 14, 0:nb, :])
                        nc.scalar.activation(g[:, 14, 0:nb, :], diff[:, 0:nb, :], AF.Tanh, scale=0.5)
                        nc.vector.tensor_mul(out=g[:, 15, 0:nb, :], in0=g[:, 14, 0:nb, :], in1=dsign_sb[:, 0:nb, :])
                        yield
                    else:
                        # vn = (wn + w*dnn)*scale = wn*a + dnn*b, b = w*a
                        nc.vector.tensor_mul(out=g[:, 5, 0:nb, :], in0=g[:, 15, 0:nb, :],
                                             in1=g[:, 10, 0:nb, :])
                        for bi in range(nblk):
                            nc.gpsimd.tensor_scalar(
                                out=scrA[:, bi, :], in0=wsn[:, bi, :],
                                scalar1=g[:, 10, bi, 0:1], scalar2=None,
                                op0=ALU.mult)
                            nc.vector.scalar_tensor_tensor(
                                out=vn[0][:, blk6(bi), :], in0=dnn[:, bi, :],
                                scalar=g[:, 5, bi, 0:1], in1=scrA[:, bi, :],
                                op0=ALU.mult, op1=ALU.add)
                            nc.gpsimd.tensor_scalar(
                                out=scrB[:, bi, :], in0=wtn[:, bi, :],
                                scalar1=g[:, 10, bi, 1:2], scalar2=None,
                                op0=ALU.mult)
                            nc.vector.scalar_tensor_tensor(
                                out=vn[1][:, blk6(bi), :], in0=dnn[:, bi, :],
                                scalar=g[:, 5, bi, 1:2], in1=scrB[:, bi, :],
                                op0=ALU.mult, op1=ALU.add)
                # ---- transpose v to vT (rolling 6-block window) ----
                for prob in range(2):
                    for bi in range(nblk):
                        pst = pspool.tile([128, 2, 128], F32R, tag="E1")
                        for o in range(2):
                            nc.tensor.transpose(pst[:, o, :], vn[prob][:, blk6(bi), o * 128:(o + 1) * 128], eye32[:])
                        m = blk6(bi)
                        if prob == 0:
                            nc.scalar.activation(vT[prob][:, :, m * 128:(m + 1) * 128],
                                                 pst[:], AF.Copy, bias=0.0, scale=1.0)
                        else:
                            nc.vector.tensor_copy(out=vT[prob][:, :, m * 128:(m + 1) * 128], in_=pst[:])
                yield
                # ---- banded time attention ----
                sfT = work.tile([128, 2, CW], F16, tag="bufA")
                tfT = work.tile([128, 2, CW], F16, tag="bufB")
                for prob in range(2):
                    vTt, vnt = vT[prob], vn[prob]
                    dstT = sfT if prob == 0 else tfT
                    q0 = 0
                    while q0 < w:
                        qw = min(256, w - q0)
                        Q0 = l0 + q0
                        mq = ((Q0 // 128) % 6) * 128
                        bbs = [bb for bb in range(1, 4)
                               if Q0 + 128 * (bb - 2) >= seg_start[0]
                               and 128 * (bb - 2) < qw]
                        Pt = wk3.tile([128, 4, 256], F32R, tag="Pbuf")
                        zones = {}
                        for bb in bbs:
                            zones.setdefault(bb // 2, []).append(bb)
                        for z, zbbs in zones.items():
                            base = zbbs[0]
                            Sps = pspool.tile([128, 2, 256], F32, tag="SbigZ",
                                              name=f"Sps_{z}")
                            for bb in zbbs:
                                K0 = Q0 + 128 * (bb - 2)
                                mk = ((K0 // 128) % 6) * 128
                                lo = max(0, 128 * (bb - 2))
                                nc.tensor.matmul(Sps[:, bb - base, 0:qw],
                                                 dbias16[0:1, bb * 128:(bb + 1) * 128],
                                                 ones16[0:1, 0:qw],
                                                 start=(bb == zbbs[0]), stop=False)
                                for o in range(2):
                                    nc.tensor.matmul(Sps[:, bb - base, lo:qw],
                                                     vTt[:, o, mk:mk + 128],
                                                     vTt[:, o, mq + lo:mq + qw],
                                                     start=False,
                                                     stop=(bb == zbbs[-1] and o == 1))
                            for bb in zbbs:
                                if bb >= 2:
                                    dlo = 128 * (bb - 2)
                                    dwdt = min(qw, dlo + 128) - dlo
                                    nc.vector.tensor_add(out=Sps[:, bb - base, dlo:dlo + dwdt],
                                                         in0=Sps[:, bb - base, dlo:dlo + dwdt],
                                                         in1=T32[:, 0:dwdt])
                            nc.scalar.activation(Pt[:, base:base + len(zbbs), 0:qw],
                                                 Sps[:, 0:len(zbbs), 0:qw], AF.Exp)
                        od = psO.tile([128, 3, 256], F32, tag="OD")
                        for bb in bbs:
                            K0 = Q0 + 128 * (bb - 2)
                            kb6 = (K0 // 128) % 6
                            lo = max(0, 128 * (bb - 2))
                            first, last = bb == bbs[0], bb == bbs[-1]
                            for o in range(2):
                                nc.tensor.matmul(od[:, o, lo:qw],
                                                 vnt[:, kb6, o * 128:(o + 1) * 128],
                                                 Pt[:, bb, lo:qw],
                                                 start=(first and o == 0),
                                                 stop=(last and o == 1))
                            nc.tensor.matmul(od[:, 2, lo:qw], ones32[:, 0:128],
                                             Pt[:, bb, lo:qw],
                                             start=first, stop=last)
                        rec = work.tile([128, 256], F32, tag="tmpB")
                        nc.vector.reciprocal_approx_fast(out=rec[:, 0:qw], in_=od[:, 2, 0:qw])
                        for o in range(2):
                            nc.vector.tensor_mul(out=dstT[:, o, q0:q0 + qw],
                                                 in0=od[:, o, 0:qw], in1=rec[:, 0:qw])
                        q0 += qw
                yield
                # ---- fusion + qkv ----
                fused = work.tile([128, 2, CW], F16, tag="bufC")
                for o in range(2):
                    psl = pspool.tile([128, 2, CW], F32, tag="L3", name="ps_fus")
                    ps = psl[:, 0]
                    for k in range(2):
                        nc.tensor.matmul(ps[:, 0:w],
                                         pk[:, C_FWT + (k * 2 + o) * 128:C_FWT + (k * 2 + o) * 128 + 128],
                                         sfT[:, k, 0:w],
                                         start=(k == 0), stop=False)
                        nc.tensor.matmul(ps[:, 0:w],
                                         pk[:, C_FWT + ((2 + k) * 2 + o) * 128:C_FWT + ((2 + k) * 2 + o) * 128 + 128],
                                         tfT[:, k, 0:w],
                                         start=False, stop=(k == 1))
                    nc.scalar.activation(fused[:, o, 0:w], ps[:, 0:w], AF.Identity,
                                         bias=pk32[:, P32_FB + o:P32_FB + o + 1], scale=1.0)
                for o in range(2):
                    psqk = pspool.tile([128, 2, CW], F32, tag="L3", name="psqk")
                    for k in range(2):
                        nc.tensor.matmul(psqk[:, 0, 0:w],
                                         pk[:, C_WQT + (k * 2 + o) * 128:C_WQT + (k * 2 + o) * 128 + 128],
                                         fused[:, k, 0:w],
                                         start=(k == 0), stop=(k == 1))
                        nc.tensor.matmul(psqk[:, 1, 0:w],
                                         pk[:, C_WKT + (k * 2 + o) * 128:C_WKT + (k * 2 + o) * 128 + 128],
                                         fused[:, k, 0:w],
                                         start=(k == 0), stop=(k == 1))
                    nc.scalar.activation(qT_all[:, o, l0:l0 + w], psqk[:, 0, 0:w], AF.Identity,
                                         bias=pk32[:, P32_BQ + o:P32_BQ + o + 1], scale=1.0)
                    nc.scalar.activation(kT_all[:, o, l0:l0 + w], psqk[:, 1, 0:w], AF.Identity,
                                         bias=pk32[:, P32_BK + o:P32_BK + o + 1], scale=1.0)
                for bi in range(nblk):
                    psvl = pspool.tile([128, 2, CW], F32, tag="L3", name="psv")
                    psv = psvl[:, 0, 0:256]
                    for k in range(2):
                        nc.tensor.matmul(psv, fused[:, k, bi * 128:(bi + 1) * 128],
                                         pk[:, C_WVT + k * 256:C_WVT + k * 256 + 256],
                                         start=(k == 0), stop=False)
                    nc.tensor.matmul(psv, ones16[0:1, 0:128], bv_row[0:1, 0:256],
                                     start=False, stop=True)
                    nc.vector.tensor_copy(out=v16[:, l0 // 128 + bi, :], in_=psv)

            def drive(chunks):
                # software-pipelined emission: chunk i+1's conv/LN instruction
                # stream is interleaved into chunk i's routing stream so PE/Act
                # have work while the serial routing chain runs on DVE.
                s1 = emit_s1f(*chunks[0])
                mv = emit_s1b1(*chunks[0], *s1)
                emit_s1b2(chunks[0][0], chunks[0][1], s1[0], *mv)
                for i, c in enumerate(chunks):
                    gen = emit_s2(*c)
                    nxt = chunks[i + 1] if i + 1 < len(chunks) else None
                    if nxt:
                        s1 = emit_s1f(*nxt)
                    next(gen)            # trend taps + transposes
                    if nxt:
                        mv = emit_s1b1(*nxt, *s1)
                    next(gen)            # invariants
                    if nxt:
                        emit_s1b2(nxt[0], nxt[1], s1[0], *mv)
                    next(gen)            # iteration 0
                    next(gen)            # iteration 1
                    for _ in gen:        # it2 + vn + vT, banded, fusion, qkv
                        pass

            def emit_mha(qlo, qhi):
                for Q0 in range(qlo, qhi, 512):
                    qw = min(512, qhi - Q0)
                    nkv = (Q0 + qw) // 128
                    obuf = work.tile([128, 4, 256], F32, tag="obuf", name="obuf")
                    for hp in range(4):
                        hg = hp // 2
                        rows = [32 * ((2 * hp) % 4), 32 * ((2 * hp + 1) % 4)]
                        Oh = psO.tile([32, 2, 512], F32, tag="OD", name=f"Oh_{hp}")
                        dh = psO.tile([32, 2, 512], F32, tag="E2", name=f"dh_{hp}")
                        sps = pspool.tile([128, 2, 512], F32, tag="L3", name="sps")
                        sps_z = pspool.tile([128, 512], F32, tag="SbigZ", name="sps_z")
                        slots = [sps[:, 0], sps[:, 1], sps_z[:]]
                        step = 0
                        pend = None

                        def flush(p):
                            kb_, jj_, Pm_, lo_, fi_, la_ = p
                            h_ = 2 * hp + jj_
                            nc.tensor.matmul(Oh[:, jj_, lo_:qw],
                                             v_all[:, kb_, h_ * 32:h_ * 32 + 32],
                                             Pm_[:, lo_:qw], start=fi_, stop=la_)
                            nc.tensor.matmul(dh[:, jj_, lo_:qw],
                                             ones32[:, 0:32],
                                             Pm_[:, lo_:qw], start=fi_, stop=la_)

                        for kb in range(nkv):
                            K0 = kb * 128
                            dlt = K0 - Q0
                            lo = max(0, dlt)
                            dwdt = min(qw, dlt + 128) - dlt if dlt >= 0 else 0
                            first, last = kb == 0, kb == nkv - 1
                            for jj in range(2):
                                sp = slots[step % len(slots)]
                                step += 1
                                nc.tensor.matmul(sp[:, lo:qw],
                                                 kT_all[rows[jj]:rows[jj] + 32, hg, K0:K0 + 128],
                                                 qT_all[rows[jj]:rows[jj] + 32, hg, Q0 + lo:Q0 + qw],
                                                 start=True, stop=True,
                                                 tile_position=(rows[jj], 0))
                                if dlt >= 0:
                                    nc.vector.tensor_add(out=sp[:, dlt:dlt + dwdt],
                                                         in0=sp[:, dlt:dlt + dwdt],
                                                         in1=T32[:, 0:dwdt])
                                Pm = wk3.tile([128, 512], F32R, tag="Pbuf", name="Pm")
                                nc.scalar.activation(Pm[:, lo:qw], sp[:, lo:qw], AF.Exp)
                                if pend is not None:
                                    flush(pend)
                                pend = (kb, jj, Pm, lo, first, last)
                        flush(pend)
                        rec = work.tile([32, 2, 512], F32, tag="bufB", name="rec_m")
                        nc.vector.reciprocal_approx_fast(out=rec[:, :, 0:qw], in_=dh[:, :, 0:qw])
                        Ohn = work.tile([32, 2, 512], F16, tag="bufA", name="Ohn")
                        nc.vector.tensor_mul(out=Ohn[:, :, 0:qw], in0=Oh[:, :, 0:qw], in1=rec[:, :, 0:qw])
                        if Q0 == 0:
                            nc.vector.tensor_copy(out=Ohn[:, :, 0:1],
                                                  in_=zeros8[0:32, 0:2].unsqueeze(-1))
                        for bi in range(qw // 128):
                            psop = pspool.tile([128, 256], F32, tag="E1", name="psop")
                            for jj in range(2):
                                nc.tensor.matmul(psop[:], Ohn[:, jj, bi * 128:(bi + 1) * 128],
                                                 woT_sb[0:32, 2 * hp + jj, :],
                                                 start=(jj == 0), stop=(jj == 1 and hp != 0))
                            if hp == 0:
                                nc.tensor.matmul(psop[:], ones16[0:1, 0:128], bo_row[0:1, 0:256],
                                                 start=False, stop=True)
                                nc.vector.tensor_copy(out=obuf[:, bi, :], in_=psop[:])
                            else:
                                nc.vector.tensor_add(out=obuf[:, bi, :], in0=obuf[:, bi, :], in1=psop[:])
                    for bi in range(qw // 128):
                        ot = work.tile([128, 256], F16, tag="tmpC", name="ot16")
                        nc.vector.tensor_copy(out=ot[:], in_=obuf[:, bi, :])
                        nc.sync.dma_start(outd[Q0 - qlo + bi * 128:Q0 - qlo + (bi + 1) * 128, :], ot[:])

            seg_start = [0]

            def emit_pipeline(vi):
                # position-local pipeline over [seg0, hi): own slab + one
                # 512-wide halo chunk (band reach 256 + conv/trend taps);
                # pack this slab's K and V for the cross-core AllGather.
                lo, hi = ABOUNDS[vi], ABOUNDS[vi + 1]
                seg0 = max(0, lo - 512)
                seg_start[0] = seg0
                chunks = []
                l0 = seg0
                while l0 < hi:
                    w = min(CW, hi - l0)
                    chunks.append((l0, w))
                    l0 += w
                drive(chunks)
                if force_variant is None:
                    nc.sync.dma_start(ag2_in[:, 0:1024], kT_all[:, :, lo:hi])
                    nc.sync.dma_start(ag2_in[:, 1024:2048],
                                      v16[:, 4 * vi:4 * vi + 4, :])

            def emit_variant(vi):
                # single-core path for TimelineSim: no collectives
                emit_pipeline(vi)
                nc.vector.tensor_copy(out=v_all[:], in_=v16[:])
                emit_mha(ABOUNDS[vi], ABOUNDS[vi + 1])

            if force_variant is not None:
                emit_variant(force_variant)
            else:
                with tc.If(pid < 2) as c0:
                    emit_pipeline(0)
                with c0.Else():
                    with tc.If(pid < 4) as c1:
                        emit_pipeline(1)
                    with c1.Else():
                        with tc.If(pid < 6) as c2:
                            emit_pipeline(2)
                        with c2.Else():
                            emit_pipeline(3)

                # cross-core K/V AllGather (top level: no control flow)
                nc.gpsimd.collective_compute(
                    "AllGather", ALU.bypass,
                    replica_groups=[list(range(8))],
                    ins=[ag2_in[:]], outs=[ag2_out[:]])

                # unpack the 4 slabs of my batch (batch = pid & 1)
                def unpack(b):
                    for c in (b, b + 2, b + 4, b + 6):
                        w0 = 512 * (c // 2)
                        nc.sync.dma_start(kT_all[:, :, w0:w0 + 512],
                                          ag2_out[c, :, 0:1024])
                        nc.sync.dma_start(v16[:, w0 // 128:w0 // 128 + 4, :],
                                          ag2_out[c, :, 1024:2048])

                def up_tree(lo_pid, hi_pid):
                    if hi_pid - lo_pid == 1:
                        unpack(lo_pid & 1)
                        return
                    mid = (lo_pid + hi_pid) // 2
                    with tc.If(pid < mid) as cc:
                        up_tree(lo_pid, mid)
                    with cc.Else():
                        up_tree(mid, hi_pid)

                up_tree(0, 8)
                nc.vector.tensor_copy(out=v_all[:], in_=v16[:])

                with tc.If(pid < 2) as d0:
                    emit_mha(ABOUNDS[0], ABOUNDS[1])
                with d0.Else():
                    with tc.If(pid < 4) as d1:
                        emit_mha(ABOUNDS[1], ABOUNDS[2])
                    with d1.Else():
                        with tc.If(pid < 6) as d2:
                            emit_mha(ABOUNDS[2], ABOUNDS[3])
                        with d2.Else():
                            emit_mha(ABOUNDS[3], ABOUNDS[4])
    nc.finalize()
    return nc


_CACHE = {}


def kernel(**inputs):
    from concourse.bass_utils import run_bass_kernel_spmd
    in_maps = build_in_maps(inputs)
    if "nc" not in _CACHE:
        _CACHE["nc"] = _build()
    nc = _CACHE["nc"]
    res = run_bass_kernel_spmd(nc, in_maps, core_ids=list(range(8)))
    out = np.zeros((B, L, D), np.float32)
    for core in range(8):
        b = core & 1
        vi = core // 2
        lo, hi = ABOUNDS[vi], ABOUNDS[vi + 1]
        out[b, lo:hi, :] = res.results[core]["out"].astype(np.float32)
    return out


# revision 25
# speedup vs baseline: 1.0511x; 1.0511x over previous
# Trainium2 Bass kernel for nn_BAKTTime (dense_transformer).
# Self-contained: hardcodes shapes B=2, L=2048, D=256, H=8, dk=32.
#
# Sharding: 8 cores, SPMD program. core i handles batch (i & 1) and query
# slab (i // 2); slab j computes the position-local pipeline (folded
# 5-tap conv + layernorm + capsule routing + banded time attention + fusion
# + qkv) over the causal prefix [0, 512*(j+1)), then flash MHA over q in
# [512*j, 512*(j+1)).  The wall-clock of this problem is dominated by
# host<->device transfer over the axon tunnel, so all inputs are packed
# into one fp16 [128, C] tensor per core; with USE_AG each core ships only
# a 1/8 column slice and an on-device AllGather reconstructs the full
# pack.  Constant matrices (identity, causal mask, decay bias, bias rows,
# depthwise-diag) are built on device.  Output is a per-core fp16
# (512, 256) slab; the host stitches the 8 slabs.
import numpy as np

B, L, D = 2, 2048, 256
H, DK = 8, 32
DECAY = 0.2
EPS_LN = 1e-12
NEG = -1e30
CW = 512
ABOUNDS = (0, 512, 1024, 1536, 2048)
SLAB = 512

USE_AG = True  # AllGather weights+x on device (ship 1/8 per core)

# ---- fp16 pack column layout (single source of truth) ----
C_WTT = 0                      # [2(k),5(t),2(o),128]
C_FWT = C_WTT + 2560           # [4(k),2(o),128]
C_WQT = C_FWT + 1024           # [2(k),2(o),128]
C_WKT = C_WQT + 512
C_WVT = C_WKT + 512            # [2(k),256]
C_WO = C_WVT + 512             # head h at partitions [32*(h%4),+32), col (h//4)*256
C_EYE = C_WO + 512             # [128,128] identity
C_DWC = C_EYE + 128            # [2(pd),5(t),2(o)] depthwise tap coefs
C_IOTA = C_DWC + 20            # [128,1] iota
C_BVC = C_IOTA + 1             # [2] bv col layout
C_BOC = C_BVC + 2              # [2] bo col layout
C_P32HI = C_BOC + 2            # [33] fp16 hi half of the f32 pack
C_P32LO = C_P32HI + 33         # [33] fp16 lo half (v - f32(hi))
W_COLS = 5888                  # C_P32LO+33 = 5851, padded to 5888
X_COLS = 2 * (4 + L)           # 4104
PK_COLS = W_COLS + X_COLS      # 9896 (per-core pack: weights + my batch xT)
AG_COLS = W_COLS + 2 * X_COLS  # 14000 (global pack: weights + both batches)
AG_SL = AG_COLS // 8           # 1750

# pack32 f32 [128, 33]
P32_BEFF, P32_LNW, P32_LNB, P32_FB, P32_BQ, P32_BK, P32_IOTA, P32_DWC = 0, 2, 4, 6, 8, 10, 12, 13
P32_COLS = 33


def _host_prep(inp):
    f32, f16 = np.float32, np.float16
    x = np.asarray(inp["x"], f32)
    c3w, c3b = np.asarray(inp["conv3_w"], f32), np.asarray(inp["conv3_b"], f32)
    c5w, c5b = np.asarray(inp["conv5_w"], f32), np.asarray(inp["conv5_b"], f32)
    b3 = np.asarray(inp["beta3"], f32).reshape(D)
    b5 = np.asarray(inp["beta5"], f32).reshape(D)
    cw = np.asarray(inp["combine_w"], f32)
    cwt = np.exp(cw - cw.max())
    cwt = (cwt / cwt.sum()).astype(f32)
    g3 = (cwt[0] * (1.0 - b3 ** 2)).astype(f32)
    g5 = (cwt[1] * (1.0 - b5 ** 2)).astype(f32)
    dscale = (1.0 + cwt[0] * b3 ** 2 + cwt[1] * b5 ** 2).astype(f32)
    W = np.zeros((5, D, D), f32)
    W[0] = g3[:, None] * c3w[:, :, 2] + g5[:, None] * c5w[:, :, 4] + np.diag(dscale)
    W[1] = g3[:, None] * c3w[:, :, 1] + g5[:, None] * c5w[:, :, 3]
    W[2] = g3[:, None] * c3w[:, :, 0] + g5[:, None] * c5w[:, :, 2]
    W[3] = g5[:, None] * c5w[:, :, 1]
    W[4] = g5[:, None] * c5w[:, :, 0]
    # lhsT layout [din_par, din_ch(k), tap, o_ch, dout]
    wtT = np.transpose(W, (0, 2, 1)).reshape(5, 2, 128, 2, 128)
    wtT = np.ascontiguousarray(np.transpose(wtT, (2, 1, 0, 3, 4)))  # [128,2,5,2,128]
    beff = (g3 * c3b + g5 * c5b).reshape(2, 128).T.copy()           # [128, 2]
    dw3 = np.asarray(inp["dw3_w"], f32)[:, 0, :]
    dw5 = np.asarray(inp["dw5_w"], f32)[:, 0, :]
    c3l = np.zeros((5, D), f32)
    c5l = np.zeros((5, D), f32)
    for l in range(3):
        c3l[l] = dw3[:, 2 - l]
    for l in range(5):
        c5l[l] = dw5[:, 4 - l]
    pco, dco = c3l + c5l, c3l - c5l                                  # [5, 256]
    # dwc [128, 2(pd), 5(t), 2(o)]
    dwc = np.zeros((128, 2, 5, 2), f32)
    for t in range(5):
        for o in range(2):
            dwc[:, 0, t, o] = pco[t, o * 128:(o + 1) * 128]
            dwc[:, 1, t, o] = dco[t, o * 128:(o + 1) * 128]
    col = lambda v: np.asarray(v, f32).reshape(2, 128).T.copy()      # [128, 2]
    fwT = np.asarray(inp["fusion_w"], f32).T.reshape(4, 128, 2, 128)
    fwT = np.ascontiguousarray(np.transpose(fwT, (1, 0, 2, 3)))      # [128,4,2,128]
    s = 1.0 / np.sqrt(DK)

    def sqT(w):
        t = np.asarray(w, f32).T.reshape(2, 128, 2, 128)
        return np.ascontiguousarray(np.transpose(t, (1, 0, 2, 3)))   # [128,2,2,128]

    def hvT(w):
        t = np.asarray(w, f32).T.reshape(2, 128, 256)
        return np.ascontiguousarray(np.transpose(t, (1, 0, 2)))      # [128,2,256]

    # wo pack [128, 512]: head h tile (32,256) at partitions [32*(h%4),+32),
    # cols [(h//4)*256,+256)
    woT = np.asarray(inp["wo"], f32).T.reshape(8, 32, 256).transpose(1, 0, 2)  # [32,8,256]
    wop = np.zeros((128, 512), f32)
    for h in range(8):
        wop[32 * (h % 4):32 * (h % 4) + 32, (h // 4) * 256:(h // 4) * 256 + 256] = woT[:, h, :]

    Wpk = np.zeros((128, W_COLS), f32)
    Wpk[:, C_WTT:C_WTT + 2560] = wtT.reshape(128, -1)
    Wpk[:, C_FWT:C_FWT + 1024] = fwT.reshape(128, -1)
    Wpk[:, C_WQT:C_WQT + 512] = sqT(np.asarray(inp["wq"], f32) * s).reshape(128, -1)
    Wpk[:, C_WKT:C_WKT + 512] = sqT(inp["wk"]).reshape(128, -1)
    Wpk[:, C_WVT:C_WVT + 512] = hvT(inp["wv"]).reshape(128, -1)
    Wpk[:, C_WO:C_WO + 512] = wop
    Wpk[:, C_EYE:C_EYE + 128] = np.eye(128, dtype=f32)
    Wpk[:, C_DWC:C_DWC + 20] = dwc.reshape(128, -1)
    Wpk[:, C_IOTA:C_IOTA + 1] = np.arange(128, dtype=f32)[:, None]
    Wpk[:, C_BVC:C_BVC + 2] = col(inp["bv"])
    Wpk[:, C_BOC:C_BOC + 2] = col(inp["bo"])

    # xT [B, 128, 2, 4+L]: 4 leading zero cols per o-half for the conv halo
    xT = np.zeros((B, 128, 2, 4 + L), f32)
    xt_full = np.transpose(x, (0, 2, 1)).reshape(B, 2, 128, L)
    xT[:, :, :, 4:] = np.transpose(xt_full, (0, 2, 1, 3))
    xT16 = xT.reshape(B, 128, X_COLS).astype(f16)

    pk32 = np.zeros((128, P32_COLS), f32)
    pk32[:, P32_BEFF:P32_BEFF + 2] = beff
    pk32[:, P32_LNW:P32_LNW + 2] = col(inp["ln_w"])
    pk32[:, P32_LNB:P32_LNB + 2] = col(inp["ln_b"])
    pk32[:, P32_FB:P32_FB + 2] = col(inp["fusion_b"])
    pk32[:, P32_BQ:P32_BQ + 2] = col(np.asarray(inp["bq"], f32) * s)
    pk32[:, P32_BK:P32_BK + 2] = col(inp["bk"])
    pk32[:, P32_IOTA:P32_IOTA + 1] = np.arange(128, dtype=f32)[:, None]
    pk32[:, P32_DWC:P32_DWC + 20] = dwc.reshape(128, -1)
    hi = pk32.astype(f16)
    lo = (pk32 - hi.astype(f32)).astype(f16)
    Wpk[:, C_P32HI:C_P32HI + P32_COLS] = hi.astype(f32)
    Wpk[:, C_P32LO:C_P32LO + P32_COLS] = lo.astype(f32)
    Wpk16 = Wpk.astype(f16)
    return Wpk16, xT16


def build_in_maps(inputs):
    Wpk16, xT16 = _host_prep(inputs)
    in_maps = []
    if USE_AG:
        gpack = np.concatenate([Wpk16, xT16[0], xT16[1]], axis=1)  # [128, AG_COLS]
        for core in range(8):
            in_maps.append(dict(
                pksl=np.ascontiguousarray(gpack[:, AG_SL * core:AG_SL * (core + 1)])))
    else:
        pk_b = [np.ascontiguousarray(np.concatenate([Wpk16, xT16[b]], axis=1))
                for b in range(B)]
        for core in range(8):
            in_maps.append(dict(pk16=pk_b[core & 1]))
    return in_maps


def _build(force_variant=None, use_ag=None):
    import concourse.mybir as mybir
    import concourse.tile as tile
    from concourse import bacc

    F32, F32R, F16 = mybir.dt.float32, mybir.dt.float32r, mybir.dt.float16
    U32 = mybir.dt.uint32
    AF = mybir.ActivationFunctionType
    ALU = mybir.AluOpType
    # 2*0x5f3759df + 1 as signed int32, for rsqrt seed (C2 + ~i) >> 1
    RSQRT_C2 = 0xBE6EB3BF - (1 << 32)

    use_ag = USE_AG if use_ag is None else use_ag

    nc = bacc.Bacc(num_devices=8) if use_ag else bacc.Bacc()

    if use_ag:
        pksl_d = nc.dram_tensor("pksl", [128, AG_SL], F16, kind="ExternalInput")
        ag_in = nc.dram_tensor("ag_in", [128, AG_SL], F16)
        ag_out = nc.dram_tensor("ag_out", [8, 128, AG_SL], F16, addr_space="Shared")
    else:
        pk16_d = nc.dram_tensor("pk16", [128, PK_COLS], F16, kind="ExternalInput")
    ag2_in = nc.dram_tensor("ag2_in", [128, 2048], F16)
    ag2_out = nc.dram_tensor("ag2_out", [8, 128, 2048], F16, addr_space="Shared")
    outd = nc.dram_tensor("out", [SLAB, D], F16, kind="ExternalOutput")

    X0 = W_COLS  # my-batch xT offset within pk

    with tile.TileContext(nc) as tc:
        pid = nc.partition_id() if force_variant is None else None
        with tc.tile_pool(name="wpool", bufs=1) as wpool, \
             tc.tile_pool(name="ppool", bufs=1) as ppool, \
             tc.tile_pool(name="work", bufs=1) as work, \
             tc.tile_pool(name="wk3", bufs=4) as wk3, \
             tc.tile_pool(name="pspool", bufs=1, space="PSUM") as pspool:
            psO = pspool

            pk = wpool.tile([128, PK_COLS], F16, name="pk")
            pk32 = wpool.tile([128, P32_COLS], F32, name="pk32s")
            if use_ag:
                nc.sync.dma_start(ag_in[:], pksl_d[:])
                nc.gpsimd.collective_compute(
                    "AllGather", ALU.bypass,
                    replica_groups=[list(range(8))],
                    ins=[ag_in[:]], outs=[ag_out[:]])

                def load_cols(dst_c0, g_c0, g_c1):
                    # copy global pack cols [g_c0,g_c1) into pk[:, dst_c0...]
                    for blk in range(8):
                        b0, b1 = blk * AG_SL, (blk + 1) * AG_SL
                        lo, hi = max(g_c0, b0), min(g_c1, b1)
                        if lo < hi:
                            nc.sync.dma_start(
                                pk[:, dst_c0 + lo - g_c0:dst_c0 + hi - g_c0],
                                ag_out[blk, :, lo - b0:hi - b0])

                load_cols(0, 0, W_COLS)

                # my batch's xT: binary branch tree on pid (batch = pid & 1)
                def xt_tree(lo_pid, hi_pid):
                    if hi_pid - lo_pid == 1:
                        bsel = lo_pid & 1
                        load_cols(X0, W_COLS + bsel * X_COLS,
                                  W_COLS + (bsel + 1) * X_COLS)
                        return
                    mid = (lo_pid + hi_pid) // 2
                    with tc.If(pid < mid) as cc:
                        xt_tree(lo_pid, mid)
                    with cc.Else():
                        xt_tree(mid, hi_pid)

                xt_tree(0, 8)
            else:
                nc.sync.dma_start(pk[:], pk16_d[:])

            # reconstruct the f32 side-pack from fp16 hi/lo halves
            p32lo = wpool.tile([128, P32_COLS], F32, name="p32lo")
            nc.vector.tensor_copy(out=pk32[:], in_=pk[:, C_P32HI:C_P32HI + P32_COLS])
            nc.vector.tensor_copy(out=p32lo[:], in_=pk[:, C_P32LO:C_P32LO + P32_COLS])
            nc.vector.tensor_add(out=pk32[:], in0=pk32[:], in1=p32lo[:])

            # ---- on-device constants ----
            ones32 = wpool.tile([128, 512], F32R, name="ones32")
            nc.vector.memset(ones32[:].bitcast(F32), 1.0)
            ones16 = wpool.tile([128, 512], F16, name="ones16")
            nc.vector.tensor_copy(out=ones16[:], in_=ones32[:])
            zeros8 = wpool.tile([128, 8], F32, name="zeros8")
            nc.vector.memset(zeros8[:], 0.0)
            eps_sb = wpool.tile([128, 1], F32, name="eps_sb")
            nc.vector.memset(eps_sb[:], EPS_LN)
            dsign_sb = wpool.tile([128, 4, 2], F32, name="dsign_sb")
            nc.vector.memset(dsign_sb[:, :, 0:1], -1.0)
            nc.vector.memset(dsign_sb[:, :, 1:2], 1.0)

            eye16 = pk[:, C_EYE:C_EYE + 128]
            eye32 = wpool.tile([128, 128], F32R, name="eye32")
            nc.vector.tensor_copy(out=eye32[:], in_=eye16)

            # iota row via M=1 matmul: out[0,j] = sum_k iota[k]*eye[k,j]
            rowp = pspool.tile([128, 128], F32, tag="E1", name="rowp")
            nc.tensor.matmul(rowp[0:1, 0:128], pk[:, C_IOTA:C_IOTA + 1], eye16,
                             start=True, stop=True)
            iota_r16 = wpool.tile([1, 128], F16, name="iota_r16")
            nc.vector.tensor_copy(out=iota_r16[:], in_=rowp[0:1, 0:128])
            iota_r32 = wpool.tile([1, 128], F32, name="iota_r32")
            nc.vector.tensor_copy(out=iota_r32[:], in_=rowp[0:1, 0:128])

            # decay bias row [1, 512]: dbias[bb*128+i] = DECAY*(i + 128*(bb-2))
            dbias16 = wpool.tile([1, 512], F16, name="dbias16")
            for bb in range(4):
                nc.vector.tensor_scalar(
                    out=dbias16[0:1, bb * 128:(bb + 1) * 128], in0=iota_r32[:],
                    scalar1=DECAY, scalar2=DECAY * 128.0 * (bb - 2),
                    op0=ALU.mult, op1=ALU.add)

            # bias rows [1, 256] from col layout via M=1 matmuls
            bv_row = wpool.tile([1, 256], F16, name="bv_row")
            bo_row = wpool.tile([1, 256], F16, name="bo_row")
            for dst, c0 in ((bv_row, C_BVC), (bo_row, C_BOC)):
                for o in range(2):
                    rp = pspool.tile([128, 128], F32, tag="E1", name="rowp2")
                    nc.tensor.matmul(rp[0:1, 0:128], pk[:, c0 + o:c0 + o + 1],
                                     eye16, start=True, stop=True)
                    nc.vector.tensor_copy(out=dst[0:1, o * 128:(o + 1) * 128],
                                          in_=rp[0:1, 0:128])

            # causal band mask T32[r, c] = NEG where c < r else 0
            Jps = pspool.tile([128, 128], F32, tag="E2", name="Jps")
            nc.tensor.matmul(Jps[:, 0:128], ones16[0:1, 0:128], iota_r16[0:1, 0:128],
                             start=True, stop=True)
            T32 = wpool.tile([128, 128], F32, name="T32")
            nc.vector.tensor_scalar(out=T32[:], in0=Jps[:, 0:128],
                                    scalar1=pk32[:, P32_IOTA:P32_IOTA + 1],
                                    scalar2=NEG, op0=ALU.is_lt, op1=ALU.mult)


            # wo tiles at partitions 0-31: [32, 8, 256]
            woT_sb = wpool.tile([32, 8, 256], F16, name="woT_sb")
            for h in range(8):
                nc.sync.dma_start(
                    woT_sb[0:32, h, :],
                    pk[32 * (h % 4):32 * (h % 4) + 32,
                       C_WO + (h // 4) * 256:C_WO + (h // 4) * 256 + 256])

            hT32 = ppool.tile([128, 2, 2, 4 + CW], F32, name="hT32")
            vT_s = ppool.tile([128, 2, 6 * 128], F16, name="vT_s")
            vT_t = ppool.tile([128, 2, 6 * 128], F16, name="vT_t")
            vn_s = ppool.tile([128, 6, 256], F32R, name="vn_s")
            vn_t = ppool.tile([128, 6, 256], F32R, name="vn_t")
            v_all = ppool.tile([128, L // 128, 8, 36], F32R, name="v_all")
            nc.vector.memset(v_all[:, :, :, 32:33].bitcast(F32), 1.0)
            v16 = ppool.tile([128, L // 128, 256], F16, name="v16")
            qT_all = ppool.tile([128, 2, L], F16, name="qT_all")
            kT_all = ppool.tile([128, 2, L], F16, name="kT_all")
            for _o in range(2):
                for _p in range(2):
                    nc.vector.memset(hT32[:, _o, _p, 0:4], 0.0)
            vT = {0: vT_s, 1: vT_t}
            vn = {0: vn_s, 1: vn_t}

            def emit_rsqrt(eng, y, x, tmp, niter=2):
                # y <- 1/sqrt(x) elementwise; x must be > 0 (pre-clamped).
                yi, xi, ti = y.bitcast(U32), x.bitcast(U32), tmp.bitcast(U32)
                nc.vector.tensor_scalar(out=ti, in0=xi, scalar1=0, scalar2=None,
                                        op0=ALU.bitwise_not)
                nc.vector.tensor_scalar(out=ti, in0=ti, scalar1=RSQRT_C2,
                                        scalar2=None, op0=ALU.add)
                nc.vector.tensor_scalar(out=yi, in0=ti, scalar1=1, scalar2=None,
                                        op0=ALU.logical_shift_right)
                for _ in range(niter):
                    eng.tensor_mul(out=tmp, in0=y, in1=y)
                    eng.tensor_mul(out=tmp, in0=tmp, in1=x)
                    eng.tensor_scalar(out=tmp, in0=tmp, scalar1=-0.5,
                                      scalar2=1.5, op0=ALU.mult, op1=ALU.add)
                    eng.tensor_mul(out=y, in0=y, in1=tmp)

            def emit_s1f(l0, w):
                # folded conv + y/sq activations (x read from resident pack)
                y = work.tile([128, 2, CW], F32R, tag="y_sb")
                sq = work.tile([128, 2, CW], F16, tag="sq_sb")
                for o in range(2):
                    ps = pspool.tile([128, CW], F32, tag="E1")
                    for t in range(5):
                        for k in range(2):
                            xc = X0 + k * (4 + L) + l0 + 4 - t
                            nc.tensor.matmul(ps[:, 0:w],
                                             pk[:, C_WTT + ((k * 5 + t) * 2 + o) * 128:
                                                C_WTT + ((k * 5 + t) * 2 + o) * 128 + 128],
                                             pk[:, xc:xc + w],
                                             start=(t == 0 and k == 0),
                                             stop=(t == 4 and k == 1))
                    nc.scalar.activation(y[:, o, 0:w], ps[:, 0:w], AF.Identity,
                                         bias=pk32[:, P32_BEFF + o:P32_BEFF + o + 1],
                                         scale=1.0)
                    nc.scalar.activation(sq[:, o, 0:w], ps[:, 0:w], AF.Square,
                                         bias=pk32[:, P32_BEFF + o:P32_BEFF + o + 1],
                                         scale=1.0)
                return y, sq

            def emit_s1b1(l0, w, y, sq):
                # layernorm moments via replicated-moment matmuls
                mps = pspool.tile([128, 2, CW], F32, tag="E2")
                for o in range(2):
                    nc.tensor.matmul(mps[:, 0, 0:w], ones32[:, 0:128], y[:, o, 0:w],
                                     start=(o == 0), stop=(o == 1))
                    nc.tensor.matmul(mps[:, 1, 0:w], ones16[:, 0:128], sq[:, o, 0:w],
                                     start=(o == 0), stop=(o == 1))
                mu = work.tile([128, CW], F32, tag="mu")
                nc.vector.tensor_scalar_mul(out=mu[:, 0:w], in0=mps[:, 0, 0:w], scalar1=1.0 / D)
                mu2 = work.tile([128, CW], F32, tag="tmpA")
                nc.scalar.activation(mu2[:, 0:w], mu[:, 0:w], AF.Square)
                var = work.tile([128, CW], F32, tag="tmpB")
                nc.vector.scalar_tensor_tensor(out=var[:, 0:w], in0=mps[:, 1, 0:w],
                                               scalar=1.0 / D, in1=mu2[:, 0:w],
                                               op0=ALU.mult, op1=ALU.subtract)
                return mu, var

            def emit_s1b2(l0, w, y, mu, var):
                par = (l0 // CW) & 1
                c0 = l0 % CW
                lnv = work.tile([128, CW], F32, tag="tmpA")
                nc.scalar.activation(lnv[:, 0:w], var[:, 0:w], AF.Ln, bias=eps_sb[:])
                rstd = work.tile([128, CW], F32, tag="tmpB")
                nc.scalar.activation(rstd[:, 0:w], lnv[:, 0:w], AF.Exp, scale=-0.5)
                for o in range(2):
                    t1 = work.tile([128, CW], F32, tag="tmpA" if o else "tmpC")
                    nc.vector.tensor_sub(out=t1[:, 0:w], in0=y[:, o, 0:w], in1=mu[:, 0:w])
                    nc.vector.tensor_mul(out=t1[:, 0:w], in0=t1[:, 0:w], in1=rstd[:, 0:w])
                    nc.vector.tensor_scalar(out=hT32[:, o, par, 4 + c0:4 + c0 + w],
                                            in0=t1[:, 0:w],
                                            scalar1=pk32[:, P32_LNW + o:P32_LNW + o + 1],
                                            scalar2=pk32[:, P32_LNB + o:P32_LNB + o + 1],
                                            op0=ALU.mult, op1=ALU.add)
                if c0 + w == CW:
                    nc.vector.tensor_copy(out=hT32[:, :, 1 - par, 0:4],
                                          in_=hT32[:, :, par, CW:CW + 4])

            def emit_s2(l0, w):
                par = (l0 // CW) & 1
                c0 = l0 % CW
                nblk = w // 128
                blk6 = lambda b: (l0 // 128 + b) % 6
                # ---- trend taps: P = t3+t5, Dt = t3-t5 (per-channel shift
                # chains on DVE/GpSimd; beats diag matmuls in this runtime) ----
                wsT = work.tile([128, 2, CW], F32R, tag="bufA")
                wtTt = work.tile([128, 2, CW], F32R, tag="bufB")
                dnT = work.tile([128, 2, CW], F32R, tag="bufC")
                for o in range(2):
                    for t in range(5):
                        src = hT32[:, o, par, 4 + c0 - t:4 + c0 - t + w]
                        cP = pk32[:, P32_DWC + t * 2 + o:P32_DWC + t * 2 + o + 1]
                        cD = pk32[:, P32_DWC + 10 + t * 2 + o:P32_DWC + 10 + t * 2 + o + 1]
                        if t == 0:
                            nc.vector.tensor_scalar(out=wtTt[:, o, 0:w], in0=src,
                                                    scalar1=cP, scalar2=None,
                                                    op0=ALU.mult)
                            nc.vector.tensor_scalar(out=dnT[:, o, 0:w], in0=src,
                                                    scalar1=cD, scalar2=None,
                                                    op0=ALU.mult)
                        else:
                            nc.vector.scalar_tensor_tensor(
                                out=wtTt[:, o, 0:w], in0=src, scalar=cP,
                                in1=wtTt[:, o, 0:w], op0=ALU.mult, op1=ALU.add)
                            nc.vector.scalar_tensor_tensor(
                                out=dnT[:, o, 0:w], in0=src, scalar=cD,
                                in1=dnT[:, o, 0:w], op0=ALU.mult, op1=ALU.add)
                    nc.vector.scalar_tensor_tensor(out=wsT[:, o, 0:w],
                                                   in0=hT32[:, o, par, 4 + c0:4 + c0 + w],
                                                   scalar=2.0, in1=wtTt[:, o, 0:w],
                                                   op0=ALU.mult, op1=ALU.subtract)
                # ---- transpose routing inputs to [l, c] ----
                wsn = work.tile([128, 4, 256], F32, tag="wsn")
                wtn = work.tile([128, 4, 256], F32, tag="wtn")
                dnn = work.tile([128, 4, 256], F32, tag="dnn")
                for srct, dst, use_act in ((wsT, wsn, False), (wtTt, wtn, True),
                                           (dnT, dnn, True)):
                    for bi in range(nblk):
                        pst = pspool.tile([128, 2, 128], F32R, tag="E1")
                        for o in range(2):
                            nc.tensor.transpose(pst[:, o, :], srct[:, o, bi * 128:(bi + 1) * 128], eye32[:])
                        if use_act:
                            nc.scalar.activation(dst[:, bi, :], pst[:],
                                                 AF.Copy, bias=0.0, scale=1.0)
                        else:
                            nc.vector.tensor_copy(out=dst[:, bi, :], in_=pst[:])
                yield
                # ---- routing invariants (st = wn + w*dnn):
                #   A_p = sum wn_p^2, B_p = sum dnn*wn_p, C = sum dnn^2
                # then per-iteration sums are analytic:
                #   S(w) = A + w*(B + D(w)),  D(w) = B + w*C.
                g = work.tile([128, 16, 4, 2], F32, tag="g")
                diff = work.tile([128, 4, 2], F32, tag="diff")
                scrA = work.tile([128, 4, 256], F32, tag="bufB", name="scrA")
                scrB = work.tile([128, 4, 256], F32, tag="bufA", name="scrB")
                nb = nblk
                nc.scalar.activation(scrA[:, 0:nb, :], wsn[:, 0:nb, :], AF.Square)
                nc.vector.tensor_reduce(out=g[:, 0, 0:nb, 0:1], in_=scrA[:, 0:nb, :],
                                        axis=mybir.AxisListType.X, op=ALU.add)
                nc.scalar.activation(scrB[:, 0:nb, :], wtn[:, 0:nb, :], AF.Square)
                nc.vector.tensor_reduce(out=g[:, 0, 0:nb, 1:2], in_=scrB[:, 0:nb, :],
                                        axis=mybir.AxisListType.X, op=ALU.add)
                nc.gpsimd.tensor_mul(out=scrA[:, 0:nb, :], in0=dnn[:, 0:nb, :],
                                     in1=dnn[:, 0:nb, :])
                nc.vector.tensor_reduce(out=g[:, 2, 0:nb, 0:1], in_=scrA[:, 0:nb, :],
                                        axis=mybir.AxisListType.X, op=ALU.add)
                nc.vector.tensor_copy(out=g[:, 2, 0:nb, 1:2], in_=g[:, 2, 0:nb, 0:1])
                nc.gpsimd.tensor_mul(out=scrB[:, 0:nb, :], in0=dnn[:, 0:nb, :],
                                     in1=wsn[:, 0:nb, :])
                nc.vector.tensor_reduce(out=g[:, 1, 0:nb, 0:1], in_=scrB[:, 0:nb, :],
                                        axis=mybir.AxisListType.X, op=ALU.add)
                nc.gpsimd.tensor_mul(out=scrA[:, 0:nb, :], in0=dnn[:, 0:nb, :],
                                     in1=wtn[:, 0:nb, :])
                nc.vector.tensor_reduce(out=g[:, 1, 0:nb, 1:2], in_=scrA[:, 0:nb, :],
                                        axis=mybir.AxisListType.X, op=ALU.add)
                yield
                for it in range(3):
                    if it == 0:
                        S, Dr = 0, 1
                    else:
                        # D = B + w*C ; S = A + w*(B + D)
                        nc.vector.tensor_mul(out=g[:, 4, 0:nb, :], in0=g[:, 15, 0:nb, :], in1=g[:, 2, 0:nb, :])
                        nc.vector.tensor_add(out=g[:, 4, 0:nb, :], in0=g[:, 1, 0:nb, :], in1=g[:, 4, 0:nb, :])
                        nc.vector.tensor_add(out=g[:, 5, 0:nb, :], in0=g[:, 1, 0:nb, :], in1=g[:, 4, 0:nb, :])
                        nc.vector.tensor_mul(out=g[:, 5, 0:nb, :], in0=g[:, 15, 0:nb, :], in1=g[:, 5, 0:nb, :])
                        nc.vector.tensor_add(out=g[:, 3, 0:nb, :], in0=g[:, 0, 0:nb, :], in1=g[:, 5, 0:nb, :])
                        S, Dr = 3, 4
                    # squash scale from S: g10 = 0.125*S / ((0.25*S+1)*(0.5*sqrt(S)+1e-9))
                    nc.vector.tensor_scalar_max(out=g[:, 6, 0:nb, :], in0=g[:, S, 0:nb, :],
                                                scalar1=1e-30)
                    emit_rsqrt(nc.vector, g[:, 7, 0:nb, :], g[:, 6, 0:nb, :], g[:, 8, 0:nb, :], niter=1)
                    nc.vector.tensor_mul(out=g[:, 8, 0:nb, :], in0=g[:, 6, 0:nb, :], in1=g[:, 7, 0:nb, :])
                    nc.vector.tensor_scalar(out=g[:, 9, 0:nb, :], in0=g[:, 6, 0:nb, :],
                                            scalar1=0.25, scalar2=1.0,
                                            op0=ALU.mult, op1=ALU.add)
                    nc.vector.tensor_scalar(out=g[:, 8, 0:nb, :], in0=g[:, 8, 0:nb, :],
                                            scalar1=0.5, scalar2=1e-9,
                                            op0=ALU.mult, op1=ALU.add)
                    nc.vector.tensor_mul(out=g[:, 9, 0:nb, :], in0=g[:, 9, 0:nb, :], in1=g[:, 8, 0:nb, :])
                    nc.vector.reciprocal_approx_fast(out=g[:, 13, 0:nb, :], in_=g[:, 9, 0:nb, :])
                    nc.vector.scalar_tensor_tensor(out=g[:, 10, 0:nb, :], in0=g[:, 6, 0:nb, :],
                                                   scalar=0.125, in1=g[:, 13, 0:nb, :],
                                                   op0=ALU.mult, op1=ALU.mult)
                    if it < 2:
                        # logit update u = D*scale ; diff += dsign*u ; w = dsign*tanh(diff/2)
                        nc.vector.tensor_mul(out=g[:, 5, 0:nb, :], in0=g[:, Dr, 0:nb, :], in1=g[:, 10, 0:nb, :])
                        if it == 0:
                            nc.vector.tensor_mul(out=diff[:, 0:nb, :], in0=g[:, 5, 0:nb, :], in1=dsign_sb[:, 0:nb, :])
                        else:
                            nc.vector.tensor_mul(out=g[:, 14, 0:nb, :], in0=g[:, 5, 0:nb, :], in1=dsign_sb[:, 0:nb, :])
                            nc.vector.tensor_add(out=diff[:, 0:nb, :], in0=diff[:, 0:nb, :], in1=g[:, 14, 0:nb, :])
                        nc.scalar.activation(g[:, 14, 0:nb, :], diff[:, 0:nb, :], AF.Tanh, scale=0.5)
                        nc.vector.tensor_mul(out=g[:, 15, 0:nb, :], in0=g[:, 14, 0:nb, :], in1=dsign_sb[:, 0:nb, :])
                        yield
                    else:
                        # vn = (wn + w*dnn)*scale = wn*a + dnn*b, b = w*a
                        nc.vector.tensor_mul(out=g[:, 5, 0:nb, :], in0=g[:, 15, 0:nb, :],
                                             in1=g[:, 10, 0:nb, :])
                        for bi in range(nblk):
                            nc.gpsimd.tensor_scalar(
                                out=scrA[:, bi, :], in0=wsn[:, bi, :],
                                scalar1=g[:, 10, bi, 0:1], scalar2=None,
                                op0=ALU.mult)
                            nc.vector.scalar_tensor_tensor(
                                out=vn[0][:, blk6(bi), :], in0=dnn[:, bi, :],
                                scalar=g[:, 5, bi, 0:1], in1=scrA[:, bi, :],
                                op0=ALU.mult, op1=ALU.add)
                            nc.gpsimd.tensor_scalar(
                                out=scrB[:, bi, :], in0=wtn[:, bi, :],
                                scalar1=g[:, 10, bi, 1:2], scalar2=None,
                                op0=ALU.mult)
                            nc.vector.scalar_tensor_tensor(
                                out=vn[1][:, blk6(bi), :], in0=dnn[:, bi, :],
                                scalar=g[:, 5, bi, 1:2], in1=scrB[:, bi, :],
                                op0=ALU.mult, op1=ALU.add)
                # ---- transpose v to vT (rolling 6-block window) ----
                for prob in range(2):
                    for bi in range(nblk):
                        pst = pspool.tile([128, 2, 128], F32R, tag="E1")
                        for o in range(2):
                            nc.tensor.transpose(pst[:, o, :], vn[prob][:, blk6(bi), o * 128:(o + 1) * 128], eye32[:])
                        m = blk6(bi)
                        if prob == 0:
                            nc.scalar.activation(vT[prob][:, :, m * 128:(m + 1) * 128],
                                                 pst[:], AF.Copy, bias=0.0, scale=1.0)
                        else:
                            nc.vector.tensor_copy(out=vT[prob][:, :, m * 128:(m + 1) * 128], in_=pst[:])
                yield
                # ---- banded time attention ----
                sfT = work.tile([128, 2, CW], F16, tag="bufA")
                tfT = work.tile([128, 2, CW], F16, tag="bufB")
                for prob in range(2):
                    vTt, vnt = vT[prob], vn[prob]
                    dstT = sfT if prob == 0 else tfT
                    q0 = 0
                    while q0 < w:
                        qw = min(256, w - q0)
                        Q0 = l0 + q0
                        mq = ((Q0 // 128) % 6) * 128
                        bbs = [bb for bb in range(1, 4)
                               if Q0 + 128 * (bb - 2) >= seg_start[0]
                               and 128 * (bb - 2) < qw]
                        Pt = wk3.tile([128, 4, 256], F32R, tag="Pbuf")
                        zones = {}
                        for bb in bbs:
                            zones.setdefault(bb // 2, []).append(bb)
                        for z, zbbs in zones.items():
                            base = zbbs[0]
                            Sps = pspool.tile([128, 2, 256], F32, tag="SbigZ",
                                              name=f"Sps_{z}")
                            for bb in zbbs:
                                K0 = Q0 + 128 * (bb - 2)
                                mk = ((K0 // 128) % 6) * 128
                                lo = max(0, 128 * (bb - 2))
                                nc.tensor.matmul(Sps[:, bb - base, 0:qw],
                                                 dbias16[0:1, bb * 128:(bb + 1) * 128],
                                                 ones16[0:1, 0:qw],
                                                 start=(bb == zbbs[0]), stop=False)
                                for o in range(2):
                                    nc.tensor.matmul(Sps[:, bb - base, lo:qw],
                                                     vTt[:, o, mk:mk + 128],
                                                     vTt[:, o, mq + lo:mq + qw],
                                                     start=False,
                                                     stop=(bb == zbbs[-1] and o == 1))
                            for bb in zbbs:
                                if bb >= 2:
                                    dlo = 128 * (bb - 2)
                                    dwdt = min(qw, dlo + 128) - dlo
                                    nc.vector.tensor_add(out=Sps[:, bb - base, dlo:dlo + dwdt],
                                                         in0=Sps[:, bb - base, dlo:dlo + dwdt],
                                                         in1=T32[:, 0:dwdt])
                            nc.scalar.activation(Pt[:, base:base + len(zbbs), 0:qw],
                                                 Sps[:, 0:len(zbbs), 0:qw], AF.Exp)
                        od = psO.tile([128, 3, 256], F32, tag="OD")
                        for bb in bbs:
                            K0 = Q0 + 128 * (bb - 2)
                            kb6 = (K0 // 128) % 6
                            lo = max(0, 128 * (bb - 2))
                            first, last = bb == bbs[0], bb == bbs[-1]
                            for o in range(2):
                                nc.tensor.matmul(od[:, o, lo:qw],
                                                 vnt[:, kb6, o * 128:(o + 1) * 128],
                                                 Pt[:, bb, lo:qw],
                                                 start=(first and o == 0),
                                                 stop=(last and o == 1))
                            nc.tensor.matmul(od[:, 2, lo:qw], ones32[:, 0:128],
                                             Pt[:, bb, lo:qw],
                                             start=first, stop=last)
                        rec = work.tile([128, 256], F32, tag="tmpB")
                        nc.vector.reciprocal_approx_fast(out=rec[:, 0:qw], in_=od[:, 2, 0:qw])
                        for o in range(2):
                            nc.vector.tensor_mul(out=dstT[:, o, q0:q0 + qw],
                                                 in0=od[:, o, 0:qw], in1=rec[:, 0:qw])
                        q0 += qw
                yield
                # ---- fusion + qkv ----
                fused = work.tile([128, 2, CW], F16, tag="bufC")
                for o in range(2):
                    psl = pspool.tile([128, 2, CW], F32, tag="L3", name="ps_fus")
                    ps = psl[:, 0]
                    for k in range(2):
                        nc.tensor.matmul(ps[:, 0:w],
                                         pk[:, C_FWT + (k * 2 + o) * 128:C_FWT + (k * 2 + o) * 128 + 128],
                                         sfT[:, k, 0:w],
                                         start=(k == 0), stop=False)
                        nc.tensor.matmul(ps[:, 0:w],
                                         pk[:, C_FWT + ((2 + k) * 2 + o) * 128:C_FWT + ((2 + k) * 2 + o) * 128 + 128],
                                         tfT[:, k, 0:w],
                                         start=False, stop=(k == 1))
                    nc.scalar.activation(fused[:, o, 0:w], ps[:, 0:w], AF.Identity,
                                         bias=pk32[:, P32_FB + o:P32_FB + o + 1], scale=1.0)
                for o in range(2):
                    psqk = pspool.tile([128, 2, CW], F32, tag="L3", name="psqk")
                    for k in range(2):
                        nc.tensor.matmul(psqk[:, 0, 0:w],
                                         pk[:, C_WQT + (k * 2 + o) * 128:C_WQT + (k * 2 + o) * 128 + 128],
                                         fused[:, k, 0:w],
                                         start=(k == 0), stop=(k == 1))
                        nc.tensor.matmul(psqk[:, 1, 0:w],
                                         pk[:, C_WKT + (k * 2 + o) * 128:C_WKT + (k * 2 + o) * 128 + 128],
                                         fused[:, k, 0:w],
                                         start=(k == 0), stop=(k == 1))
                    nc.scalar.activation(qT_all[:, o, l0:l0 + w], psqk[:, 0, 0:w], AF.Identity,
                                         bias=pk32[:, P32_BQ + o:P32_BQ + o + 1], scale=1.0)
                    nc.scalar.activation(kT_all[:, o, l0:l0 + w], psqk[:, 1, 0:w], AF.Identity,
                                         bias=pk32[:, P32_BK + o:P32_BK + o + 1], scale=1.0)
                for bi in range(nblk):
                    psvl = pspool.tile([128, 2, CW], F32, tag="L3", name="psv")
                    psv = psvl[:, 0, 0:256]
                    for k in range(2):
                        nc.tensor.matmul(psv, fused[:, k, bi * 128:(bi + 1) * 128],
                                         pk[:, C_WVT + k * 256:C_WVT + k * 256 + 256],
                                         start=(k == 0), stop=False)
                    nc.tensor.matmul(psv, ones16[0:1, 0:128], bv_row[0:1, 0:256],
                                     start=False, stop=True)
                    nc.vector.tensor_copy(out=v16[:, l0 // 128 + bi, :], in_=psv)

            def drive(chunks):
                # software-pipelined emission: chunk i+1's conv/LN instruction
                # stream is interleaved into chunk i's routing stream so PE/Act
                # have work while the serial routing chain runs on DVE.
                s1 = emit_s1f(*chunks[0])
                mv = emit_s1b1(*chunks[0], *s1)
                emit_s1b2(chunks[0][0], chunks[0][1], s1[0], *mv)
                for i, c in enumerate(chunks):
                    gen = emit_s2(*c)
                    nxt = chunks[i + 1] if i + 1 < len(chunks) else None
                    if nxt:
                        s1 = emit_s1f(*nxt)
                    next(gen)            # trend taps + transposes
                    if nxt:
                        mv = emit_s1b1(*nxt, *s1)
                    next(gen)            # invariants
                    if nxt:
                        emit_s1b2(nxt[0], nxt[1], s1[0], *mv)
                    next(gen)            # iteration 0
                    next(gen)            # iteration 1
                    for _ in gen:        # it2 + vn + vT, banded, fusion, qkv
                        pass

            def emit_mha(qlo, qhi):
                for Q0 in range(qlo, qhi, 512):
                    qw = min(512, qhi - Q0)
                    nkv = (Q0 + qw) // 128
                    obuf = work.tile([128, 4, 256], F32, tag="obuf", name="obuf")
                    for hp in range(4):
                        hg = hp // 2
                        rows = [32 * ((2 * hp) % 4), 32 * ((2 * hp + 1) % 4)]
                        Oh = psO.tile([33, 2, 512], F32, tag="OD", name=f"Oh_{hp}")
                        sps = pspool.tile([128, 2, 512], F32, tag="L3", name="sps")
                        sps_z = pspool.tile([128, 512], F32, tag="SbigZ", name="sps_z")
                        slots = [sps[:, 0], sps[:, 1], sps_z[:]]
                        step = 0
                        pend = None

                        def flush(p):
                            kb_, jj_, Pm_, lo_, fi_, la_ = p
                            h_ = 2 * hp + jj_
                            nc.tensor.matmul(Oh[:, jj_, lo_:qw],
                                             v_all[:, kb_, h_, 0:33],
                                             Pm_[:, lo_:qw], start=fi_, stop=la_)

                        for kb in range(nkv):
                            K0 = kb * 128
                            dlt = K0 - Q0
                            lo = max(0, dlt)
                            dwdt = min(qw, dlt + 128) - dlt if dlt >= 0 else 0
                            first, last = kb == 0, kb == nkv - 1
                            for jj in range(2):
                                sp = slots[step % len(slots)]
                                step += 1
                                nc.tensor.matmul(sp[:, lo:qw],
                                                 kT_all[rows[jj]:rows[jj] + 32, hg, K0:K0 + 128],
                                                 qT_all[rows[jj]:rows[jj] + 32, hg, Q0 + lo:Q0 + qw],
                                                 start=True, stop=True,
                                                 tile_position=(rows[jj], 0))
                                if dlt >= 0:
                                    nc.vector.tensor_add(out=sp[:, dlt:dlt + dwdt],
                                                         in0=sp[:, dlt:dlt + dwdt],
                                                         in1=T32[:, 0:dwdt])
                                Pm = wk3.tile([128, 512], F32R, tag="Pbuf", name="Pm")
                                nc.scalar.activation(Pm[:, lo:qw], sp[:, lo:qw], AF.Exp)
                                if pend is not None:
                                    flush(pend)
                                pend = (kb, jj, Pm, lo, first, last)
                        flush(pend)
                        # denominator row 32 -> partition 0, reciprocal there,
                        # broadcast to 32 partitions via a K=1 matmul
                        den = work.tile([33, 2, 512], F32, tag="tmpC", name="den_m")
                        nc.vector.tensor_copy(out=den[32:33, :, 0:qw],
                                              in_=Oh[32:33, :, 0:qw])
                        d0 = work.tile([1, 2, 512], F32, tag="r0row", name="d0row")
                        nc.sync.dma_start(d0[0:1, :, 0:qw], den[32:33, :, 0:qw])
                        r16 = work.tile([1, 2, 512], F16, tag="r1row", name="r16row")
                        nc.vector.reciprocal_approx_fast(out=d0[0:1, :, 0:qw],
                                                         in_=d0[0:1, :, 0:qw])
                        nc.vector.tensor_copy(out=r16[0:1, :, 0:qw],
                                              in_=d0[0:1, :, 0:qw])
                        recBp = pspool.tile([32, 2, 512], F32, tag="E2", name="recBp")
                        for jj in range(2):
                            nc.tensor.matmul(recBp[:, jj, 0:qw], ones16[0:1, 0:32],
                                             r16[0:1, jj, 0:qw], start=True, stop=True)
                        recB = work.tile([32, 2, 512], F32, tag="bufC", name="recB_sb")
                        nc.vector.tensor_copy(out=recB[:, :, 0:qw], in_=recBp[:, :, 0:qw])
                        Ohn = work.tile([32, 2, 512], F16, tag="bufA", name="Ohn")
                        nc.vector.tensor_mul(out=Ohn[:, :, 0:qw], in0=Oh[0:32, :, 0:qw],
                                             in1=recB[:, :, 0:qw])
                        if Q0 == 0:
                            nc.vector.tensor_copy(out=Ohn[:, :, 0:1],
                                                  in_=zeros8[0:32, 0:2].unsqueeze(-1))
                        for bi in range(qw // 128):
                            psop = pspool.tile([128, 256], F32, tag="E1", name="psop")
                            for jj in range(2):
                                nc.tensor.matmul(psop[:], Ohn[:, jj, bi * 128:(bi + 1) * 128],
                                                 woT_sb[0:32, 2 * hp + jj, :],
                                                 start=(jj == 0), stop=(jj == 1 and hp != 0))
                            if hp == 0:
                                nc.tensor.matmul(psop[:], ones16[0:1, 0:128], bo_row[0:1, 0:256],
                                                 start=False, stop=True)
                                nc.vector.tensor_copy(out=obuf[:, bi, :], in_=psop[:])
                            else:
                                nc.vector.tensor_add(out=obuf[:, bi, :], in0=obuf[:, bi, :], in1=psop[:])
                    for bi in range(qw // 128):
                        ot = work.tile([128, 256], F16, tag="tmpC", name="ot16")
                        nc.vector.tensor_copy(out=ot[:], in_=obuf[:, bi, :])
                        nc.sync.dma_start(outd[Q0 - qlo + bi * 128:Q0 - qlo + (bi + 1) * 128, :], ot[:])

            seg_start = [0]

            def emit_pipeline(vi):
                # position-local pipeline over [seg0, hi): own slab + one
                # 512-wide halo chunk (band reach 256 + conv/trend taps);
                # pack this slab's K and V for the cross-core AllGather.
                lo, hi = ABOUNDS[vi], ABOUNDS[vi + 1]
                seg0 = max(0, lo - 512)
                seg_start[0] = seg0
                chunks = []
                l0 = seg0
                while l0 < hi:
                    w = min(CW, hi - l0)
                    chunks.append((l0, w))
                    l0 += w
                drive(chunks)
                if force_variant is None:
                    nc.sync.dma_start(ag2_in[:, 0:1024], kT_all[:, :, lo:hi])
                    nc.sync.dma_start(ag2_in[:, 1024:2048],
                                      v16[:, 4 * vi:4 * vi + 4, :])

            def emit_variant(vi):
                # single-core path for TimelineSim: no collectives
                emit_pipeline(vi)
                for h in range(8):
                    nc.vector.tensor_copy(out=v_all[:, :, h, 0:32],
                                          in_=v16[:, :, h * 32:(h + 1) * 32])
                emit_mha(ABOUNDS[vi], ABOUNDS[vi + 1])

            if force_variant is not None:
                emit_variant(force_variant)
            else:
                with tc.If(pid < 2) as c0:
                    emit_pipeline(0)
                with c0.Else():
                    with tc.If(pid < 4) as c1:
                        emit_pipeline(1)
                    with c1.Else():
                        with tc.If(pid < 6) as c2:
                            emit_pipeline(2)
                        with c2.Else():
                            emit_pipeline(3)

                # cross-core K/V AllGather (top level: no control flow)
                nc.gpsimd.collective_compute(
                    "AllGather", ALU.bypass,
                    replica_groups=[list(range(8))],
                    ins=[ag2_in[:]], outs=[ag2_out[:]])

                # unpack the 4 slabs of my batch (batch = pid & 1)
                def unpack(b):
                    for c in (b, b + 2, b + 4, b + 6):
                        w0 = 512 * (c // 2)
                        nc.sync.dma_start(kT_all[:, :, w0:w0 + 512],
                                          ag2_out[c, :, 0:1024])
                        nc.sync.dma_start(v16[:, w0 // 128:w0 // 128 + 4, :],
                                          ag2_out[c, :, 1024:2048])

                def up_tree(lo_pid, hi_pid):
                    if hi_pid - lo_pid == 1:
                        unpack(lo_pid & 1)
                        return
                    mid = (lo_pid + hi_pid) // 2
                    with tc.If(pid < mid) as cc:
                        up_tree(lo_pid, mid)
                    with cc.Else():
                        up_tree(mid, hi_pid)

                up_tree(0, 8)
                for h in range(8):
                    nc.vector.tensor_copy(out=v_all[:, :, h, 0:32],
                                          in_=v16[:, :, h * 32:(h + 1) * 32])

                with tc.If(pid < 2) as d0:
                    emit_mha(ABOUNDS[0], ABOUNDS[1])
                with d0.Else():
                    with tc.If(pid < 4) as d1:
                        emit_mha(ABOUNDS[1], ABOUNDS[2])
                    with d1.Else():
                        with tc.If(pid < 6) as d2:
                            emit_mha(ABOUNDS[2], ABOUNDS[3])
                        with d2.Else():
                            emit_mha(ABOUNDS[3], ABOUNDS[4])
    nc.finalize()
    return nc


_CACHE = {}


def kernel(**inputs):
    from concourse.bass_utils import run_bass_kernel_spmd
    in_maps = build_in_maps(inputs)
    if "nc" not in _CACHE:
        _CACHE["nc"] = _build()
    nc = _CACHE["nc"]
    res = run_bass_kernel_spmd(nc, in_maps, core_ids=list(range(8)))
    out = np.zeros((B, L, D), np.float32)
    for core in range(8):
        b = core & 1
        vi = core // 2
        lo, hi = ABOUNDS[vi], ABOUNDS[vi + 1]
        out[b, lo:hi, :] = res.results[core]["out"].astype(np.float32)
    return out


# revision 26
# speedup vs baseline: 1.0516x; 1.0005x over previous
# Trainium2 Bass kernel for nn_BAKTTime (dense_transformer).
# Self-contained: hardcodes shapes B=2, L=2048, D=256, H=8, dk=32.
#
# Sharding: 8 cores, SPMD program. core i handles batch (i & 1) and query
# slab (i // 2); slab j computes the position-local pipeline (folded
# 5-tap conv + layernorm + capsule routing + banded time attention + fusion
# + qkv) over the causal prefix [0, 512*(j+1)), then flash MHA over q in
# [512*j, 512*(j+1)).  The wall-clock of this problem is dominated by
# host<->device transfer over the axon tunnel, so all inputs are packed
# into one fp16 [128, C] tensor per core; with USE_AG each core ships only
# a 1/8 column slice and an on-device AllGather reconstructs the full
# pack.  Constant matrices (identity, causal mask, decay bias, bias rows,
# depthwise-diag) are built on device.  Output is a per-core fp16
# (512, 256) slab; the host stitches the 8 slabs.
import numpy as np

B, L, D = 2, 2048, 256
H, DK = 8, 32
DECAY = 0.2
EPS_LN = 1e-12
NEG = -1e30
CW = 512
ABOUNDS = (0, 512, 1024, 1536, 2048)
SLAB = 512

USE_AG = True  # AllGather weights+x on device (ship 1/8 per core)

# ---- fp16 pack column layout (single source of truth) ----
C_WTT = 0                      # [2(k),5(t),2(o),128]
C_FWT = C_WTT + 2560           # [4(k),2(o),128]
C_WQT = C_FWT + 1024           # [2(k),2(o),128]
C_WKT = C_WQT + 512
C_WVT = C_WKT + 512            # [2(k),256]
C_WO = C_WVT + 512             # head h at partitions [32*(h%4),+32), col (h//4)*256
C_EYE = C_WO + 512             # [128,128] identity
C_DWC = C_EYE + 128            # [2(pd),5(t),2(o)] depthwise tap coefs
C_IOTA = C_DWC + 20            # [128,1] iota
C_BVC = C_IOTA + 1             # [2] bv col layout
C_BOC = C_BVC + 2              # [2] bo col layout
C_P32HI = C_BOC + 2            # [33] fp16 hi half of the f32 pack
C_P32LO = C_P32HI + 33         # [33] fp16 lo half (v - f32(hi))
W_COLS = 5888                  # C_P32LO+33 = 5851, padded to 5888
X_COLS = 2 * (4 + L)           # 4104
PK_COLS = W_COLS + X_COLS      # 9896 (per-core pack: weights + my batch xT)
AG_COLS = W_COLS + 2 * X_COLS  # 14000 (global pack: weights + both batches)
AG_SL = AG_COLS // 8           # 1750

# pack32 f32 [128, 33]
P32_BEFF, P32_LNW, P32_LNB, P32_FB, P32_BQ, P32_BK, P32_IOTA, P32_DWC = 0, 2, 4, 6, 8, 10, 12, 13
P32_COLS = 33


def _host_prep(inp):
    f32, f16 = np.float32, np.float16
    x = np.asarray(inp["x"], f32)
    c3w, c3b = np.asarray(inp["conv3_w"], f32), np.asarray(inp["conv3_b"], f32)
    c5w, c5b = np.asarray(inp["conv5_w"], f32), np.asarray(inp["conv5_b"], f32)
    b3 = np.asarray(inp["beta3"], f32).reshape(D)
    b5 = np.asarray(inp["beta5"], f32).reshape(D)
    cw = np.asarray(inp["combine_w"], f32)
    cwt = np.exp(cw - cw.max())
    cwt = (cwt / cwt.sum()).astype(f32)
    g3 = (cwt[0] * (1.0 - b3 ** 2)).astype(f32)
    g5 = (cwt[1] * (1.0 - b5 ** 2)).astype(f32)
    dscale = (1.0 + cwt[0] * b3 ** 2 + cwt[1] * b5 ** 2).astype(f32)
    W = np.zeros((5, D, D), f32)
    W[0] = g3[:, None] * c3w[:, :, 2] + g5[:, None] * c5w[:, :, 4] + np.diag(dscale)
    W[1] = g3[:, None] * c3w[:, :, 1] + g5[:, None] * c5w[:, :, 3]
    W[2] = g3[:, None] * c3w[:, :, 0] + g5[:, None] * c5w[:, :, 2]
    W[3] = g5[:, None] * c5w[:, :, 1]
    W[4] = g5[:, None] * c5w[:, :, 0]
    # lhsT layout [din_par, din_ch(k), tap, o_ch, dout]
    wtT = np.transpose(W, (0, 2, 1)).reshape(5, 2, 128, 2, 128)
    wtT = np.ascontiguousarray(np.transpose(wtT, (2, 1, 0, 3, 4)))  # [128,2,5,2,128]
    beff = (g3 * c3b + g5 * c5b).reshape(2, 128).T.copy()           # [128, 2]
    dw3 = np.asarray(inp["dw3_w"], f32)[:, 0, :]
    dw5 = np.asarray(inp["dw5_w"], f32)[:, 0, :]
    c3l = np.zeros((5, D), f32)
    c5l = np.zeros((5, D), f32)
    for l in range(3):
        c3l[l] = dw3[:, 2 - l]
    for l in range(5):
        c5l[l] = dw5[:, 4 - l]
    pco, dco = c3l + c5l, c3l - c5l                                  # [5, 256]
    # dwc [128, 2(pd), 5(t), 2(o)]
    dwc = np.zeros((128, 2, 5, 2), f32)
    for t in range(5):
        for o in range(2):
            dwc[:, 0, t, o] = pco[t, o * 128:(o + 1) * 128]
            dwc[:, 1, t, o] = dco[t, o * 128:(o + 1) * 128]
    col = lambda v: np.asarray(v, f32).reshape(2, 128).T.copy()      # [128, 2]
    fwT = np.asarray(inp["fusion_w"], f32).T.reshape(4, 128, 2, 128)
    fwT = np.ascontiguousarray(np.transpose(fwT, (1, 0, 2, 3)))      # [128,4,2,128]
    s = 1.0 / np.sqrt(DK)

    def sqT(w):
        t = np.asarray(w, f32).T.reshape(2, 128, 2, 128)
        return np.ascontiguousarray(np.transpose(t, (1, 0, 2, 3)))   # [128,2,2,128]

    def hvT(w):
        t = np.asarray(w, f32).T.reshape(2, 128, 256)
        return np.ascontiguousarray(np.transpose(t, (1, 0, 2)))      # [128,2,256]

    # wo pack [128, 512]: head h tile (32,256) at partitions [32*(h%4),+32),
    # cols [(h//4)*256,+256)
    woT = np.asarray(inp["wo"], f32).T.reshape(8, 32, 256).transpose(1, 0, 2)  # [32,8,256]
    wop = np.zeros((128, 512), f32)
    for h in range(8):
        wop[32 * (h % 4):32 * (h % 4) + 32, (h // 4) * 256:(h // 4) * 256 + 256] = woT[:, h, :]

    Wpk = np.zeros((128, W_COLS), f32)
    Wpk[:, C_WTT:C_WTT + 2560] = wtT.reshape(128, -1)
    Wpk[:, C_FWT:C_FWT + 1024] = fwT.reshape(128, -1)
    Wpk[:, C_WQT:C_WQT + 512] = sqT(np.asarray(inp["wq"], f32) * s).reshape(128, -1)
    Wpk[:, C_WKT:C_WKT + 512] = sqT(inp["wk"]).reshape(128, -1)
    Wpk[:, C_WVT:C_WVT + 512] = hvT(inp["wv"]).reshape(128, -1)
    Wpk[:, C_WO:C_WO + 512] = wop
    Wpk[:, C_EYE:C_EYE + 128] = np.eye(128, dtype=f32)
    Wpk[:, C_DWC:C_DWC + 20] = dwc.reshape(128, -1)
    Wpk[:, C_IOTA:C_IOTA + 1] = np.arange(128, dtype=f32)[:, None]
    Wpk[:, C_BVC:C_BVC + 2] = col(inp["bv"])
    Wpk[:, C_BOC:C_BOC + 2] = col(inp["bo"])

    # xT [B, 128, 2, 4+L]: 4 leading zero cols per o-half for the conv halo
    xT = np.zeros((B, 128, 2, 4 + L), f32)
    xt_full = np.transpose(x, (0, 2, 1)).reshape(B, 2, 128, L)
    xT[:, :, :, 4:] = np.transpose(xt_full, (0, 2, 1, 3))
    xT16 = xT.reshape(B, 128, X_COLS).astype(f16)

    pk32 = np.zeros((128, P32_COLS), f32)
    pk32[:, P32_BEFF:P32_BEFF + 2] = beff
    pk32[:, P32_LNW:P32_LNW + 2] = col(inp["ln_w"])
    pk32[:, P32_LNB:P32_LNB + 2] = col(inp["ln_b"])
    pk32[:, P32_FB:P32_FB + 2] = col(inp["fusion_b"])
    pk32[:, P32_BQ:P32_BQ + 2] = col(np.asarray(inp["bq"], f32) * s)
    pk32[:, P32_BK:P32_BK + 2] = col(inp["bk"])
    pk32[:, P32_IOTA:P32_IOTA + 1] = np.arange(128, dtype=f32)[:, None]
    pk32[:, P32_DWC:P32_DWC + 20] = dwc.reshape(128, -1)
    hi = pk32.astype(f16)
    lo = (pk32 - hi.astype(f32)).astype(f16)
    Wpk[:, C_P32HI:C_P32HI + P32_COLS] = hi.astype(f32)
    Wpk[:, C_P32LO:C_P32LO + P32_COLS] = lo.astype(f32)
    Wpk16 = Wpk.astype(f16)
    return Wpk16, xT16


def build_in_maps(inputs):
    Wpk16, xT16 = _host_prep(inputs)
    in_maps = []
    if USE_AG:
        gpack = np.concatenate([Wpk16, xT16[0], xT16[1]], axis=1)  # [128, AG_COLS]
        for core in range(8):
            in_maps.append(dict(
                pksl=np.ascontiguousarray(gpack[:, AG_SL * core:AG_SL * (core + 1)])))
    else:
        pk_b = [np.ascontiguousarray(np.concatenate([Wpk16, xT16[b]], axis=1))
                for b in range(B)]
        for core in range(8):
            in_maps.append(dict(pk16=pk_b[core & 1]))
    return in_maps


def _build(force_variant=None, use_ag=None):
    import concourse.mybir as mybir
    import concourse.tile as tile
    from concourse import bacc

    F32, F32R, F16 = mybir.dt.float32, mybir.dt.float32r, mybir.dt.float16
    U32 = mybir.dt.uint32
    AF = mybir.ActivationFunctionType
    ALU = mybir.AluOpType
    # 2*0x5f3759df + 1 as signed int32, for rsqrt seed (C2 + ~i) >> 1
    RSQRT_C2 = 0xBE6EB3BF - (1 << 32)

    use_ag = USE_AG if use_ag is None else use_ag

    nc = bacc.Bacc(num_devices=8) if use_ag else bacc.Bacc()

    if use_ag:
        pksl_d = nc.dram_tensor("pksl", [128, AG_SL], F16, kind="ExternalInput")
        ag_in = nc.dram_tensor("ag_in", [128, AG_SL], F16)
        ag_out = nc.dram_tensor("ag_out", [8, 128, AG_SL], F16, addr_space="Shared")
    else:
        pk16_d = nc.dram_tensor("pk16", [128, PK_COLS], F16, kind="ExternalInput")
    ag2_in = nc.dram_tensor("ag2_in", [128, 2048], F16)
    ag2_out = nc.dram_tensor("ag2_out", [8, 128, 2048], F16, addr_space="Shared")
    outd = nc.dram_tensor("out", [SLAB, D], F16, kind="ExternalOutput")

    X0 = W_COLS  # my-batch xT offset within pk

    with tile.TileContext(nc) as tc:
        pid = nc.partition_id() if force_variant is None else None
        with tc.tile_pool(name="wpool", bufs=1) as wpool, \
             tc.tile_pool(name="ppool", bufs=1) as ppool, \
             tc.tile_pool(name="work", bufs=1) as work, \
             tc.tile_pool(name="wk3", bufs=4) as wk3, \
             tc.tile_pool(name="pspool", bufs=1, space="PSUM") as pspool:
            psO = pspool

            pk = wpool.tile([128, PK_COLS], F16, name="pk")
            pk32 = wpool.tile([128, P32_COLS], F32, name="pk32s")
            if use_ag:
                nc.sync.dma_start(ag_in[:], pksl_d[:])
                nc.gpsimd.collective_compute(
                    "AllGather", ALU.bypass,
                    replica_groups=[list(range(8))],
                    ins=[ag_in[:]], outs=[ag_out[:]])

                def load_cols(dst_c0, g_c0, g_c1):
                    # copy global pack cols [g_c0,g_c1) into pk[:, dst_c0...]
                    for blk in range(8):
                        b0, b1 = blk * AG_SL, (blk + 1) * AG_SL
                        lo, hi = max(g_c0, b0), min(g_c1, b1)
                        if lo < hi:
                            nc.sync.dma_start(
                                pk[:, dst_c0 + lo - g_c0:dst_c0 + hi - g_c0],
                                ag_out[blk, :, lo - b0:hi - b0])

                load_cols(0, 0, W_COLS)

                # my batch's xT: binary branch tree on pid (batch = pid & 1)
                def xt_tree(lo_pid, hi_pid):
                    if hi_pid - lo_pid == 1:
                        bsel = lo_pid & 1
                        load_cols(X0, W_COLS + bsel * X_COLS,
                                  W_COLS + (bsel + 1) * X_COLS)
                        return
                    mid = (lo_pid + hi_pid) // 2
                    with tc.If(pid < mid) as cc:
                        xt_tree(lo_pid, mid)
                    with cc.Else():
                        xt_tree(mid, hi_pid)

                xt_tree(0, 8)
            else:
                nc.sync.dma_start(pk[:], pk16_d[:])

            # reconstruct the f32 side-pack from fp16 hi/lo halves
            p32lo = wpool.tile([128, P32_COLS], F32, name="p32lo")
            nc.vector.tensor_copy(out=pk32[:], in_=pk[:, C_P32HI:C_P32HI + P32_COLS])
            nc.vector.tensor_copy(out=p32lo[:], in_=pk[:, C_P32LO:C_P32LO + P32_COLS])
            nc.vector.tensor_add(out=pk32[:], in0=pk32[:], in1=p32lo[:])

            # ---- on-device constants ----
            ones32 = wpool.tile([128, 512], F32R, name="ones32")
            nc.vector.memset(ones32[:].bitcast(F32), 1.0)
            ones16 = wpool.tile([128, 512], F16, name="ones16")
            nc.vector.tensor_copy(out=ones16[:], in_=ones32[:])
            zeros8 = wpool.tile([128, 8], F32, name="zeros8")
            nc.vector.memset(zeros8[:], 0.0)
            eps_sb = wpool.tile([128, 1], F32, name="eps_sb")
            nc.vector.memset(eps_sb[:], EPS_LN)
            dsign_sb = wpool.tile([128, 4, 2], F32, name="dsign_sb")
            nc.vector.memset(dsign_sb[:, :, 0:1], -1.0)
            nc.vector.memset(dsign_sb[:, :, 1:2], 1.0)

            eye16 = pk[:, C_EYE:C_EYE + 128]
            eye32 = wpool.tile([128, 128], F32R, name="eye32")
            nc.vector.tensor_copy(out=eye32[:], in_=eye16)

            # iota row via M=1 matmul: out[0,j] = sum_k iota[k]*eye[k,j]
            rowp = pspool.tile([128, 128], F32, tag="E1", name="rowp")
            nc.tensor.matmul(rowp[0:1, 0:128], pk[:, C_IOTA:C_IOTA + 1], eye16,
                             start=True, stop=True)
            iota_r16 = wpool.tile([1, 128], F16, name="iota_r16")
            nc.vector.tensor_copy(out=iota_r16[:], in_=rowp[0:1, 0:128])
            iota_r32 = wpool.tile([1, 128], F32, name="iota_r32")
            nc.vector.tensor_copy(out=iota_r32[:], in_=rowp[0:1, 0:128])

            # decay bias row [1, 512]: dbias[bb*128+i] = DECAY*(i + 128*(bb-2))
            dbias16 = wpool.tile([1, 512], F16, name="dbias16")
            for bb in range(4):
                nc.vector.tensor_scalar(
                    out=dbias16[0:1, bb * 128:(bb + 1) * 128], in0=iota_r32[:],
                    scalar1=DECAY, scalar2=DECAY * 128.0 * (bb - 2),
                    op0=ALU.mult, op1=ALU.add)

            # bias rows [1, 256] from col layout via M=1 matmuls
            bv_row = wpool.tile([1, 256], F16, name="bv_row")
            bo_row = wpool.tile([1, 256], F16, name="bo_row")
            for dst, c0 in ((bv_row, C_BVC), (bo_row, C_BOC)):
                for o in range(2):
                    rp = pspool.tile([128, 128], F32, tag="E1", name="rowp2")
                    nc.tensor.matmul(rp[0:1, 0:128], pk[:, c0 + o:c0 + o + 1],
                                     eye16, start=True, stop=True)
                    nc.vector.tensor_copy(out=dst[0:1, o * 128:(o + 1) * 128],
                                          in_=rp[0:1, 0:128])

            # causal band mask T32[r, c] = NEG where c < r else 0
            Jps = pspool.tile([128, 128], F32, tag="E2", name="Jps")
            nc.tensor.matmul(Jps[:, 0:128], ones16[0:1, 0:128], iota_r16[0:1, 0:128],
                             start=True, stop=True)
            T32 = wpool.tile([128, 128], F32, name="T32")
            nc.vector.tensor_scalar(out=T32[:], in0=Jps[:, 0:128],
                                    scalar1=pk32[:, P32_IOTA:P32_IOTA + 1],
                                    scalar2=NEG, op0=ALU.is_lt, op1=ALU.mult)


            # wo tiles at partitions 0-31: [32, 8, 256]
            woT_sb = wpool.tile([32, 8, 256], F16, name="woT_sb")
            for h in range(8):
                nc.sync.dma_start(
                    woT_sb[0:32, h, :],
                    pk[32 * (h % 4):32 * (h % 4) + 32,
                       C_WO + (h // 4) * 256:C_WO + (h // 4) * 256 + 256])

            hT32 = ppool.tile([128, 2, 2, 4 + CW], F32, name="hT32")
            vT_s = ppool.tile([128, 2, 6 * 128], F16, name="vT_s")
            vT_t = ppool.tile([128, 2, 6 * 128], F16, name="vT_t")
            vn_s = ppool.tile([128, 6, 256], F32R, name="vn_s")
            vn_t = ppool.tile([128, 6, 256], F32R, name="vn_t")
            v_all = ppool.tile([128, L // 128, 8, 36], F32R, name="v_all")
            nc.vector.memset(v_all[:, :, :, 32:33].bitcast(F32), 1.0)
            v16 = ppool.tile([128, L // 128, 256], F16, name="v16")
            qT_all = ppool.tile([128, 2, L], F16, name="qT_all")
            kT_all = ppool.tile([128, 2, L], F16, name="kT_all")
            for _o in range(2):
                for _p in range(2):
                    nc.vector.memset(hT32[:, _o, _p, 0:4], 0.0)
            vT = {0: vT_s, 1: vT_t}
            vn = {0: vn_s, 1: vn_t}

            def emit_rsqrt(eng, y, x, tmp, niter=2):
                # y <- 1/sqrt(x) elementwise; x must be > 0 (pre-clamped).
                yi, xi, ti = y.bitcast(U32), x.bitcast(U32), tmp.bitcast(U32)
                nc.vector.tensor_scalar(out=ti, in0=xi, scalar1=0, scalar2=None,
                                        op0=ALU.bitwise_not)
                nc.vector.tensor_scalar(out=ti, in0=ti, scalar1=RSQRT_C2,
                                        scalar2=None, op0=ALU.add)
                nc.vector.tensor_scalar(out=yi, in0=ti, scalar1=1, scalar2=None,
                                        op0=ALU.logical_shift_right)
                for _ in range(niter):
                    eng.tensor_mul(out=tmp, in0=y, in1=y)
                    eng.tensor_mul(out=tmp, in0=tmp, in1=x)
                    eng.tensor_scalar(out=tmp, in0=tmp, scalar1=-0.5,
                                      scalar2=1.5, op0=ALU.mult, op1=ALU.add)
                    eng.tensor_mul(out=y, in0=y, in1=tmp)

            def emit_s1f(l0, w):
                # folded conv + y/sq activations (x read from resident pack)
                y = work.tile([128, 2, CW], F32R, tag="y_sb")
                sq = work.tile([128, 2, CW], F16, tag="sq_sb")
                for o in range(2):
                    ps = pspool.tile([128, CW], F32, tag="E1")
                    for t in range(5):
                        for k in range(2):
                            xc = X0 + k * (4 + L) + l0 + 4 - t
                            nc.tensor.matmul(ps[:, 0:w],
                                             pk[:, C_WTT + ((k * 5 + t) * 2 + o) * 128:
                                                C_WTT + ((k * 5 + t) * 2 + o) * 128 + 128],
                                             pk[:, xc:xc + w],
                                             start=(t == 0 and k == 0),
                                             stop=(t == 4 and k == 1))
                    nc.scalar.activation(y[:, o, 0:w], ps[:, 0:w], AF.Identity,
                                         bias=pk32[:, P32_BEFF + o:P32_BEFF + o + 1],
                                         scale=1.0)
                    nc.scalar.activation(sq[:, o, 0:w], ps[:, 0:w], AF.Square,
                                         bias=pk32[:, P32_BEFF + o:P32_BEFF + o + 1],
                                         scale=1.0)
                return y, sq

            def emit_s1b1(l0, w, y, sq):
                # layernorm moments via replicated-moment matmuls
                mps = pspool.tile([128, 2, CW], F32, tag="E2")
                for o in range(2):
                    nc.tensor.matmul(mps[:, 0, 0:w], ones32[:, 0:128], y[:, o, 0:w],
                                     start=(o == 0), stop=(o == 1))
                    nc.tensor.matmul(mps[:, 1, 0:w], ones16[:, 0:128], sq[:, o, 0:w],
                                     start=(o == 0), stop=(o == 1))
                mu = work.tile([128, CW], F32, tag="mu")
                nc.vector.tensor_scalar_mul(out=mu[:, 0:w], in0=mps[:, 0, 0:w], scalar1=1.0 / D)
                mu2 = work.tile([128, CW], F32, tag="tmpA")
                nc.vector.tensor_mul(out=mu2[:, 0:w], in0=mu[:, 0:w], in1=mu[:, 0:w])
                var = work.tile([128, CW], F32, tag="tmpB")
                nc.vector.scalar_tensor_tensor(out=var[:, 0:w], in0=mps[:, 1, 0:w],
                                               scalar=1.0 / D, in1=mu2[:, 0:w],
                                               op0=ALU.mult, op1=ALU.subtract)
                return mu, var

            def emit_s1b2(l0, w, y, mu, var):
                par = (l0 // CW) & 1
                c0 = l0 % CW
                lnv = work.tile([128, CW], F32, tag="tmpA")
                nc.scalar.activation(lnv[:, 0:w], var[:, 0:w], AF.Ln, bias=eps_sb[:])
                rstd = work.tile([128, CW], F32, tag="tmpB")
                nc.scalar.activation(rstd[:, 0:w], lnv[:, 0:w], AF.Exp, scale=-0.5)
                for o in range(2):
                    t1 = work.tile([128, CW], F32, tag="tmpA" if o else "tmpC")
                    nc.vector.tensor_sub(out=t1[:, 0:w], in0=y[:, o, 0:w], in1=mu[:, 0:w])
                    nc.vector.tensor_mul(out=t1[:, 0:w], in0=t1[:, 0:w], in1=rstd[:, 0:w])
                    nc.vector.tensor_scalar(out=hT32[:, o, par, 4 + c0:4 + c0 + w],
                                            in0=t1[:, 0:w],
                                            scalar1=pk32[:, P32_LNW + o:P32_LNW + o + 1],
                                            scalar2=pk32[:, P32_LNB + o:P32_LNB + o + 1],
                                            op0=ALU.mult, op1=ALU.add)
                if c0 + w == CW:
                    nc.vector.tensor_copy(out=hT32[:, :, 1 - par, 0:4],
                                          in_=hT32[:, :, par, CW:CW + 4])

            def emit_s2(l0, w):
                par = (l0 // CW) & 1
                c0 = l0 % CW
                nblk = w // 128
                blk6 = lambda b: (l0 // 128 + b) % 6
                # ---- trend taps: P = t3+t5, Dt = t3-t5 (per-channel shift
                # chains on DVE/GpSimd; beats diag matmuls in this runtime) ----
                wsT = work.tile([128, 2, CW], F32R, tag="bufA")
                wtTt = work.tile([128, 2, CW], F32R, tag="bufB")
                dnT = work.tile([128, 2, CW], F32R, tag="bufC")
                for o in range(2):
                    for t in range(5):
                        src = hT32[:, o, par, 4 + c0 - t:4 + c0 - t + w]
                        cP = pk32[:, P32_DWC + t * 2 + o:P32_DWC + t * 2 + o + 1]
                        cD = pk32[:, P32_DWC + 10 + t * 2 + o:P32_DWC + 10 + t * 2 + o + 1]
                        if t == 0:
                            nc.vector.tensor_scalar(out=wtTt[:, o, 0:w], in0=src,
                                                    scalar1=cP, scalar2=None,
                                                    op0=ALU.mult)
                            nc.vector.tensor_scalar(out=dnT[:, o, 0:w], in0=src,
                                                    scalar1=cD, scalar2=None,
                                                    op0=ALU.mult)
                        else:
                            nc.vector.scalar_tensor_tensor(
                                out=wtTt[:, o, 0:w], in0=src, scalar=cP,
                                in1=wtTt[:, o, 0:w], op0=ALU.mult, op1=ALU.add)
                            nc.vector.scalar_tensor_tensor(
                                out=dnT[:, o, 0:w], in0=src, scalar=cD,
                                in1=dnT[:, o, 0:w], op0=ALU.mult, op1=ALU.add)
                    nc.vector.scalar_tensor_tensor(out=wsT[:, o, 0:w],
                                                   in0=hT32[:, o, par, 4 + c0:4 + c0 + w],
                                                   scalar=2.0, in1=wtTt[:, o, 0:w],
                                                   op0=ALU.mult, op1=ALU.subtract)
                # ---- transpose routing inputs to [l, c] ----
                wsn = work.tile([128, 4, 256], F32, tag="wsn")
                wtn = work.tile([128, 4, 256], F32, tag="wtn")
                dnn = work.tile([128, 4, 256], F32, tag="dnn")
                for srct, dst, use_act in ((wsT, wsn, False), (wtTt, wtn, True),
                                           (dnT, dnn, True)):
                    for bi in range(nblk):
                        pst = pspool.tile([128, 2, 128], F32R, tag="E1")
                        for o in range(2):
                            nc.tensor.transpose(pst[:, o, :], srct[:, o, bi * 128:(bi + 1) * 128], eye32[:])
                        if use_act:
                            nc.scalar.activation(dst[:, bi, :], pst[:],
                                                 AF.Copy, bias=0.0, scale=1.0)
                        else:
                            nc.vector.tensor_copy(out=dst[:, bi, :], in_=pst[:])
                yield
                # ---- routing invariants (st = wn + w*dnn):
                #   A_p = sum wn_p^2, B_p = sum dnn*wn_p, C = sum dnn^2
                # then per-iteration sums are analytic:
                #   S(w) = A + w*(B + D(w)),  D(w) = B + w*C.
                g = work.tile([128, 16, 4, 2], F32, tag="g")
                diff = work.tile([128, 4, 2], F32, tag="diff")
                scrA = work.tile([128, 4, 256], F32, tag="bufB", name="scrA")
                scrB = work.tile([128, 4, 256], F32, tag="bufA", name="scrB")
                nb = nblk
                nc.vector.tensor_mul(out=scrA[:, 0:nb, :], in0=wsn[:, 0:nb, :],
                                     in1=wsn[:, 0:nb, :])
                nc.vector.tensor_reduce(out=g[:, 0, 0:nb, 0:1], in_=scrA[:, 0:nb, :],
                                        axis=mybir.AxisListType.X, op=ALU.add)
                nc.vector.tensor_mul(out=scrB[:, 0:nb, :], in0=wtn[:, 0:nb, :],
                                     in1=wtn[:, 0:nb, :])
                nc.vector.tensor_reduce(out=g[:, 0, 0:nb, 1:2], in_=scrB[:, 0:nb, :],
                                        axis=mybir.AxisListType.X, op=ALU.add)
                nc.vector.tensor_mul(out=scrA[:, 0:nb, :], in0=dnn[:, 0:nb, :],
                                     in1=dnn[:, 0:nb, :])
                nc.vector.tensor_reduce(out=g[:, 2, 0:nb, 0:1], in_=scrA[:, 0:nb, :],
                                        axis=mybir.AxisListType.X, op=ALU.add)
                nc.vector.tensor_copy(out=g[:, 2, 0:nb, 1:2], in_=g[:, 2, 0:nb, 0:1])
                nc.vector.tensor_mul(out=scrB[:, 0:nb, :], in0=dnn[:, 0:nb, :],
                                     in1=wsn[:, 0:nb, :])
                nc.vector.tensor_reduce(out=g[:, 1, 0:nb, 0:1], in_=scrB[:, 0:nb, :],
                                        axis=mybir.AxisListType.X, op=ALU.add)
                nc.vector.tensor_mul(out=scrA[:, 0:nb, :], in0=dnn[:, 0:nb, :],
                                     in1=wtn[:, 0:nb, :])
                nc.vector.tensor_reduce(out=g[:, 1, 0:nb, 1:2], in_=scrA[:, 0:nb, :],
                                        axis=mybir.AxisListType.X, op=ALU.add)
                yield
                for it in range(3):
                    if it == 0:
                        S, Dr = 0, 1
                    else:
                        # D = B + w*C ; S = A + w*(B + D)
                        nc.vector.tensor_mul(out=g[:, 4, 0:nb, :], in0=g[:, 15, 0:nb, :], in1=g[:, 2, 0:nb, :])
                        nc.vector.tensor_add(out=g[:, 4, 0:nb, :], in0=g[:, 1, 0:nb, :], in1=g[:, 4, 0:nb, :])
                        nc.vector.tensor_add(out=g[:, 5, 0:nb, :], in0=g[:, 1, 0:nb, :], in1=g[:, 4, 0:nb, :])
                        nc.vector.tensor_mul(out=g[:, 5, 0:nb, :], in0=g[:, 15, 0:nb, :], in1=g[:, 5, 0:nb, :])
                        nc.vector.tensor_add(out=g[:, 3, 0:nb, :], in0=g[:, 0, 0:nb, :], in1=g[:, 5, 0:nb, :])
                        S, Dr = 3, 4
                    # squash scale from S: g10 = 0.125*S / ((0.25*S+1)*(0.5*sqrt(S)+1e-9))
                    nc.vector.tensor_scalar_max(out=g[:, 6, 0:nb, :], in0=g[:, S, 0:nb, :],
                                                scalar1=1e-30)
                    emit_rsqrt(nc.vector, g[:, 7, 0:nb, :], g[:, 6, 0:nb, :], g[:, 8, 0:nb, :], niter=1)
                    nc.vector.tensor_mul(out=g[:, 8, 0:nb, :], in0=g[:, 6, 0:nb, :], in1=g[:, 7, 0:nb, :])
                    nc.vector.tensor_scalar(out=g[:, 9, 0:nb, :], in0=g[:, 6, 0:nb, :],
                                            scalar1=0.25, scalar2=1.0,
                                            op0=ALU.mult, op1=ALU.add)
                    nc.vector.tensor_scalar(out=g[:, 8, 0:nb, :], in0=g[:, 8, 0:nb, :],
                                            scalar1=0.5, scalar2=1e-9,
                                            op0=ALU.mult, op1=ALU.add)
                    nc.vector.tensor_mul(out=g[:, 9, 0:nb, :], in0=g[:, 9, 0:nb, :], in1=g[:, 8, 0:nb, :])
                    nc.vector.reciprocal_approx_fast(out=g[:, 13, 0:nb, :], in_=g[:, 9, 0:nb, :])
                    nc.vector.scalar_tensor_tensor(out=g[:, 10, 0:nb, :], in0=g[:, 6, 0:nb, :],
                                                   scalar=0.125, in1=g[:, 13, 0:nb, :],
                                                   op0=ALU.mult, op1=ALU.mult)
                    if it < 2:
                        # logit update u = D*scale ; diff += dsign*u ; w = dsign*tanh(diff/2)
                        nc.vector.tensor_mul(out=g[:, 5, 0:nb, :], in0=g[:, Dr, 0:nb, :], in1=g[:, 10, 0:nb, :])
                        if it == 0:
                            nc.vector.tensor_mul(out=diff[:, 0:nb, :], in0=g[:, 5, 0:nb, :], in1=dsign_sb[:, 0:nb, :])
                        else:
                            nc.vector.tensor_mul(out=g[:, 14, 0:nb, :], in0=g[:, 5, 0:nb, :], in1=dsign_sb[:, 0:nb, :])
                            nc.vector.tensor_add(out=diff[:, 0:nb, :], in0=diff[:, 0:nb, :], in1=g[:, 14, 0:nb, :])
                        nc.scalar.activation(g[:, 14, 0:nb, :], diff[:, 0:nb, :], AF.Tanh, scale=0.5)
                        nc.vector.tensor_mul(out=g[:, 15, 0:nb, :], in0=g[:, 14, 0:nb, :], in1=dsign_sb[:, 0:nb, :])
                        yield
                    else:
                        # vn = (wn + w*dnn)*scale = wn*a + dnn*b, b = w*a
                        nc.vector.tensor_mul(out=g[:, 5, 0:nb, :], in0=g[:, 15, 0:nb, :],
                                             in1=g[:, 10, 0:nb, :])
                        for bi in range(nblk):
                            nc.vector.tensor_scalar(
                                out=scrA[:, bi, :], in0=wsn[:, bi, :],
                                scalar1=g[:, 10, bi, 0:1], scalar2=None,
                                op0=ALU.mult)
                            nc.vector.scalar_tensor_tensor(
                                out=vn[0][:, blk6(bi), :], in0=dnn[:, bi, :],
                                scalar=g[:, 5, bi, 0:1], in1=scrA[:, bi, :],
                                op0=ALU.mult, op1=ALU.add)
                            nc.vector.tensor_scalar(
                                out=scrB[:, bi, :], in0=wtn[:, bi, :],
                                scalar1=g[:, 10, bi, 1:2], scalar2=None,
                                op0=ALU.mult)
                            nc.vector.scalar_tensor_tensor(
                                out=vn[1][:, blk6(bi), :], in0=dnn[:, bi, :],
                                scalar=g[:, 5, bi, 1:2], in1=scrB[:, bi, :],
                                op0=ALU.mult, op1=ALU.add)
                # ---- transpose v to vT (rolling 6-block window) ----
                for prob in range(2):
                    for bi in range(nblk):
                        pst = pspool.tile([128, 2, 128], F32R, tag="E1")
                        for o in range(2):
                            nc.tensor.transpose(pst[:, o, :], vn[prob][:, blk6(bi), o * 128:(o + 1) * 128], eye32[:])
                        m = blk6(bi)
                        if prob == 0:
                            nc.scalar.activation(vT[prob][:, :, m * 128:(m + 1) * 128],
                                                 pst[:], AF.Copy, bias=0.0, scale=1.0)
                        else:
                            nc.vector.tensor_copy(out=vT[prob][:, :, m * 128:(m + 1) * 128], in_=pst[:])
                yield
                # ---- banded time attention ----
                sfT = work.tile([128, 2, CW], F16, tag="bufA")
                tfT = work.tile([128, 2, CW], F16, tag="bufB")
                for prob in range(2):
                    vTt, vnt = vT[prob], vn[prob]
                    dstT = sfT if prob == 0 else tfT
                    q0 = 0
                    while q0 < w:
                        qw = min(256, w - q0)
                        Q0 = l0 + q0
                        mq = ((Q0 // 128) % 6) * 128
                        bbs = [bb for bb in range(1, 4)
                               if Q0 + 128 * (bb - 2) >= seg_start[0]
                               and 128 * (bb - 2) < qw]
                        Pt = wk3.tile([128, 4, 256], F32R, tag="Pbuf")
                        zones = {}
                        for bb in bbs:
                            zones.setdefault(bb // 2, []).append(bb)
                        for z, zbbs in zones.items():
                            base = zbbs[0]
                            Sps = pspool.tile([128, 2, 256], F32, tag="SbigZ",
                                              name=f"Sps_{z}")
                            for bb in zbbs:
                                K0 = Q0 + 128 * (bb - 2)
                                mk = ((K0 // 128) % 6) * 128
                                lo = max(0, 128 * (bb - 2))
                                nc.tensor.matmul(Sps[:, bb - base, 0:qw],
                                                 dbias16[0:1, bb * 128:(bb + 1) * 128],
                                                 ones16[0:1, 0:qw],
                                                 start=(bb == zbbs[0]), stop=False)
                                for o in range(2):
                                    nc.tensor.matmul(Sps[:, bb - base, lo:qw],
                                                     vTt[:, o, mk:mk + 128],
                                                     vTt[:, o, mq + lo:mq + qw],
                                                     start=False,
                                                     stop=(bb == zbbs[-1] and o == 1))
                            for bb in zbbs:
                                if bb >= 2:
                                    dlo = 128 * (bb - 2)
                                    dwdt = min(qw, dlo + 128) - dlo
                                    nc.vector.tensor_add(out=Sps[:, bb - base, dlo:dlo + dwdt],
                                                         in0=Sps[:, bb - base, dlo:dlo + dwdt],
                                                         in1=T32[:, 0:dwdt])
                            nc.scalar.activation(Pt[:, base:base + len(zbbs), 0:qw],
                                                 Sps[:, 0:len(zbbs), 0:qw], AF.Exp)
                        od = psO.tile([128, 3, 256], F32, tag="OD")
                        for bb in bbs:
                            K0 = Q0 + 128 * (bb - 2)
                            kb6 = (K0 // 128) % 6
                            lo = max(0, 128 * (bb - 2))
                            first, last = bb == bbs[0], bb == bbs[-1]
                            for o in range(2):
                                nc.tensor.matmul(od[:, o, lo:qw],
                                                 vnt[:, kb6, o * 128:(o + 1) * 128],
                                                 Pt[:, bb, lo:qw],
                                                 start=(first and o == 0),
                                                 stop=(last and o == 1))
                            nc.tensor.matmul(od[:, 2, lo:qw], ones32[:, 0:128],
                                             Pt[:, bb, lo:qw],
                                             start=first, stop=last)
                        rec = work.tile([128, 256], F32, tag="tmpB")
                        nc.vector.reciprocal_approx_fast(out=rec[:, 0:qw], in_=od[:, 2, 0:qw])
                        for o in range(2):
                            nc.vector.tensor_mul(out=dstT[:, o, q0:q0 + qw],
                                                 in0=od[:, o, 0:qw], in1=rec[:, 0:qw])
                        q0 += qw
                yield
                # ---- fusion + qkv ----
                fused = work.tile([128, 2, CW], F16, tag="bufC")
                for o in range(2):
                    psl = pspool.tile([128, 2, CW], F32, tag="L3", name="ps_fus")
                    ps = psl[:, 0]
                    for k in range(2):
                        nc.tensor.matmul(ps[:, 0:w],
                                         pk[:, C_FWT + (k * 2 + o) * 128:C_FWT + (k * 2 + o) * 128 + 128],
                                         sfT[:, k, 0:w],
                                         start=(k == 0), stop=False)
                        nc.tensor.matmul(ps[:, 0:w],
                                         pk[:, C_FWT + ((2 + k) * 2 + o) * 128:C_FWT + ((2 + k) * 2 + o) * 128 + 128],
                                         tfT[:, k, 0:w],
                                         start=False, stop=(k == 1))
                    nc.scalar.activation(fused[:, o, 0:w], ps[:, 0:w], AF.Identity,
                                         bias=pk32[:, P32_FB + o:P32_FB + o + 1], scale=1.0)
                for o in range(2):
                    psqk = pspool.tile([128, 2, CW], F32, tag="L3", name="psqk")
                    for k in range(2):
                        nc.tensor.matmul(psqk[:, 0, 0:w],
                                         pk[:, C_WQT + (k * 2 + o) * 128:C_WQT + (k * 2 + o) * 128 + 128],
                                         fused[:, k, 0:w],
                                         start=(k == 0), stop=(k == 1))
                        nc.tensor.matmul(psqk[:, 1, 0:w],
                                         pk[:, C_WKT + (k * 2 + o) * 128:C_WKT + (k * 2 + o) * 128 + 128],
                                         fused[:, k, 0:w],
                                         start=(k == 0), stop=(k == 1))
                    nc.scalar.activation(qT_all[:, o, l0:l0 + w], psqk[:, 0, 0:w], AF.Identity,
                                         bias=pk32[:, P32_BQ + o:P32_BQ + o + 1], scale=1.0)
                    nc.scalar.activation(kT_all[:, o, l0:l0 + w], psqk[:, 1, 0:w], AF.Identity,
                                         bias=pk32[:, P32_BK + o:P32_BK + o + 1], scale=1.0)
                for bi in range(nblk):
                    psvl = pspool.tile([128, 2, CW], F32, tag="L3", name="psv")
                    psv = psvl[:, 0, 0:256]
                    for k in range(2):
                        nc.tensor.matmul(psv, fused[:, k, bi * 128:(bi + 1) * 128],
                                         pk[:, C_WVT + k * 256:C_WVT + k * 256 + 256],
                                         start=(k == 0), stop=False)
                    nc.tensor.matmul(psv, ones16[0:1, 0:128], bv_row[0:1, 0:256],
                                     start=False, stop=True)
                    nc.vector.tensor_copy(out=v16[:, l0 // 128 + bi, :], in_=psv)

            def drive(chunks):
                # software-pipelined emission: chunk i+1's conv/LN instruction
                # stream is interleaved into chunk i's routing stream so PE/Act
                # have work while the serial routing chain runs on DVE.
                s1 = emit_s1f(*chunks[0])
                mv = emit_s1b1(*chunks[0], *s1)
                emit_s1b2(chunks[0][0], chunks[0][1], s1[0], *mv)
                for i, c in enumerate(chunks):
                    gen = emit_s2(*c)
                    nxt = chunks[i + 1] if i + 1 < len(chunks) else None
                    if nxt:
                        s1 = emit_s1f(*nxt)
                    next(gen)            # trend taps + transposes
                    if nxt:
                        mv = emit_s1b1(*nxt, *s1)
                    next(gen)            # invariants
                    if nxt:
                        emit_s1b2(nxt[0], nxt[1], s1[0], *mv)
                    next(gen)            # iteration 0
                    next(gen)            # iteration 1
                    for _ in gen:        # it2 + vn + vT, banded, fusion, qkv
                        pass

            def emit_mha(qlo, qhi):
                for Q0 in range(qlo, qhi, 512):
                    qw = min(512, qhi - Q0)
                    nkv = (Q0 + qw) // 128
                    obuf = work.tile([128, 4, 256], F32, tag="obuf", name="obuf")
                    for hp in range(4):
                        hg = hp // 2
                        rows = [32 * ((2 * hp) % 4), 32 * ((2 * hp + 1) % 4)]
                        Oh = psO.tile([33, 2, 512], F32, tag="OD", name=f"Oh_{hp}")
                        sps = pspool.tile([128, 2, 512], F32, tag="L3", name="sps")
                        sps_z = pspool.tile([128, 512], F32, tag="SbigZ", name="sps_z")
                        slots = [sps[:, 0], sps[:, 1], sps_z[:]]
                        step = 0
                        pend = None

                        def flush(p):
                            kb_, jj_, Pm_, lo_, fi_, la_ = p
                            h_ = 2 * hp + jj_
                            nc.tensor.matmul(Oh[:, jj_, lo_:qw],
                                             v_all[:, kb_, h_, 0:33],
                                             Pm_[:, lo_:qw], start=fi_, stop=la_)

                        for kb in range(nkv):
                            K0 = kb * 128
                            dlt = K0 - Q0
                            lo = max(0, dlt)
                            dwdt = min(qw, dlt + 128) - dlt if dlt >= 0 else 0
                            first, last = kb == 0, kb == nkv - 1
                            for jj in range(2):
                                sp = slots[step % len(slots)]
                                step += 1
                                nc.tensor.matmul(sp[:, lo:qw],
                                                 kT_all[rows[jj]:rows[jj] + 32, hg, K0:K0 + 128],
                                                 qT_all[rows[jj]:rows[jj] + 32, hg, Q0 + lo:Q0 + qw],
                                                 start=True, stop=True,
                                                 tile_position=(rows[jj], 0))
                                if dlt >= 0:
                                    nc.vector.tensor_add(out=sp[:, dlt:dlt + dwdt],
                                                         in0=sp[:, dlt:dlt + dwdt],
                                                         in1=T32[:, 0:dwdt])
                                Pm = wk3.tile([128, 512], F32R, tag="Pbuf", name="Pm")
                                nc.scalar.activation(Pm[:, lo:qw], sp[:, lo:qw], AF.Exp)
                                if pend is not None:
                                    flush(pend)
                                pend = (kb, jj, Pm, lo, first, last)
                        flush(pend)
                        # denominator row 32 -> partition 0, reciprocal there,
                        # broadcast to 32 partitions via a K=1 matmul
                        den = work.tile([33, 2, 512], F32, tag="tmpC", name="den_m")
                        nc.vector.tensor_copy(out=den[32:33, :, 0:qw],
                                              in_=Oh[32:33, :, 0:qw])
                        d0 = work.tile([1, 2, 512], F32, tag="r0row", name="d0row")
                        nc.sync.dma_start(d0[0:1, :, 0:qw], den[32:33, :, 0:qw])
                        r16 = work.tile([1, 2, 512], F16, tag="r1row", name="r16row")
                        nc.vector.reciprocal_approx_fast(out=d0[0:1, :, 0:qw],
                                                         in_=d0[0:1, :, 0:qw])
                        nc.vector.tensor_copy(out=r16[0:1, :, 0:qw],
                                              in_=d0[0:1, :, 0:qw])
                        recBp = pspool.tile([32, 2, 512], F32, tag="E2", name="recBp")
                        for jj in range(2):
                            nc.tensor.matmul(recBp[:, jj, 0:qw], ones16[0:1, 0:32],
                                             r16[0:1, jj, 0:qw], start=True, stop=True)
                        recB = work.tile([32, 2, 512], F32, tag="bufC", name="recB_sb")
                        nc.vector.tensor_copy(out=recB[:, :, 0:qw], in_=recBp[:, :, 0:qw])
                        Ohn = work.tile([32, 2, 512], F16, tag="bufA", name="Ohn")
                        nc.vector.tensor_mul(out=Ohn[:, :, 0:qw], in0=Oh[0:32, :, 0:qw],
                                             in1=recB[:, :, 0:qw])
                        if Q0 == 0:
                            nc.vector.tensor_copy(out=Ohn[:, :, 0:1],
                                                  in_=zeros8[0:32, 0:2].unsqueeze(-1))
                        for bi in range(qw // 128):
                            psop = pspool.tile([128, 256], F32, tag="E1", name="psop")
                            for jj in range(2):
                                nc.tensor.matmul(psop[:], Ohn[:, jj, bi * 128:(bi + 1) * 128],
                                                 woT_sb[0:32, 2 * hp + jj, :],
                                                 start=(jj == 0), stop=(jj == 1 and hp != 0))
                            if hp == 0:
                                nc.tensor.matmul(psop[:], ones16[0:1, 0:128], bo_row[0:1, 0:256],
                                                 start=False, stop=True)
                                nc.vector.tensor_copy(out=obuf[:, bi, :], in_=psop[:])
                            else:
                                nc.vector.tensor_add(out=obuf[:, bi, :], in0=obuf[:, bi, :], in1=psop[:])
                    for bi in range(qw // 128):
                        ot = work.tile([128, 256], F16, tag="tmpC", name="ot16")
                        nc.vector.tensor_copy(out=ot[:], in_=obuf[:, bi, :])
                        nc.sync.dma_start(outd[Q0 - qlo + bi * 128:Q0 - qlo + (bi + 1) * 128, :], ot[:])

            seg_start = [0]

            def emit_pipeline(vi):
                # position-local pipeline over [seg0, hi): own slab + one
                # 512-wide halo chunk (band reach 256 + conv/trend taps);
                # pack this slab's K and V for the cross-core AllGather.
                lo, hi = ABOUNDS[vi], ABOUNDS[vi + 1]
                seg0 = max(0, lo - 512)
                seg_start[0] = seg0
                chunks = []
                l0 = seg0
                while l0 < hi:
                    w = min(CW, hi - l0)
                    chunks.append((l0, w))
                    l0 += w
                drive(chunks)
                if force_variant is None:
                    nc.sync.dma_start(ag2_in[:, 0:1024], kT_all[:, :, lo:hi])
                    nc.sync.dma_start(ag2_in[:, 1024:2048],
                                      v16[:, 4 * vi:4 * vi + 4, :])

            def emit_variant(vi):
                # single-core path for TimelineSim: no collectives
                emit_pipeline(vi)
                for h in range(8):
                    nc.vector.tensor_copy(out=v_all[:, :, h, 0:32],
                                          in_=v16[:, :, h * 32:(h + 1) * 32])
                emit_mha(ABOUNDS[vi], ABOUNDS[vi + 1])

            if force_variant is not None:
                emit_variant(force_variant)
            else:
                with tc.If(pid < 2) as c0:
                    emit_pipeline(0)
                with c0.Else():
                    with tc.If(pid < 4) as c1:
                        emit_pipeline(1)
                    with c1.Else():
                        with tc.If(pid < 6) as c2:
                            emit_pipeline(2)
                        with c2.Else():
                            emit_pipeline(3)

                # cross-core K/V AllGather (top level: no control flow)
                nc.gpsimd.collective_compute(
                    "AllGather", ALU.bypass,
                    replica_groups=[list(range(8))],
                    ins=[ag2_in[:]], outs=[ag2_out[:]])

                # unpack the 4 slabs of my batch (batch = pid & 1)
                def unpack(b):
                    for c in (b, b + 2, b + 4, b + 6):
                        w0 = 512 * (c // 2)
                        nc.sync.dma_start(kT_all[:, :, w0:w0 + 512],
                                          ag2_out[c, :, 0:1024])
                        nc.sync.dma_start(v16[:, w0 // 128:w0 // 128 + 4, :],
                                          ag2_out[c, :, 1024:2048])

                def up_tree(lo_pid, hi_pid):
                    if hi_pid - lo_pid == 1:
                        unpack(lo_pid & 1)
                        return
                    mid = (lo_pid + hi_pid) // 2
                    with tc.If(pid < mid) as cc:
                        up_tree(lo_pid, mid)
                    with cc.Else():
                        up_tree(mid, hi_pid)

                up_tree(0, 8)
                for h in range(8):
                    nc.vector.tensor_copy(out=v_all[:, :, h, 0:32],
                                          in_=v16[:, :, h * 32:(h + 1) * 32])

                with tc.If(pid < 2) as d0:
                    emit_mha(ABOUNDS[0], ABOUNDS[1])
                with d0.Else():
                    with tc.If(pid < 4) as d1:
                        emit_mha(ABOUNDS[1], ABOUNDS[2])
                    with d1.Else():
                        with tc.If(pid < 6) as d2:
                            emit_mha(ABOUNDS[2], ABOUNDS[3])
                        with d2.Else():
                            emit_mha(ABOUNDS[3], ABOUNDS[4])
    nc.finalize()
    return nc


_CACHE = {}


def kernel(**inputs):
    from concourse.bass_utils import run_bass_kernel_spmd
    in_maps = build_in_maps(inputs)
    if "nc" not in _CACHE:
        _CACHE["nc"] = _build()
    nc = _CACHE["nc"]
    res = run_bass_kernel_spmd(nc, in_maps, core_ids=list(range(8)))
    out = np.zeros((B, L, D), np.float32)
    for core in range(8):
        b = core & 1
        vi = core // 2
        lo, hi = ABOUNDS[vi], ABOUNDS[vi + 1]
        out[b, lo:hi, :] = res.results[core]["out"].astype(np.float32)
    return out


# revision 27
# speedup vs baseline: 1.0722x; 1.0196x over previous
# Trainium2 Bass kernel for nn_BAKTTime (dense_transformer).
# Self-contained: hardcodes shapes B=2, L=2048, D=256, H=8, dk=32.
#
# Sharding: 8 cores, SPMD program. core i handles batch (i & 1) and query
# slab (i // 2); slab j computes the position-local pipeline (folded
# 5-tap conv + layernorm + capsule routing + banded time attention + fusion
# + qkv) over the causal prefix [0, 512*(j+1)), then flash MHA over q in
# [512*j, 512*(j+1)).  The wall-clock of this problem is dominated by
# host<->device transfer over the axon tunnel, so all inputs are packed
# into one fp16 [128, C] tensor per core; with USE_AG each core ships only
# a 1/8 column slice and an on-device AllGather reconstructs the full
# pack.  Constant matrices (identity, causal mask, decay bias, bias rows,
# depthwise-diag) are built on device.  Output is a per-core fp16
# (512, 256) slab; the host stitches the 8 slabs.
import numpy as np

B, L, D = 2, 2048, 256
H, DK = 8, 32
DECAY = 0.2
EPS_LN = 1e-12
NEG = -1e30
CW = 512
ABOUNDS = (0, 512, 1024, 1536, 2048)
SLAB = 512

USE_AG = True  # AllGather weights+x on device (ship 1/8 per core)

# ---- fp16 pack column layout (single source of truth) ----
C_WTT = 0                      # [2(k),5(t),2(o),128]
C_FWT = C_WTT + 2560           # [4(k),2(o),128]
C_WQT = C_FWT + 1024           # [2(k),2(o),128]
C_WKT = C_WQT + 512
C_WVT = C_WKT + 512            # [2(k),256]
C_WO = C_WVT + 512             # head h at partitions [32*(h%4),+32), col (h//4)*256
C_EYE = C_WO + 512             # [128,128] identity
C_DWC = C_EYE + 128            # [2(pd),5(t),2(o)] depthwise tap coefs
C_IOTA = C_DWC + 20            # [128,1] iota
C_BVC = C_IOTA + 1             # [2] bv col layout
C_BOC = C_BVC + 2              # [2] bo col layout
C_P32HI = C_BOC + 2            # [33] fp16 hi half of the f32 pack
C_P32LO = C_P32HI + 33         # [33] fp16 lo half (v - f32(hi))
W_COLS = 5888                  # C_P32LO+33 = 5851, padded to 5888
X_COLS = 2 * (4 + L)           # 4104
PK_COLS = W_COLS + X_COLS      # 9896 (per-core pack: weights + my batch xT)
AG_COLS = W_COLS + 2 * X_COLS  # 14000 (global pack: weights + both batches)
AG_SL = AG_COLS // 8           # 1750

# pack32 f32 [128, 33]
P32_BEFF, P32_LNW, P32_LNB, P32_FB, P32_BQ, P32_BK, P32_IOTA, P32_DWC = 0, 2, 4, 6, 8, 10, 12, 13
P32_COLS = 33


def _host_prep(inp):
    f32, f16 = np.float32, np.float16
    x = np.asarray(inp["x"], f32)
    c3w, c3b = np.asarray(inp["conv3_w"], f32), np.asarray(inp["conv3_b"], f32)
    c5w, c5b = np.asarray(inp["conv5_w"], f32), np.asarray(inp["conv5_b"], f32)
    b3 = np.asarray(inp["beta3"], f32).reshape(D)
    b5 = np.asarray(inp["beta5"], f32).reshape(D)
    cw = np.asarray(inp["combine_w"], f32)
    cwt = np.exp(cw - cw.max())
    cwt = (cwt / cwt.sum()).astype(f32)
    g3 = (cwt[0] * (1.0 - b3 ** 2)).astype(f32)
    g5 = (cwt[1] * (1.0 - b5 ** 2)).astype(f32)
    dscale = (1.0 + cwt[0] * b3 ** 2 + cwt[1] * b5 ** 2).astype(f32)
    W = np.zeros((5, D, D), f32)
    W[0] = g3[:, None] * c3w[:, :, 2] + g5[:, None] * c5w[:, :, 4] + np.diag(dscale)
    W[1] = g3[:, None] * c3w[:, :, 1] + g5[:, None] * c5w[:, :, 3]
    W[2] = g3[:, None] * c3w[:, :, 0] + g5[:, None] * c5w[:, :, 2]
    W[3] = g5[:, None] * c5w[:, :, 1]
    W[4] = g5[:, None] * c5w[:, :, 0]
    # lhsT layout [din_par, din_ch(k), tap, o_ch, dout]
    wtT = np.transpose(W, (0, 2, 1)).reshape(5, 2, 128, 2, 128)
    wtT = np.ascontiguousarray(np.transpose(wtT, (2, 1, 0, 3, 4)))  # [128,2,5,2,128]
    beff = (g3 * c3b + g5 * c5b).reshape(2, 128).T.copy()           # [128, 2]
    dw3 = np.asarray(inp["dw3_w"], f32)[:, 0, :]
    dw5 = np.asarray(inp["dw5_w"], f32)[:, 0, :]
    c3l = np.zeros((5, D), f32)
    c5l = np.zeros((5, D), f32)
    for l in range(3):
        c3l[l] = dw3[:, 2 - l]
    for l in range(5):
        c5l[l] = dw5[:, 4 - l]
    pco, dco = c3l + c5l, c3l - c5l                                  # [5, 256]
    # dwc [128, 2(pd), 5(t), 2(o)]
    dwc = np.zeros((128, 2, 5, 2), f32)
    for t in range(5):
        for o in range(2):
            dwc[:, 0, t, o] = pco[t, o * 128:(o + 1) * 128]
            dwc[:, 1, t, o] = dco[t, o * 128:(o + 1) * 128]
    col = lambda v: np.asarray(v, f32).reshape(2, 128).T.copy()      # [128, 2]
    fwT = np.asarray(inp["fusion_w"], f32).T.reshape(4, 128, 2, 128)
    fwT = np.ascontiguousarray(np.transpose(fwT, (1, 0, 2, 3)))      # [128,4,2,128]
    s = 1.0 / np.sqrt(DK)

    def sqT(w):
        t = np.asarray(w, f32).T.reshape(2, 128, 2, 128)
        return np.ascontiguousarray(np.transpose(t, (1, 0, 2, 3)))   # [128,2,2,128]

    def hvT(w):
        t = np.asarray(w, f32).T.reshape(2, 128, 256)
        return np.ascontiguousarray(np.transpose(t, (1, 0, 2)))      # [128,2,256]

    # wo pack [128, 512]: head h tile (32,256) at partitions [32*(h%4),+32),
    # cols [(h//4)*256,+256)
    woT = np.asarray(inp["wo"], f32).T.reshape(8, 32, 256).transpose(1, 0, 2)  # [32,8,256]
    wop = np.zeros((128, 512), f32)
    for h in range(8):
        wop[32 * (h % 4):32 * (h % 4) + 32, (h // 4) * 256:(h // 4) * 256 + 256] = woT[:, h, :]

    Wpk = np.zeros((128, W_COLS), f32)
    Wpk[:, C_WTT:C_WTT + 2560] = wtT.reshape(128, -1)
    Wpk[:, C_FWT:C_FWT + 1024] = fwT.reshape(128, -1)
    Wpk[:, C_WQT:C_WQT + 512] = sqT(np.asarray(inp["wq"], f32) * s).reshape(128, -1)
    Wpk[:, C_WKT:C_WKT + 512] = sqT(inp["wk"]).reshape(128, -1)
    Wpk[:, C_WVT:C_WVT + 512] = hvT(inp["wv"]).reshape(128, -1)
    Wpk[:, C_WO:C_WO + 512] = wop
    Wpk[:, C_EYE:C_EYE + 128] = np.eye(128, dtype=f32)
    Wpk[:, C_DWC:C_DWC + 20] = dwc.reshape(128, -1)
    Wpk[:, C_IOTA:C_IOTA + 1] = np.arange(128, dtype=f32)[:, None]
    Wpk[:, C_BVC:C_BVC + 2] = col(inp["bv"])
    Wpk[:, C_BOC:C_BOC + 2] = col(inp["bo"])

    # xT [B, 128, 2, 4+L]: 4 leading zero cols per o-half for the conv halo
    xT = np.zeros((B, 128, 2, 4 + L), f32)
    xt_full = np.transpose(x, (0, 2, 1)).reshape(B, 2, 128, L)
    xT[:, :, :, 4:] = np.transpose(xt_full, (0, 2, 1, 3))
    xT16 = xT.reshape(B, 128, X_COLS).astype(f16)

    pk32 = np.zeros((128, P32_COLS), f32)
    pk32[:, P32_BEFF:P32_BEFF + 2] = beff
    pk32[:, P32_LNW:P32_LNW + 2] = col(inp["ln_w"])
    pk32[:, P32_LNB:P32_LNB + 2] = col(inp["ln_b"])
    pk32[:, P32_FB:P32_FB + 2] = col(inp["fusion_b"])
    pk32[:, P32_BQ:P32_BQ + 2] = col(np.asarray(inp["bq"], f32) * s)
    pk32[:, P32_BK:P32_BK + 2] = col(inp["bk"])
    pk32[:, P32_IOTA:P32_IOTA + 1] = np.arange(128, dtype=f32)[:, None]
    pk32[:, P32_DWC:P32_DWC + 20] = dwc.reshape(128, -1)
    hi = pk32.astype(f16)
    lo = (pk32 - hi.astype(f32)).astype(f16)
    Wpk[:, C_P32HI:C_P32HI + P32_COLS] = hi.astype(f32)
    Wpk[:, C_P32LO:C_P32LO + P32_COLS] = lo.astype(f32)
    Wpk16 = Wpk.astype(f16)
    return Wpk16, xT16


def build_in_maps(inputs):
    Wpk16, xT16 = _host_prep(inputs)
    in_maps = []
    if USE_AG:
        gpack = np.concatenate([Wpk16, xT16[0], xT16[1]], axis=1)  # [128, AG_COLS]
        for core in range(8):
            in_maps.append(dict(
                pksl=np.ascontiguousarray(gpack[:, AG_SL * core:AG_SL * (core + 1)])))
    else:
        pk_b = [np.ascontiguousarray(np.concatenate([Wpk16, xT16[b]], axis=1))
                for b in range(B)]
        for core in range(8):
            in_maps.append(dict(pk16=pk_b[core & 1]))
    return in_maps


def _build(force_variant=None, use_ag=None):
    import concourse.mybir as mybir
    import concourse.tile as tile
    from concourse import bacc

    F32, F32R, F16 = mybir.dt.float32, mybir.dt.float32r, mybir.dt.float16
    U32 = mybir.dt.uint32
    AF = mybir.ActivationFunctionType
    ALU = mybir.AluOpType
    # 2*0x5f3759df + 1 as signed int32, for rsqrt seed (C2 + ~i) >> 1
    RSQRT_C2 = 0xBE6EB3BF - (1 << 32)

    use_ag = USE_AG if use_ag is None else use_ag

    nc = bacc.Bacc(num_devices=8) if use_ag else bacc.Bacc()

    if use_ag:
        pksl_d = nc.dram_tensor("pksl", [128, AG_SL], F16, kind="ExternalInput")
        ag_in = nc.dram_tensor("ag_in", [128, AG_SL], F16)
        ag_out = nc.dram_tensor("ag_out", [8, 128, AG_SL], F16, addr_space="Shared")
    else:
        pk16_d = nc.dram_tensor("pk16", [128, PK_COLS], F16, kind="ExternalInput")
    ag2_in = nc.dram_tensor("ag2_in", [128, 2048], F16)
    ag2_out = nc.dram_tensor("ag2_out", [8, 128, 2048], F16, addr_space="Shared")
    outd = nc.dram_tensor("out", [SLAB, D], F16, kind="ExternalOutput")

    X0 = W_COLS  # my-batch xT offset within pk

    with tile.TileContext(nc) as tc:
        pid = nc.partition_id() if force_variant is None else None
        with tc.tile_pool(name="wpool", bufs=1) as wpool, \
             tc.tile_pool(name="ppool", bufs=1) as ppool, \
             tc.tile_pool(name="work", bufs=1) as work, \
             tc.tile_pool(name="wk3", bufs=4) as wk3, \
             tc.tile_pool(name="pspool", bufs=1, space="PSUM") as pspool:
            psO = pspool

            pk = wpool.tile([128, PK_COLS], F16, name="pk")
            pk32 = wpool.tile([128, P32_COLS], F32, name="pk32s")
            if use_ag:
                nc.sync.dma_start(ag_in[:], pksl_d[:])
                nc.gpsimd.collective_compute(
                    "AllGather", ALU.bypass,
                    replica_groups=[list(range(8))],
                    ins=[ag_in[:]], outs=[ag_out[:]])

                def load_cols(dst_c0, g_c0, g_c1):
                    # copy global pack cols [g_c0,g_c1) into pk[:, dst_c0...]
                    for blk in range(8):
                        b0, b1 = blk * AG_SL, (blk + 1) * AG_SL
                        lo, hi = max(g_c0, b0), min(g_c1, b1)
                        if lo < hi:
                            nc.sync.dma_start(
                                pk[:, dst_c0 + lo - g_c0:dst_c0 + hi - g_c0],
                                ag_out[blk, :, lo - b0:hi - b0])

                load_cols(0, 0, W_COLS)

                # my batch's xT: binary branch tree on pid (batch = pid & 1)
                def xt_tree(lo_pid, hi_pid):
                    if hi_pid - lo_pid == 1:
                        bsel = lo_pid & 1
                        load_cols(X0, W_COLS + bsel * X_COLS,
                                  W_COLS + (bsel + 1) * X_COLS)
                        return
                    mid = (lo_pid + hi_pid) // 2
                    with tc.If(pid < mid) as cc:
                        xt_tree(lo_pid, mid)
                    with cc.Else():
                        xt_tree(mid, hi_pid)

                xt_tree(0, 8)
            else:
                nc.sync.dma_start(pk[:], pk16_d[:])

            # reconstruct the f32 side-pack from fp16 hi/lo halves
            p32lo = wpool.tile([128, P32_COLS], F32, name="p32lo")
            nc.vector.tensor_copy(out=pk32[:], in_=pk[:, C_P32HI:C_P32HI + P32_COLS])
            nc.vector.tensor_copy(out=p32lo[:], in_=pk[:, C_P32LO:C_P32LO + P32_COLS])
            nc.vector.tensor_add(out=pk32[:], in0=pk32[:], in1=p32lo[:])

            # ---- on-device constants ----
            ones32 = wpool.tile([128, 512], F32R, name="ones32")
            nc.vector.memset(ones32[:].bitcast(F32), 1.0)
            ones16 = wpool.tile([128, 512], F16, name="ones16")
            nc.vector.tensor_copy(out=ones16[:], in_=ones32[:])
            zeros8 = wpool.tile([128, 8], F32, name="zeros8")
            nc.vector.memset(zeros8[:], 0.0)
            eps_sb = wpool.tile([128, 1], F32, name="eps_sb")
            nc.vector.memset(eps_sb[:], EPS_LN)
            dsign_sb = wpool.tile([128, 4, 2], F32, name="dsign_sb")
            nc.vector.memset(dsign_sb[:, :, 0:1], -1.0)
            nc.vector.memset(dsign_sb[:, :, 1:2], 1.0)

            eye16 = pk[:, C_EYE:C_EYE + 128]
            eye32 = wpool.tile([128, 128], F32R, name="eye32")
            nc.vector.tensor_copy(out=eye32[:], in_=eye16)

            # iota row via M=1 matmul: out[0,j] = sum_k iota[k]*eye[k,j]
            rowp = pspool.tile([128, 128], F32, tag="E1", name="rowp")
            nc.tensor.matmul(rowp[0:1, 0:128], pk[:, C_IOTA:C_IOTA + 1], eye16,
                             start=True, stop=True)
            iota_r16 = wpool.tile([1, 128], F16, name="iota_r16")
            nc.vector.tensor_copy(out=iota_r16[:], in_=rowp[0:1, 0:128])
            iota_r32 = wpool.tile([1, 128], F32, name="iota_r32")
            nc.vector.tensor_copy(out=iota_r32[:], in_=rowp[0:1, 0:128])

            # decay bias row [1, 512]: dbias[bb*128+i] = DECAY*(i + 128*(bb-2))
            dbias16 = wpool.tile([1, 512], F16, name="dbias16")
            for bb in range(4):
                nc.vector.tensor_scalar(
                    out=dbias16[0:1, bb * 128:(bb + 1) * 128], in0=iota_r32[:],
                    scalar1=DECAY, scalar2=DECAY * 128.0 * (bb - 2),
                    op0=ALU.mult, op1=ALU.add)

            # bias rows [1, 256] from col layout via M=1 matmuls
            bv_row = wpool.tile([1, 256], F16, name="bv_row")
            bo_row = wpool.tile([1, 256], F16, name="bo_row")
            for dst, c0 in ((bv_row, C_BVC), (bo_row, C_BOC)):
                for o in range(2):
                    rp = pspool.tile([128, 128], F32, tag="E1", name="rowp2")
                    nc.tensor.matmul(rp[0:1, 0:128], pk[:, c0 + o:c0 + o + 1],
                                     eye16, start=True, stop=True)
                    nc.vector.tensor_copy(out=dst[0:1, o * 128:(o + 1) * 128],
                                          in_=rp[0:1, 0:128])

            # causal band mask T32[r, c] = NEG where c < r else 0
            Jps = pspool.tile([128, 128], F32, tag="E2", name="Jps")
            nc.tensor.matmul(Jps[:, 0:128], ones16[0:1, 0:128], iota_r16[0:1, 0:128],
                             start=True, stop=True)
            T32 = wpool.tile([128, 128], F32, name="T32")
            nc.vector.tensor_scalar(out=T32[:], in0=Jps[:, 0:128],
                                    scalar1=pk32[:, P32_IOTA:P32_IOTA + 1],
                                    scalar2=NEG, op0=ALU.is_lt, op1=ALU.mult)


            # wo tiles at partitions 0-31: [32, 8, 256]
            woT_sb = wpool.tile([32, 8, 256], F16, name="woT_sb")
            for h in range(8):
                nc.sync.dma_start(
                    woT_sb[0:32, h, :],
                    pk[32 * (h % 4):32 * (h % 4) + 32,
                       C_WO + (h // 4) * 256:C_WO + (h // 4) * 256 + 256])

            hT32 = ppool.tile([128, 2, 2, 4 + CW], F32, name="hT32")
            vT_s = ppool.tile([128, 2, 6 * 128], F16, name="vT_s")
            vT_t = ppool.tile([128, 2, 6 * 128], F16, name="vT_t")
            vn_s = ppool.tile([128, 6, 256], F32R, name="vn_s")
            vn_t = ppool.tile([128, 6, 256], F32R, name="vn_t")
            v_all = ppool.tile([128, L // 128, 8, 36], F32R, name="v_all")
            nc.vector.memset(v_all[:, :, :, 32:33].bitcast(F32), 1.0)
            v16 = ppool.tile([128, L // 128, 256], F16, name="v16")
            qT_all = ppool.tile([128, 2, L], F16, name="qT_all")
            kT_all = ppool.tile([128, 2, L], F16, name="kT_all")
            for _o in range(2):
                for _p in range(2):
                    nc.vector.memset(hT32[:, _o, _p, 0:4], 0.0)
            vT = {0: vT_s, 1: vT_t}
            vn = {0: vn_s, 1: vn_t}

            def emit_rsqrt(eng, y, x, tmp, niter=2):
                # y <- 1/sqrt(x) elementwise; x must be > 0 (pre-clamped).
                yi, xi, ti = y.bitcast(U32), x.bitcast(U32), tmp.bitcast(U32)
                nc.vector.tensor_scalar(out=ti, in0=xi, scalar1=0, scalar2=None,
                                        op0=ALU.bitwise_not)
                nc.vector.tensor_scalar(out=ti, in0=ti, scalar1=RSQRT_C2,
                                        scalar2=None, op0=ALU.add)
                nc.vector.tensor_scalar(out=yi, in0=ti, scalar1=1, scalar2=None,
                                        op0=ALU.logical_shift_right)
                for _ in range(niter):
                    eng.tensor_mul(out=tmp, in0=y, in1=y)
                    eng.tensor_mul(out=tmp, in0=tmp, in1=x)
                    eng.tensor_scalar(out=tmp, in0=tmp, scalar1=-0.5,
                                      scalar2=1.5, op0=ALU.mult, op1=ALU.add)
                    eng.tensor_mul(out=y, in0=y, in1=tmp)

            def emit_s1f(l0, w):
                # folded conv + y/sq activations (x read from resident pack)
                y = work.tile([128, 2, CW], F32R, tag="y_sb")
                sq = work.tile([128, 2, CW], F16, tag="sq_sb")
                for o in range(2):
                    ps = pspool.tile([128, CW], F32, tag="E1")
                    for t in range(5):
                        for k in range(2):
                            xc = X0 + k * (4 + L) + l0 + 4 - t
                            nc.tensor.matmul(ps[:, 0:w],
                                             pk[:, C_WTT + ((k * 5 + t) * 2 + o) * 128:
                                                C_WTT + ((k * 5 + t) * 2 + o) * 128 + 128],
                                             pk[:, xc:xc + w],
                                             start=(t == 0 and k == 0),
                                             stop=(t == 4 and k == 1))
                    nc.scalar.activation(y[:, o, 0:w], ps[:, 0:w], AF.Identity,
                                         bias=pk32[:, P32_BEFF + o:P32_BEFF + o + 1],
                                         scale=1.0)
                    nc.scalar.activation(sq[:, o, 0:w], ps[:, 0:w], AF.Square,
                                         bias=pk32[:, P32_BEFF + o:P32_BEFF + o + 1],
                                         scale=1.0)
                return y, sq

            def emit_s1b1(l0, w, y, sq):
                # layernorm moments via replicated-moment matmuls
                mps = pspool.tile([128, 2, CW], F32, tag="E2")
                for o in range(2):
                    nc.tensor.matmul(mps[:, 0, 0:w], ones32[:, 0:128], y[:, o, 0:w],
                                     start=(o == 0), stop=(o == 1))
                    nc.tensor.matmul(mps[:, 1, 0:w], ones16[:, 0:128], sq[:, o, 0:w],
                                     start=(o == 0), stop=(o == 1))
                mu = work.tile([128, CW], F32, tag="mu")
                nc.vector.tensor_scalar_mul(out=mu[:, 0:w], in0=mps[:, 0, 0:w], scalar1=1.0 / D)
                mu2 = work.tile([128, CW], F32, tag="tmpA")
                nc.vector.tensor_mul(out=mu2[:, 0:w], in0=mu[:, 0:w], in1=mu[:, 0:w])
                var = work.tile([128, CW], F32, tag="tmpB")
                nc.vector.scalar_tensor_tensor(out=var[:, 0:w], in0=mps[:, 1, 0:w],
                                               scalar=1.0 / D, in1=mu2[:, 0:w],
                                               op0=ALU.mult, op1=ALU.subtract)
                return mu, var

            def emit_s1b2(l0, w, y, mu, var):
                par = (l0 // CW) & 1
                c0 = l0 % CW
                lnv = work.tile([128, CW], F32, tag="tmpA")
                nc.scalar.activation(lnv[:, 0:w], var[:, 0:w], AF.Ln, bias=eps_sb[:])
                rstd = work.tile([128, CW], F32, tag="tmpB")
                nc.scalar.activation(rstd[:, 0:w], lnv[:, 0:w], AF.Exp, scale=-0.5)
                for o in range(2):
                    t1 = work.tile([128, CW], F32, tag="tmpA" if o else "tmpC")
                    nc.vector.tensor_sub(out=t1[:, 0:w], in0=y[:, o, 0:w], in1=mu[:, 0:w])
                    nc.vector.tensor_mul(out=t1[:, 0:w], in0=t1[:, 0:w], in1=rstd[:, 0:w])
                    nc.vector.tensor_scalar(out=hT32[:, o, par, 4 + c0:4 + c0 + w],
                                            in0=t1[:, 0:w],
                                            scalar1=pk32[:, P32_LNW + o:P32_LNW + o + 1],
                                            scalar2=pk32[:, P32_LNB + o:P32_LNB + o + 1],
                                            op0=ALU.mult, op1=ALU.add)
                if c0 + w == CW:
                    nc.vector.tensor_copy(out=hT32[:, :, 1 - par, 0:4],
                                          in_=hT32[:, :, par, CW:CW + 4])

            def emit_s2(l0, w):
                par = (l0 // CW) & 1
                c0 = l0 % CW
                nblk = w // 128
                blk6 = lambda b: (l0 // 128 + b) % 6
                # ---- trend taps: P = t3+t5, Dt = t3-t5 (per-channel shift
                # chains on DVE/GpSimd; beats diag matmuls in this runtime) ----
                wsT = work.tile([128, 2, CW], F32R, tag="bufA")
                wtTt = work.tile([128, 2, CW], F32R, tag="bufB")
                dnT = work.tile([128, 2, CW], F32R, tag="bufC")
                for o in range(2):
                    for t in range(5):
                        src = hT32[:, o, par, 4 + c0 - t:4 + c0 - t + w]
                        cP = pk32[:, P32_DWC + t * 2 + o:P32_DWC + t * 2 + o + 1]
                        cD = pk32[:, P32_DWC + 10 + t * 2 + o:P32_DWC + 10 + t * 2 + o + 1]
                        if t == 0:
                            nc.vector.tensor_scalar(out=wtTt[:, o, 0:w], in0=src,
                                                    scalar1=cP, scalar2=None,
                                                    op0=ALU.mult)
                            nc.vector.tensor_scalar(out=dnT[:, o, 0:w], in0=src,
                                                    scalar1=cD, scalar2=None,
                                                    op0=ALU.mult)
                        else:
                            nc.vector.scalar_tensor_tensor(
                                out=wtTt[:, o, 0:w], in0=src, scalar=cP,
                                in1=wtTt[:, o, 0:w], op0=ALU.mult, op1=ALU.add)
                            nc.vector.scalar_tensor_tensor(
                                out=dnT[:, o, 0:w], in0=src, scalar=cD,
                                in1=dnT[:, o, 0:w], op0=ALU.mult, op1=ALU.add)
                    nc.vector.scalar_tensor_tensor(out=wsT[:, o, 0:w],
                                                   in0=hT32[:, o, par, 4 + c0:4 + c0 + w],
                                                   scalar=2.0, in1=wtTt[:, o, 0:w],
                                                   op0=ALU.mult, op1=ALU.subtract)
                # ---- transpose routing inputs to [l, c] ----
                wsn = work.tile([128, 4, 256], F32, tag="wsn")
                wtn = work.tile([128, 4, 256], F32, tag="wtn")
                dnn = work.tile([128, 4, 256], F32, tag="dnn")
                for srct, dst, use_act in ((wsT, wsn, False), (wtTt, wtn, True),
                                           (dnT, dnn, True)):
                    for bi in range(nblk):
                        pst = pspool.tile([128, 2, 128], F32R, tag="E1")
                        for o in range(2):
                            nc.tensor.transpose(pst[:, o, :], srct[:, o, bi * 128:(bi + 1) * 128], eye32[:])
                        if use_act:
                            nc.scalar.activation(dst[:, bi, :], pst[:],
                                                 AF.Copy, bias=0.0, scale=1.0)
                        else:
                            nc.vector.tensor_copy(out=dst[:, bi, :], in_=pst[:])
                yield
                # ---- routing invariants (st = wn + w*dnn):
                #   A_p = sum wn_p^2, B_p = sum dnn*wn_p, C = sum dnn^2
                # then per-iteration sums are analytic:
                #   S(w) = A + w*(B + D(w)),  D(w) = B + w*C.
                g = work.tile([128, 16, 4, 2], F32, tag="g")
                diff = work.tile([128, 4, 2], F32, tag="diff")
                scrA = work.tile([128, 4, 256], F32, tag="bufB", name="scrA")
                scrB = work.tile([128, 4, 256], F32, tag="bufA", name="scrB")
                nb = nblk
                nc.vector.tensor_mul(out=scrA[:, 0:nb, :], in0=wsn[:, 0:nb, :],
                                     in1=wsn[:, 0:nb, :])
                nc.vector.tensor_reduce(out=g[:, 0, 0:nb, 0:1], in_=scrA[:, 0:nb, :],
                                        axis=mybir.AxisListType.X, op=ALU.add)
                nc.vector.tensor_mul(out=scrB[:, 0:nb, :], in0=wtn[:, 0:nb, :],
                                     in1=wtn[:, 0:nb, :])
                nc.vector.tensor_reduce(out=g[:, 0, 0:nb, 1:2], in_=scrB[:, 0:nb, :],
                                        axis=mybir.AxisListType.X, op=ALU.add)
                nc.vector.tensor_mul(out=scrA[:, 0:nb, :], in0=dnn[:, 0:nb, :],
                                     in1=dnn[:, 0:nb, :])
                nc.vector.tensor_reduce(out=g[:, 2, 0:nb, 0:1], in_=scrA[:, 0:nb, :],
                                        axis=mybir.AxisListType.X, op=ALU.add)
                nc.vector.tensor_copy(out=g[:, 2, 0:nb, 1:2], in_=g[:, 2, 0:nb, 0:1])
                nc.vector.tensor_mul(out=scrB[:, 0:nb, :], in0=dnn[:, 0:nb, :],
                                     in1=wsn[:, 0:nb, :])
                nc.vector.tensor_reduce(out=g[:, 1, 0:nb, 0:1], in_=scrB[:, 0:nb, :],
                                        axis=mybir.AxisListType.X, op=ALU.add)
                nc.vector.tensor_mul(out=scrA[:, 0:nb, :], in0=dnn[:, 0:nb, :],
                                     in1=wtn[:, 0:nb, :])
                nc.vector.tensor_reduce(out=g[:, 1, 0:nb, 1:2], in_=scrA[:, 0:nb, :],
                                        axis=mybir.AxisListType.X, op=ALU.add)
                yield
                for it in range(3):
                    if it == 0:
                        S, Dr = 0, 1
                    else:
                        # D = B + w*C ; S = A + w*(B + D)
                        nc.vector.tensor_mul(out=g[:, 4, 0:nb, :], in0=g[:, 15, 0:nb, :], in1=g[:, 2, 0:nb, :])
                        nc.vector.tensor_add(out=g[:, 4, 0:nb, :], in0=g[:, 1, 0:nb, :], in1=g[:, 4, 0:nb, :])
                        nc.vector.tensor_add(out=g[:, 5, 0:nb, :], in0=g[:, 1, 0:nb, :], in1=g[:, 4, 0:nb, :])
                        nc.vector.tensor_mul(out=g[:, 5, 0:nb, :], in0=g[:, 15, 0:nb, :], in1=g[:, 5, 0:nb, :])
                        nc.vector.tensor_add(out=g[:, 3, 0:nb, :], in0=g[:, 0, 0:nb, :], in1=g[:, 5, 0:nb, :])
                        S, Dr = 3, 4
                    # squash scale from S: g10 = 0.125*S / ((0.25*S+1)*(0.5*sqrt(S)+1e-9))
                    nc.vector.tensor_scalar_max(out=g[:, 6, 0:nb, :], in0=g[:, S, 0:nb, :],
                                                scalar1=1e-30)
                    emit_rsqrt(nc.vector, g[:, 7, 0:nb, :], g[:, 6, 0:nb, :], g[:, 8, 0:nb, :], niter=1)
                    nc.vector.tensor_mul(out=g[:, 8, 0:nb, :], in0=g[:, 6, 0:nb, :], in1=g[:, 7, 0:nb, :])
                    nc.vector.tensor_scalar(out=g[:, 9, 0:nb, :], in0=g[:, 6, 0:nb, :],
                                            scalar1=0.25, scalar2=1.0,
                                            op0=ALU.mult, op1=ALU.add)
                    nc.vector.tensor_scalar(out=g[:, 8, 0:nb, :], in0=g[:, 8, 0:nb, :],
                                            scalar1=0.5, scalar2=1e-9,
                                            op0=ALU.mult, op1=ALU.add)
                    nc.vector.tensor_mul(out=g[:, 9, 0:nb, :], in0=g[:, 9, 0:nb, :], in1=g[:, 8, 0:nb, :])
                    nc.vector.reciprocal_approx_fast(out=g[:, 13, 0:nb, :], in_=g[:, 9, 0:nb, :])
                    nc.vector.scalar_tensor_tensor(out=g[:, 10, 0:nb, :], in0=g[:, 6, 0:nb, :],
                                                   scalar=0.125, in1=g[:, 13, 0:nb, :],
                                                   op0=ALU.mult, op1=ALU.mult)
                    if it < 2:
                        # logit update u = D*scale ; diff += dsign*u ; w = dsign*tanh(diff/2)
                        nc.vector.tensor_mul(out=g[:, 5, 0:nb, :], in0=g[:, Dr, 0:nb, :], in1=g[:, 10, 0:nb, :])
                        if it == 0:
                            nc.vector.tensor_mul(out=diff[:, 0:nb, :], in0=g[:, 5, 0:nb, :], in1=dsign_sb[:, 0:nb, :])
                        else:
                            nc.vector.tensor_mul(out=g[:, 14, 0:nb, :], in0=g[:, 5, 0:nb, :], in1=dsign_sb[:, 0:nb, :])
                            nc.vector.tensor_add(out=diff[:, 0:nb, :], in0=diff[:, 0:nb, :], in1=g[:, 14, 0:nb, :])
                        nc.scalar.activation(g[:, 14, 0:nb, :], diff[:, 0:nb, :], AF.Tanh, scale=0.5)
                        nc.vector.tensor_mul(out=g[:, 15, 0:nb, :], in0=g[:, 14, 0:nb, :], in1=dsign_sb[:, 0:nb, :])
                        yield
                    else:
                        # vn = (wn + w*dnn)*scale = wn*a + dnn*b, b = w*a
                        nc.vector.tensor_mul(out=g[:, 5, 0:nb, :], in0=g[:, 15, 0:nb, :],
                                             in1=g[:, 10, 0:nb, :])
                        for bi in range(nblk):
                            nc.vector.tensor_scalar(
                                out=scrA[:, bi, :], in0=wsn[:, bi, :],
                                scalar1=g[:, 10, bi, 0:1], scalar2=None,
                                op0=ALU.mult)
                            nc.vector.scalar_tensor_tensor(
                                out=vn[0][:, blk6(bi), :], in0=dnn[:, bi, :],
                                scalar=g[:, 5, bi, 0:1], in1=scrA[:, bi, :],
                                op0=ALU.mult, op1=ALU.add)
                            nc.vector.tensor_scalar(
                                out=scrB[:, bi, :], in0=wtn[:, bi, :],
                                scalar1=g[:, 10, bi, 1:2], scalar2=None,
                                op0=ALU.mult)
                            nc.vector.scalar_tensor_tensor(
                                out=vn[1][:, blk6(bi), :], in0=dnn[:, bi, :],
                                scalar=g[:, 5, bi, 1:2], in1=scrB[:, bi, :],
                                op0=ALU.mult, op1=ALU.add)
                # ---- transpose v to vT (rolling 6-block window) ----
                for prob in range(2):
                    for bi in range(nblk):
                        pst = pspool.tile([128, 2, 128], F32R, tag="E1")
                        for o in range(2):
                            nc.tensor.transpose(pst[:, o, :], vn[prob][:, blk6(bi), o * 128:(o + 1) * 128], eye32[:])
                        m = blk6(bi)
                        if prob == 0:
                            nc.scalar.activation(vT[prob][:, :, m * 128:(m + 1) * 128],
                                                 pst[:], AF.Copy, bias=0.0, scale=1.0)
                        else:
                            nc.vector.tensor_copy(out=vT[prob][:, :, m * 128:(m + 1) * 128], in_=pst[:])
                yield
                # ---- banded time attention ----
                sfT = work.tile([128, 2, CW], F16, tag="bufA")
                tfT = work.tile([128, 2, CW], F16, tag="bufB")
                for prob in range(2):
                    vTt, vnt = vT[prob], vn[prob]
                    dstT = sfT if prob == 0 else tfT
                    q0 = 0
                    while q0 < w:
                        qw = min(256, w - q0)
                        Q0 = l0 + q0
                        mq = ((Q0 // 128) % 6) * 128
                        bbs = [bb for bb in range(1, 4)
                               if Q0 + 128 * (bb - 2) >= seg_start[0]
                               and 128 * (bb - 2) < qw]
                        Pt = wk3.tile([128, 4, 256], F32R, tag="Pbuf")
                        zones = {}
                        for bb in bbs:
                            zones.setdefault(bb // 2, []).append(bb)
                        for z, zbbs in zones.items():
                            base = zbbs[0]
                            Sps = pspool.tile([128, 2, 256], F32, tag="SbigZ",
                                              name=f"Sps_{z}")
                            for bb in zbbs:
                                K0 = Q0 + 128 * (bb - 2)
                                mk = ((K0 // 128) % 6) * 128
                                lo = max(0, 128 * (bb - 2))
                                nc.tensor.matmul(Sps[:, bb - base, 0:qw],
                                                 dbias16[0:1, bb * 128:(bb + 1) * 128],
                                                 ones16[0:1, 0:qw],
                                                 start=(bb == zbbs[0]), stop=False)
                                for o in range(2):
                                    nc.tensor.matmul(Sps[:, bb - base, lo:qw],
                                                     vTt[:, o, mk:mk + 128],
                                                     vTt[:, o, mq + lo:mq + qw],
                                                     start=False,
                                                     stop=(bb == zbbs[-1] and o == 1))
                            for bb in zbbs:
                                if bb >= 2:
                                    dlo = 128 * (bb - 2)
                                    dwdt = min(qw, dlo + 128) - dlo
                                    nc.vector.tensor_add(out=Sps[:, bb - base, dlo:dlo + dwdt],
                                                         in0=Sps[:, bb - base, dlo:dlo + dwdt],
                                                         in1=T32[:, 0:dwdt])
                            nc.scalar.activation(Pt[:, base:base + len(zbbs), 0:qw],
                                                 Sps[:, 0:len(zbbs), 0:qw], AF.Exp)
                        od = psO.tile([128, 3, 256], F32, tag="OD")
                        for bb in bbs:
                            K0 = Q0 + 128 * (bb - 2)
                            kb6 = (K0 // 128) % 6
                            lo = max(0, 128 * (bb - 2))
                            first, last = bb == bbs[0], bb == bbs[-1]
                            for o in range(2):
                                nc.tensor.matmul(od[:, o, lo:qw],
                                                 vnt[:, kb6, o * 128:(o + 1) * 128],
                                                 Pt[:, bb, lo:qw],
                                                 start=(first and o == 0),
                                                 stop=(last and o == 1))
                            nc.tensor.matmul(od[:, 2, lo:qw], ones32[:, 0:128],
                                             Pt[:, bb, lo:qw],
                                             start=first, stop=last)
                        rec = work.tile([128, 256], F32, tag="tmpB")
                        nc.vector.reciprocal_approx_fast(out=rec[:, 0:qw], in_=od[:, 2, 0:qw])
                        for o in range(2):
                            nc.vector.tensor_mul(out=dstT[:, o, q0:q0 + qw],
                                                 in0=od[:, o, 0:qw], in1=rec[:, 0:qw])
                        q0 += qw
                yield
                # ---- fusion + qkv ----
                fused = work.tile([128, 2, CW], F16, tag="bufC")
                for o in range(2):
                    psl = pspool.tile([128, 2, CW], F32, tag="L3", name="ps_fus")
                    ps = psl[:, 0]
                    for k in range(2):
                        nc.tensor.matmul(ps[:, 0:w],
                                         pk[:, C_FWT + (k * 2 + o) * 128:C_FWT + (k * 2 + o) * 128 + 128],
                                         sfT[:, k, 0:w],
                                         start=(k == 0), stop=False)
                        nc.tensor.matmul(ps[:, 0:w],
                                         pk[:, C_FWT + ((2 + k) * 2 + o) * 128:C_FWT + ((2 + k) * 2 + o) * 128 + 128],
                                         tfT[:, k, 0:w],
                                         start=False, stop=(k == 1))
                    nc.scalar.activation(fused[:, o, 0:w], ps[:, 0:w], AF.Identity,
                                         bias=pk32[:, P32_FB + o:P32_FB + o + 1], scale=1.0)
                for o in range(2):
                    psqk = pspool.tile([128, 2, CW], F32, tag="L3", name="psqk")
                    for k in range(2):
                        nc.tensor.matmul(psqk[:, 0, 0:w],
                                         pk[:, C_WQT + (k * 2 + o) * 128:C_WQT + (k * 2 + o) * 128 + 128],
                                         fused[:, k, 0:w],
                                         start=(k == 0), stop=(k == 1))
                        nc.tensor.matmul(psqk[:, 1, 0:w],
                                         pk[:, C_WKT + (k * 2 + o) * 128:C_WKT + (k * 2 + o) * 128 + 128],
                                         fused[:, k, 0:w],
                                         start=(k == 0), stop=(k == 1))
                    nc.scalar.activation(qT_all[:, o, l0:l0 + w], psqk[:, 0, 0:w], AF.Identity,
                                         bias=pk32[:, P32_BQ + o:P32_BQ + o + 1], scale=1.0)
                    nc.scalar.activation(kT_all[:, o, l0:l0 + w], psqk[:, 1, 0:w], AF.Identity,
                                         bias=pk32[:, P32_BK + o:P32_BK + o + 1], scale=1.0)
                for bi in range(nblk):
                    psvl = pspool.tile([128, 2, CW], F32, tag="L3", name="psv")
                    psv = psvl[:, 0, 0:256]
                    for k in range(2):
                        nc.tensor.matmul(psv, fused[:, k, bi * 128:(bi + 1) * 128],
                                         pk[:, C_WVT + k * 256:C_WVT + k * 256 + 256],
                                         start=(k == 0), stop=False)
                    nc.tensor.matmul(psv, ones16[0:1, 0:128], bv_row[0:1, 0:256],
                                     start=False, stop=True)
                    nc.vector.tensor_copy(out=v16[:, l0 // 128 + bi, :], in_=psv)

            def drive(chunks):
                # software-pipelined emission: chunk i+1's conv/LN instruction
                # stream is interleaved into chunk i's routing stream so PE/Act
                # have work while the serial routing chain runs on DVE.
                s1 = emit_s1f(*chunks[0])
                mv = emit_s1b1(*chunks[0], *s1)
                emit_s1b2(chunks[0][0], chunks[0][1], s1[0], *mv)
                for i, c in enumerate(chunks):
                    gen = emit_s2(*c)
                    nxt = chunks[i + 1] if i + 1 < len(chunks) else None
                    if nxt:
                        s1 = emit_s1f(*nxt)
                    next(gen)            # trend taps + transposes
                    if nxt:
                        mv = emit_s1b1(*nxt, *s1)
                    next(gen)            # invariants
                    if nxt:
                        emit_s1b2(nxt[0], nxt[1], s1[0], *mv)
                    next(gen)            # iteration 0
                    next(gen)            # iteration 1
                    for _ in gen:        # it2 + vn + vT, banded, fusion, qkv
                        pass

            def emit_mha(qlo, qhi):
                for Q0 in range(qlo, qhi, 512):
                    qw = min(512, qhi - Q0)
                    nkv = (Q0 + qw) // 128
                    ohall = work.tile([32, 4, 2, 512], F16, tag="bufA",
                                      name="ohall")
                    for hp in range(4):
                        hg = hp // 2
                        rows = [32 * ((2 * hp) % 4), 32 * ((2 * hp + 1) % 4)]
                        Oh = psO.tile([33, 2, 512], F32, tag="OD", name=f"Oh_{hp}")
                        sps = pspool.tile([128, 2, 512], F32, tag="L3", name="sps")
                        sps_z = pspool.tile([128, 512], F32, tag="SbigZ", name="sps_z")
                        slots = [sps[:, 0], sps[:, 1], sps_z[:]]
                        step = 0
                        pend = None

                        def flush(p):
                            kb_, jj_, Pm_, lo_, fi_, la_ = p
                            h_ = 2 * hp + jj_
                            nc.tensor.matmul(Oh[:, jj_, lo_:qw],
                                             v_all[:, kb_, h_, 0:33],
                                             Pm_[:, lo_:qw], start=fi_, stop=la_)

                        for kb in range(nkv):
                            K0 = kb * 128
                            dlt = K0 - Q0
                            lo = max(0, dlt)
                            dwdt = min(qw, dlt + 128) - dlt if dlt >= 0 else 0
                            first, last = kb == 0, kb == nkv - 1
                            for jj in range(2):
                                sp = slots[step % len(slots)]
                                step += 1
                                nc.tensor.matmul(sp[:, lo:qw],
                                                 kT_all[rows[jj]:rows[jj] + 32, hg, K0:K0 + 128],
                                                 qT_all[rows[jj]:rows[jj] + 32, hg, Q0 + lo:Q0 + qw],
                                                 start=True, stop=True,
                                                 tile_position=(rows[jj], 0))
                                if dlt >= 0:
                                    nc.vector.tensor_add(out=sp[:, dlt:dlt + dwdt],
                                                         in0=sp[:, dlt:dlt + dwdt],
                                                         in1=T32[:, 0:dwdt])
                                Pm = wk3.tile([128, 512], F32R, tag="Pbuf", name="Pm")
                                nc.scalar.activation(Pm[:, lo:qw], sp[:, lo:qw], AF.Exp)
                                if pend is not None:
                                    flush(pend)
                                pend = (kb, jj, Pm, lo, first, last)
                        flush(pend)
                        # denominator row 32 -> partition 0, reciprocal there,
                        # broadcast to 32 partitions via a K=1 matmul
                        den = work.tile([33, 2, 512], F32, tag="tmpC", name="den_m")
                        nc.vector.tensor_copy(out=den[32:33, :, 0:qw],
                                              in_=Oh[32:33, :, 0:qw])
                        d0 = work.tile([1, 2, 512], F32, tag="r0row", name="d0row")
                        nc.sync.dma_start(d0[0:1, :, 0:qw], den[32:33, :, 0:qw])
                        r16 = work.tile([1, 2, 512], F16, tag="r1row", name="r16row")
                        nc.vector.reciprocal_approx_fast(out=d0[0:1, :, 0:qw],
                                                         in_=d0[0:1, :, 0:qw])
                        nc.vector.tensor_copy(out=r16[0:1, :, 0:qw],
                                              in_=d0[0:1, :, 0:qw])
                        recBp = pspool.tile([32, 2, 512], F32, tag="E2", name="recBp")
                        for jj in range(2):
                            nc.tensor.matmul(recBp[:, jj, 0:qw], ones16[0:1, 0:32],
                                             r16[0:1, jj, 0:qw], start=True, stop=True)
                        recB = work.tile([32, 2, 512], F32, tag="bufC", name="recB_sb")
                        nc.vector.tensor_copy(out=recB[:, :, 0:qw], in_=recBp[:, :, 0:qw])
                        nc.vector.tensor_mul(out=ohall[:, hp, :, 0:qw],
                                             in0=Oh[0:32, :, 0:qw],
                                             in1=recB[:, :, 0:qw])
                        if Q0 == 0:
                            nc.vector.tensor_copy(out=ohall[:, hp, :, 0:1],
                                                  in_=zeros8[0:32, 0:2].unsqueeze(-1))
                    # out projection: one 9-matmul PSUM accumulation per block
                    for bi in range(qw // 128):
                        psop = pspool.tile([128, 256], F32, tag="E1", name="psop")
                        for hp in range(4):
                            for jj in range(2):
                                nc.tensor.matmul(psop[:],
                                                 ohall[:, hp, jj, bi * 128:(bi + 1) * 128],
                                                 woT_sb[0:32, 2 * hp + jj, :],
                                                 start=(hp == 0 and jj == 0), stop=False)
                        nc.tensor.matmul(psop[:], ones16[0:1, 0:128], bo_row[0:1, 0:256],
                                         start=False, stop=True)
                        ot = work.tile([128, 256], F16, tag="tmpC", name="ot16")
                        nc.vector.tensor_copy(out=ot[:], in_=psop[:])
                        nc.sync.dma_start(outd[Q0 - qlo + bi * 128:Q0 - qlo + (bi + 1) * 128, :], ot[:])

            seg_start = [0]

            def emit_pipeline(vi):
                # position-local pipeline over [seg0, hi): own slab + one
                # 512-wide halo chunk (band reach 256 + conv/trend taps);
                # pack this slab's K and V for the cross-core AllGather.
                lo, hi = ABOUNDS[vi], ABOUNDS[vi + 1]
                seg0 = max(0, lo - 512)
                seg_start[0] = seg0
                chunks = []
                l0 = seg0
                while l0 < hi:
                    w = min(CW, hi - l0)
                    chunks.append((l0, w))
                    l0 += w
                drive(chunks)
                if force_variant is None:
                    nc.sync.dma_start(ag2_in[:, 0:1024], kT_all[:, :, lo:hi])
                    nc.sync.dma_start(ag2_in[:, 1024:2048],
                                      v16[:, 4 * vi:4 * vi + 4, :])

            def emit_variant(vi):
                # single-core path for TimelineSim: no collectives
                emit_pipeline(vi)
                for h in range(8):
                    nc.vector.tensor_copy(out=v_all[:, :, h, 0:32],
                                          in_=v16[:, :, h * 32:(h + 1) * 32])
                emit_mha(ABOUNDS[vi], ABOUNDS[vi + 1])

            if force_variant is not None:
                emit_variant(force_variant)
            else:
                with tc.If(pid < 2) as c0:
                    emit_pipeline(0)
                with c0.Else():
                    with tc.If(pid < 4) as c1:
                        emit_pipeline(1)
                    with c1.Else():
                        with tc.If(pid < 6) as c2:
                            emit_pipeline(2)
                        with c2.Else():
                            emit_pipeline(3)

                # cross-core K/V AllGather (top level: no control flow)
                nc.gpsimd.collective_compute(
                    "AllGather", ALU.bypass,
                    replica_groups=[list(range(8))],
                    ins=[ag2_in[:]], outs=[ag2_out[:]])

                # unpack the 4 slabs of my batch (batch = pid & 1)
                def unpack(b):
                    for c in (b, b + 2, b + 4, b + 6):
                        w0 = 512 * (c // 2)
                        nc.sync.dma_start(kT_all[:, :, w0:w0 + 512],
                                          ag2_out[c, :, 0:1024])
                        nc.sync.dma_start(v16[:, w0 // 128:w0 // 128 + 4, :],
                                          ag2_out[c, :, 1024:2048])

                def up_tree(lo_pid, hi_pid):
                    if hi_pid - lo_pid == 1:
                        unpack(lo_pid & 1)
                        return
                    mid = (lo_pid + hi_pid) // 2
                    with tc.If(pid < mid) as cc:
                        up_tree(lo_pid, mid)
                    with cc.Else():
                        up_tree(mid, hi_pid)

                up_tree(0, 8)
                for h in range(8):
                    nc.vector.tensor_copy(out=v_all[:, :, h, 0:32],
                                          in_=v16[:, :, h * 32:(h + 1) * 32])

                with tc.If(pid < 2) as d0:
                    emit_mha(ABOUNDS[0], ABOUNDS[1])
                with d0.Else():
                    with tc.If(pid < 4) as d1:
                        emit_mha(ABOUNDS[1], ABOUNDS[2])
                    with d1.Else():
                        with tc.If(pid < 6) as d2:
                            emit_mha(ABOUNDS[2], ABOUNDS[3])
                        with d2.Else():
                            emit_mha(ABOUNDS[3], ABOUNDS[4])
    nc.finalize()
    return nc


_CACHE = {}


def kernel(**inputs):
    from concourse.bass_utils import run_bass_kernel_spmd
    in_maps = build_in_maps(inputs)
    if "nc" not in _CACHE:
        _CACHE["nc"] = _build()
    nc = _CACHE["nc"]
    res = run_bass_kernel_spmd(nc, in_maps, core_ids=list(range(8)))
    out = np.zeros((B, L, D), np.float32)
    for core in range(8):
        b = core & 1
        vi = core // 2
        lo, hi = ABOUNDS[vi], ABOUNDS[vi + 1]
        out[b, lo:hi, :] = res.results[core]["out"].astype(np.float32)
    return out


# revision 28
# speedup vs baseline: 1.1094x; 1.0347x over previous
# Trainium2 Bass kernel for nn_BAKTTime (dense_transformer).
# Self-contained: hardcodes shapes B=2, L=2048, D=256, H=8, dk=32.
#
# Sharding: 8 cores, SPMD program. core i handles batch (i & 1) and query
# slab (i // 2); slab j computes the position-local pipeline (folded
# 5-tap conv + layernorm + capsule routing + banded time attention + fusion
# + qkv) over the causal prefix [0, 512*(j+1)), then flash MHA over q in
# [512*j, 512*(j+1)).  The wall-clock of this problem is dominated by
# host<->device transfer over the axon tunnel, so all inputs are packed
# into one fp16 [128, C] tensor per core; with USE_AG each core ships only
# a 1/8 column slice and an on-device AllGather reconstructs the full
# pack.  Constant matrices (identity, causal mask, decay bias, bias rows,
# depthwise-diag) are built on device.  Output is a per-core fp16
# (512, 256) slab; the host stitches the 8 slabs.
import numpy as np

B, L, D = 2, 2048, 256
H, DK = 8, 32
DECAY = 0.2
EPS_LN = 1e-12
NEG = -1e30
CW = 512
ABOUNDS = (0, 512, 1024, 1536, 2048)
SLAB = 512

USE_AG = True  # AllGather weights+x on device (ship 1/8 per core)

# ---- fp16 pack column layout (single source of truth) ----
C_WTT = 0                      # [2(k),5(t),2(o),128]
C_FWT = C_WTT + 2560           # [4(k),2(o),128]
C_WQT = C_FWT + 1024           # [2(k),2(o),128]
C_WKT = C_WQT + 512
C_WVT = C_WKT + 512            # [2(k),256]
C_WO = C_WVT + 512             # head h at partitions [32*(h%4),+32), col (h//4)*256
C_EYE = C_WO + 512             # [128,128] identity
C_DWC = C_EYE + 128            # [2(pd),5(t),2(o)] depthwise tap coefs
C_IOTA = C_DWC + 20            # [128,1] iota
C_BVC = C_IOTA + 1             # [2] bv col layout
C_BOC = C_BVC + 2              # [2] bo col layout
C_P32HI = C_BOC + 2            # [33] fp16 hi half of the f32 pack
C_P32LO = C_P32HI + 33         # [33] fp16 lo half (v - f32(hi))
W_COLS = 5888                  # C_P32LO+33 = 5851, padded to 5888
X_COLS = 2 * (4 + L)           # 4104
PK_COLS = W_COLS + X_COLS      # 9896 (per-core pack: weights + my batch xT)
AG_COLS = W_COLS + 2 * X_COLS  # 14000 (global pack: weights + both batches)
AG_SL = AG_COLS // 8           # 1750

# pack32 f32 [128, 33]
P32_BEFF, P32_LNW, P32_LNB, P32_FB, P32_BQ, P32_BK, P32_IOTA, P32_DWC = 0, 2, 4, 6, 8, 10, 12, 13
P32_COLS = 33


def _host_prep(inp):
    f32, f16 = np.float32, np.float16
    x = np.asarray(inp["x"], f32)
    c3w, c3b = np.asarray(inp["conv3_w"], f32), np.asarray(inp["conv3_b"], f32)
    c5w, c5b = np.asarray(inp["conv5_w"], f32), np.asarray(inp["conv5_b"], f32)
    b3 = np.asarray(inp["beta3"], f32).reshape(D)
    b5 = np.asarray(inp["beta5"], f32).reshape(D)
    cw = np.asarray(inp["combine_w"], f32)
    cwt = np.exp(cw - cw.max())
    cwt = (cwt / cwt.sum()).astype(f32)
    g3 = (cwt[0] * (1.0 - b3 ** 2)).astype(f32)
    g5 = (cwt[1] * (1.0 - b5 ** 2)).astype(f32)
    dscale = (1.0 + cwt[0] * b3 ** 2 + cwt[1] * b5 ** 2).astype(f32)
    W = np.zeros((5, D, D), f32)
    W[0] = g3[:, None] * c3w[:, :, 2] + g5[:, None] * c5w[:, :, 4] + np.diag(dscale)
    W[1] = g3[:, None] * c3w[:, :, 1] + g5[:, None] * c5w[:, :, 3]
    W[2] = g3[:, None] * c3w[:, :, 0] + g5[:, None] * c5w[:, :, 2]
    W[3] = g5[:, None] * c5w[:, :, 1]
    W[4] = g5[:, None] * c5w[:, :, 0]
    # lhsT layout [din_par, din_ch(k), tap, o_ch, dout]
    wtT = np.transpose(W, (0, 2, 1)).reshape(5, 2, 128, 2, 128)
    wtT = np.ascontiguousarray(np.transpose(wtT, (2, 1, 0, 3, 4)))  # [128,2,5,2,128]
    beff = (g3 * c3b + g5 * c5b).reshape(2, 128).T.copy()           # [128, 2]
    dw3 = np.asarray(inp["dw3_w"], f32)[:, 0, :]
    dw5 = np.asarray(inp["dw5_w"], f32)[:, 0, :]
    c3l = np.zeros((5, D), f32)
    c5l = np.zeros((5, D), f32)
    for l in range(3):
        c3l[l] = dw3[:, 2 - l]
    for l in range(5):
        c5l[l] = dw5[:, 4 - l]
    pco, dco = c3l + c5l, c3l - c5l                                  # [5, 256]
    # dwc [128, 2(pd), 5(t), 2(o)]
    dwc = np.zeros((128, 2, 5, 2), f32)
    for t in range(5):
        for o in range(2):
            dwc[:, 0, t, o] = pco[t, o * 128:(o + 1) * 128]
            dwc[:, 1, t, o] = dco[t, o * 128:(o + 1) * 128]
    col = lambda v: np.asarray(v, f32).reshape(2, 128).T.copy()      # [128, 2]
    fwT = np.asarray(inp["fusion_w"], f32).T.reshape(4, 128, 2, 128)
    fwT = np.ascontiguousarray(np.transpose(fwT, (1, 0, 2, 3)))      # [128,4,2,128]
    s = 1.0 / np.sqrt(DK)

    def sqT(w):
        t = np.asarray(w, f32).T.reshape(2, 128, 2, 128)
        return np.ascontiguousarray(np.transpose(t, (1, 0, 2, 3)))   # [128,2,2,128]

    def hvT(w):
        t = np.asarray(w, f32).T.reshape(2, 128, 256)
        return np.ascontiguousarray(np.transpose(t, (1, 0, 2)))      # [128,2,256]

    # wo pack [128, 512]: head h tile (32,256) at partitions [32*(h%4),+32),
    # cols [(h//4)*256,+256)
    woT = np.asarray(inp["wo"], f32).T.reshape(8, 32, 256).transpose(1, 0, 2)  # [32,8,256]
    wop = np.zeros((128, 512), f32)
    for h in range(8):
        wop[32 * (h % 4):32 * (h % 4) + 32, (h // 4) * 256:(h // 4) * 256 + 256] = woT[:, h, :]

    Wpk = np.zeros((128, W_COLS), f32)
    Wpk[:, C_WTT:C_WTT + 2560] = wtT.reshape(128, -1)
    Wpk[:, C_FWT:C_FWT + 1024] = fwT.reshape(128, -1)
    Wpk[:, C_WQT:C_WQT + 512] = sqT(np.asarray(inp["wq"], f32) * s).reshape(128, -1)
    Wpk[:, C_WKT:C_WKT + 512] = sqT(inp["wk"]).reshape(128, -1)
    Wpk[:, C_WVT:C_WVT + 512] = hvT(inp["wv"]).reshape(128, -1)
    Wpk[:, C_WO:C_WO + 512] = wop
    Wpk[:, C_EYE:C_EYE + 128] = np.eye(128, dtype=f32)
    Wpk[:, C_DWC:C_DWC + 20] = dwc.reshape(128, -1)
    Wpk[:, C_IOTA:C_IOTA + 1] = np.arange(128, dtype=f32)[:, None]
    Wpk[:, C_BVC:C_BVC + 2] = col(inp["bv"])
    Wpk[:, C_BOC:C_BOC + 2] = col(inp["bo"])

    # xT [B, 128, 2, 4+L]: 4 leading zero cols per o-half for the conv halo
    xT = np.zeros((B, 128, 2, 4 + L), f32)
    xt_full = np.transpose(x, (0, 2, 1)).reshape(B, 2, 128, L)
    xT[:, :, :, 4:] = np.transpose(xt_full, (0, 2, 1, 3))
    xT16 = xT.reshape(B, 128, X_COLS).astype(f16)

    pk32 = np.zeros((128, P32_COLS), f32)
    pk32[:, P32_BEFF:P32_BEFF + 2] = beff
    pk32[:, P32_LNW:P32_LNW + 2] = col(inp["ln_w"])
    pk32[:, P32_LNB:P32_LNB + 2] = col(inp["ln_b"])
    pk32[:, P32_FB:P32_FB + 2] = col(inp["fusion_b"])
    pk32[:, P32_BQ:P32_BQ + 2] = col(np.asarray(inp["bq"], f32) * s)
    pk32[:, P32_BK:P32_BK + 2] = col(inp["bk"])
    pk32[:, P32_IOTA:P32_IOTA + 1] = np.arange(128, dtype=f32)[:, None]
    pk32[:, P32_DWC:P32_DWC + 20] = dwc.reshape(128, -1)
    hi = pk32.astype(f16)
    lo = (pk32 - hi.astype(f32)).astype(f16)
    Wpk[:, C_P32HI:C_P32HI + P32_COLS] = hi.astype(f32)
    Wpk[:, C_P32LO:C_P32LO + P32_COLS] = lo.astype(f32)
    Wpk16 = Wpk.astype(f16)
    return Wpk16, xT16


def build_in_maps(inputs):
    Wpk16, xT16 = _host_prep(inputs)
    in_maps = []
    if USE_AG:
        gpack = np.concatenate([Wpk16, xT16[0], xT16[1]], axis=1)  # [128, AG_COLS]
        for core in range(8):
            in_maps.append(dict(
                pksl=np.ascontiguousarray(gpack[:, AG_SL * core:AG_SL * (core + 1)])))
    else:
        pk_b = [np.ascontiguousarray(np.concatenate([Wpk16, xT16[b]], axis=1))
                for b in range(B)]
        for core in range(8):
            in_maps.append(dict(pk16=pk_b[core & 1]))
    return in_maps


def _build(force_variant=None, use_ag=None):
    import concourse.mybir as mybir
    import concourse.tile as tile
    from concourse import bacc

    F32, F32R, F16 = mybir.dt.float32, mybir.dt.float32r, mybir.dt.float16
    U32 = mybir.dt.uint32
    AF = mybir.ActivationFunctionType
    ALU = mybir.AluOpType
    # 2*0x5f3759df + 1 as signed int32, for rsqrt seed (C2 + ~i) >> 1
    RSQRT_C2 = 0xBE6EB3BF - (1 << 32)

    use_ag = USE_AG if use_ag is None else use_ag

    nc = bacc.Bacc(num_devices=8) if use_ag else bacc.Bacc()

    if use_ag:
        pksl_d = nc.dram_tensor("pksl", [128, AG_SL], F16, kind="ExternalInput")
        ag_in = nc.dram_tensor("ag_in", [128, AG_SL], F16)
        ag_out = nc.dram_tensor("ag_out", [8, 128, AG_SL], F16, addr_space="Shared")
    else:
        pk16_d = nc.dram_tensor("pk16", [128, PK_COLS], F16, kind="ExternalInput")
    ag2_in = nc.dram_tensor("ag2_in", [128, 2048], F16)
    ag2_out = nc.dram_tensor("ag2_out", [8, 128, 2048], F16, addr_space="Shared")
    outd = nc.dram_tensor("out", [SLAB, D], F16, kind="ExternalOutput")

    X0 = W_COLS  # my-batch xT offset within pk

    with tile.TileContext(nc) as tc:
        pid = nc.partition_id() if force_variant is None else None
        with tc.tile_pool(name="wpool", bufs=1) as wpool, \
             tc.tile_pool(name="ppool", bufs=1) as ppool, \
             tc.tile_pool(name="work", bufs=1) as work, \
             tc.tile_pool(name="wk3", bufs=4) as wk3, \
             tc.tile_pool(name="pspool", bufs=1, space="PSUM") as pspool:
            psO = pspool

            pk = wpool.tile([128, PK_COLS], F16, name="pk")
            pk32 = wpool.tile([128, P32_COLS], F32, name="pk32s")
            if use_ag:
                nc.sync.dma_start(ag_in[:], pksl_d[:])
                nc.gpsimd.collective_compute(
                    "AllGather", ALU.bypass,
                    replica_groups=[list(range(8))],
                    ins=[ag_in[:]], outs=[ag_out[:]])

                def load_cols(dst_c0, g_c0, g_c1):
                    # copy global pack cols [g_c0,g_c1) into pk[:, dst_c0...]
                    for blk in range(8):
                        b0, b1 = blk * AG_SL, (blk + 1) * AG_SL
                        lo, hi = max(g_c0, b0), min(g_c1, b1)
                        if lo < hi:
                            nc.sync.dma_start(
                                pk[:, dst_c0 + lo - g_c0:dst_c0 + hi - g_c0],
                                ag_out[blk, :, lo - b0:hi - b0])

                load_cols(0, 0, W_COLS)

                # my batch's xT: binary branch tree on pid (batch = pid & 1)
                def xt_tree(lo_pid, hi_pid):
                    if hi_pid - lo_pid == 1:
                        bsel = lo_pid & 1
                        load_cols(X0, W_COLS + bsel * X_COLS,
                                  W_COLS + (bsel + 1) * X_COLS)
                        return
                    mid = (lo_pid + hi_pid) // 2
                    with tc.If(pid < mid) as cc:
                        xt_tree(lo_pid, mid)
                    with cc.Else():
                        xt_tree(mid, hi_pid)

                xt_tree(0, 8)
            else:
                nc.sync.dma_start(pk[:], pk16_d[:])

            # reconstruct the f32 side-pack from fp16 hi/lo halves
            p32lo = wpool.tile([128, P32_COLS], F32, name="p32lo")
            nc.vector.tensor_copy(out=pk32[:], in_=pk[:, C_P32HI:C_P32HI + P32_COLS])
            nc.vector.tensor_copy(out=p32lo[:], in_=pk[:, C_P32LO:C_P32LO + P32_COLS])
            nc.vector.tensor_add(out=pk32[:], in0=pk32[:], in1=p32lo[:])

            # ---- on-device constants ----
            ones32 = wpool.tile([128, 512], F32R, name="ones32")
            nc.vector.memset(ones32[:].bitcast(F32), 1.0)
            ones16 = wpool.tile([128, 512], F16, name="ones16")
            nc.vector.tensor_copy(out=ones16[:], in_=ones32[:])
            zeros8 = wpool.tile([128, 8], F32, name="zeros8")
            nc.vector.memset(zeros8[:], 0.0)
            eps_sb = wpool.tile([128, 1], F32, name="eps_sb")
            nc.vector.memset(eps_sb[:], EPS_LN)
            dsign_sb = wpool.tile([128, 4, 2], F32, name="dsign_sb")
            nc.vector.memset(dsign_sb[:, :, 0:1], -1.0)
            nc.vector.memset(dsign_sb[:, :, 1:2], 1.0)

            eye16 = pk[:, C_EYE:C_EYE + 128]
            eye32 = wpool.tile([128, 128], F32R, name="eye32")
            nc.vector.tensor_copy(out=eye32[:], in_=eye16)

            # iota row via M=1 matmul: out[0,j] = sum_k iota[k]*eye[k,j]
            rowp = pspool.tile([128, 128], F32, tag="E1", name="rowp")
            nc.tensor.matmul(rowp[0:1, 0:128], pk[:, C_IOTA:C_IOTA + 1], eye16,
                             start=True, stop=True)
            iota_r16 = wpool.tile([1, 128], F16, name="iota_r16")
            nc.vector.tensor_copy(out=iota_r16[:], in_=rowp[0:1, 0:128])
            iota_r32 = wpool.tile([1, 128], F32, name="iota_r32")
            nc.vector.tensor_copy(out=iota_r32[:], in_=rowp[0:1, 0:128])

            # decay bias row [1, 512]: dbias[bb*128+i] = DECAY*(i + 128*(bb-2))
            dbias16 = wpool.tile([1, 512], F16, name="dbias16")
            for bb in range(4):
                nc.vector.tensor_scalar(
                    out=dbias16[0:1, bb * 128:(bb + 1) * 128], in0=iota_r32[:],
                    scalar1=DECAY, scalar2=DECAY * 128.0 * (bb - 2),
                    op0=ALU.mult, op1=ALU.add)

            # bias rows [1, 256] from col layout via M=1 matmuls
            bv_row = wpool.tile([1, 256], F16, name="bv_row")
            bo_row = wpool.tile([1, 256], F16, name="bo_row")
            for dst, c0 in ((bv_row, C_BVC), (bo_row, C_BOC)):
                for o in range(2):
                    rp = pspool.tile([128, 128], F32, tag="E1", name="rowp2")
                    nc.tensor.matmul(rp[0:1, 0:128], pk[:, c0 + o:c0 + o + 1],
                                     eye16, start=True, stop=True)
                    nc.vector.tensor_copy(out=dst[0:1, o * 128:(o + 1) * 128],
                                          in_=rp[0:1, 0:128])

            # causal band mask T32[r, c] = NEG where c < r else 0
            Jps = pspool.tile([128, 128], F32, tag="E2", name="Jps")
            nc.tensor.matmul(Jps[:, 0:128], ones16[0:1, 0:128], iota_r16[0:1, 0:128],
                             start=True, stop=True)
            T32 = wpool.tile([128, 128], F32, name="T32")
            nc.vector.tensor_scalar(out=T32[:], in0=Jps[:, 0:128],
                                    scalar1=pk32[:, P32_IOTA:P32_IOTA + 1],
                                    scalar2=NEG, op0=ALU.is_lt, op1=ALU.mult)


            # wo tiles at partitions 0-31: [32, 8, 256]
            woT_sb = wpool.tile([32, 8, 256], F16, name="woT_sb")
            for h in range(8):
                nc.sync.dma_start(
                    woT_sb[0:32, h, :],
                    pk[32 * (h % 4):32 * (h % 4) + 32,
                       C_WO + (h // 4) * 256:C_WO + (h // 4) * 256 + 256])

            hT32 = ppool.tile([128, 2, 2, 4 + CW], F32, name="hT32")
            vT_s = ppool.tile([128, 2, 6 * 128], F16, name="vT_s")
            vT_t = ppool.tile([128, 2, 6 * 128], F16, name="vT_t")
            vn_s = ppool.tile([128, 6, 256], F32R, name="vn_s")
            vn_t = ppool.tile([128, 6, 256], F32R, name="vn_t")
            v_all = ppool.tile([128, L // 128, 8, 36], F32R, name="v_all")
            nc.vector.memset(v_all[:, :, :, 32:33].bitcast(F32), 1.0)
            v16 = ppool.tile([128, L // 128, 256], F16, name="v16")
            qT_all = ppool.tile([128, 2, L], F16, name="qT_all")
            kT_all = ppool.tile([128, 2, L], F16, name="kT_all")
            for _o in range(2):
                for _p in range(2):
                    nc.vector.memset(hT32[:, _o, _p, 0:4], 0.0)
            vT = {0: vT_s, 1: vT_t}
            vn = {0: vn_s, 1: vn_t}

            def emit_rsqrt(eng, y, x, tmp, niter=2):
                # y <- 1/sqrt(x) elementwise; x must be > 0 (pre-clamped).
                yi, xi, ti = y.bitcast(U32), x.bitcast(U32), tmp.bitcast(U32)
                nc.vector.tensor_scalar(out=ti, in0=xi, scalar1=0, scalar2=None,
                                        op0=ALU.bitwise_not)
                nc.vector.tensor_scalar(out=ti, in0=ti, scalar1=RSQRT_C2,
                                        scalar2=None, op0=ALU.add)
                nc.vector.tensor_scalar(out=yi, in0=ti, scalar1=1, scalar2=None,
                                        op0=ALU.logical_shift_right)
                for _ in range(niter):
                    eng.tensor_mul(out=tmp, in0=y, in1=y)
                    eng.tensor_mul(out=tmp, in0=tmp, in1=x)
                    eng.tensor_scalar(out=tmp, in0=tmp, scalar1=-0.5,
                                      scalar2=1.5, op0=ALU.mult, op1=ALU.add)
                    eng.tensor_mul(out=y, in0=y, in1=tmp)

            def emit_s1f(l0, w):
                # folded conv + y/sq activations (x read from resident pack)
                y = work.tile([128, 2, CW], F32R, tag="y_sb")
                sq = work.tile([128, 2, CW], F16, tag="sq_sb")
                for o in range(2):
                    ps = pspool.tile([128, CW], F32, tag="E1")
                    for t in range(5):
                        for k in range(2):
                            xc = X0 + k * (4 + L) + l0 + 4 - t
                            nc.tensor.matmul(ps[:, 0:w],
                                             pk[:, C_WTT + ((k * 5 + t) * 2 + o) * 128:
                                                C_WTT + ((k * 5 + t) * 2 + o) * 128 + 128],
                                             pk[:, xc:xc + w],
                                             start=(t == 0 and k == 0),
                                             stop=(t == 4 and k == 1))
                    nc.scalar.activation(y[:, o, 0:w], ps[:, 0:w], AF.Identity,
                                         bias=pk32[:, P32_BEFF + o:P32_BEFF + o + 1],
                                         scale=1.0)
                    nc.scalar.activation(sq[:, o, 0:w], ps[:, 0:w], AF.Square,
                                         bias=pk32[:, P32_BEFF + o:P32_BEFF + o + 1],
                                         scale=1.0)
                return y, sq

            def emit_s1b1(l0, w, y, sq):
                # layernorm moments via replicated-moment matmuls
                mps = pspool.tile([128, 2, CW], F32, tag="E2")
                for o in range(2):
                    nc.tensor.matmul(mps[:, 0, 0:w], ones32[:, 0:128], y[:, o, 0:w],
                                     start=(o == 0), stop=(o == 1))
                    nc.tensor.matmul(mps[:, 1, 0:w], ones16[:, 0:128], sq[:, o, 0:w],
                                     start=(o == 0), stop=(o == 1))
                mu = work.tile([128, CW], F32, tag="mu")
                nc.vector.tensor_scalar_mul(out=mu[:, 0:w], in0=mps[:, 0, 0:w], scalar1=1.0 / D)
                mu2 = work.tile([128, CW], F32, tag="tmpA")
                nc.vector.tensor_mul(out=mu2[:, 0:w], in0=mu[:, 0:w], in1=mu[:, 0:w])
                var = work.tile([128, CW], F32, tag="tmpB")
                nc.vector.scalar_tensor_tensor(out=var[:, 0:w], in0=mps[:, 1, 0:w],
                                               scalar=1.0 / D, in1=mu2[:, 0:w],
                                               op0=ALU.mult, op1=ALU.subtract)
                return mu, var

            def emit_s1b2(l0, w, y, mu, var):
                par = (l0 // CW) & 1
                c0 = l0 % CW
                lnv = work.tile([128, CW], F32, tag="tmpA")
                nc.scalar.activation(lnv[:, 0:w], var[:, 0:w], AF.Ln, bias=eps_sb[:])
                rstd = work.tile([128, CW], F32, tag="tmpB")
                nc.scalar.activation(rstd[:, 0:w], lnv[:, 0:w], AF.Exp, scale=-0.5)
                for o in range(2):
                    t1 = work.tile([128, CW], F32, tag="tmpA" if o else "tmpC")
                    nc.vector.tensor_sub(out=t1[:, 0:w], in0=y[:, o, 0:w], in1=mu[:, 0:w])
                    nc.vector.tensor_mul(out=t1[:, 0:w], in0=t1[:, 0:w], in1=rstd[:, 0:w])
                    nc.vector.tensor_scalar(out=hT32[:, o, par, 4 + c0:4 + c0 + w],
                                            in0=t1[:, 0:w],
                                            scalar1=pk32[:, P32_LNW + o:P32_LNW + o + 1],
                                            scalar2=pk32[:, P32_LNB + o:P32_LNB + o + 1],
                                            op0=ALU.mult, op1=ALU.add)
                if c0 + w == CW:
                    nc.vector.tensor_copy(out=hT32[:, :, 1 - par, 0:4],
                                          in_=hT32[:, :, par, CW:CW + 4])

            def emit_s2(l0, w):
                par = (l0 // CW) & 1
                c0 = l0 % CW
                nblk = w // 128
                blk6 = lambda b: (l0 // 128 + b) % 6
                # ---- trend taps: P = t3+t5, Dt = t3-t5 (per-channel shift
                # chains on DVE/GpSimd; beats diag matmuls in this runtime) ----
                wsT = work.tile([128, 2, CW], F32R, tag="bufA")
                wtTt = work.tile([128, 2, CW], F32R, tag="bufB")
                dnT = work.tile([128, 2, CW], F32R, tag="bufC")
                for o in range(2):
                    for t in range(5):
                        src = hT32[:, o, par, 4 + c0 - t:4 + c0 - t + w]
                        cP = pk32[:, P32_DWC + t * 2 + o:P32_DWC + t * 2 + o + 1]
                        cD = pk32[:, P32_DWC + 10 + t * 2 + o:P32_DWC + 10 + t * 2 + o + 1]
                        if t == 0:
                            nc.vector.tensor_scalar(out=wtTt[:, o, 0:w], in0=src,
                                                    scalar1=cP, scalar2=None,
                                                    op0=ALU.mult)
                            nc.vector.tensor_scalar(out=dnT[:, o, 0:w], in0=src,
                                                    scalar1=cD, scalar2=None,
                                                    op0=ALU.mult)
                        else:
                            nc.vector.scalar_tensor_tensor(
                                out=wtTt[:, o, 0:w], in0=src, scalar=cP,
                                in1=wtTt[:, o, 0:w], op0=ALU.mult, op1=ALU.add)
                            nc.vector.scalar_tensor_tensor(
                                out=dnT[:, o, 0:w], in0=src, scalar=cD,
                                in1=dnT[:, o, 0:w], op0=ALU.mult, op1=ALU.add)
                    nc.vector.scalar_tensor_tensor(out=wsT[:, o, 0:w],
                                                   in0=hT32[:, o, par, 4 + c0:4 + c0 + w],
                                                   scalar=2.0, in1=wtTt[:, o, 0:w],
                                                   op0=ALU.mult, op1=ALU.subtract)
                # ---- transpose routing inputs to [l, c] ----
                wsn = work.tile([128, 4, 256], F32, tag="wsn")
                wtn = work.tile([128, 4, 256], F32, tag="wtn")
                dnn = work.tile([128, 4, 256], F32, tag="dnn")
                for srct, dst, use_act in ((wsT, wsn, False), (wtTt, wtn, True),
                                           (dnT, dnn, True)):
                    for bi in range(nblk):
                        pst = pspool.tile([128, 2, 128], F32R, tag="E1")
                        for o in range(2):
                            nc.tensor.transpose(pst[:, o, :], srct[:, o, bi * 128:(bi + 1) * 128], eye32[:])
                        nc.vector.tensor_copy(out=dst[:, bi, :], in_=pst[:])
                yield
                # ---- routing invariants (st = wn + w*dnn):
                #   A_p = sum wn_p^2, B_p = sum dnn*wn_p, C = sum dnn^2
                # then per-iteration sums are analytic:
                #   S(w) = A + w*(B + D(w)),  D(w) = B + w*C.
                g = work.tile([128, 16, 4, 2], F32, tag="g")
                diff = work.tile([128, 4, 2], F32, tag="diff")
                scrA = work.tile([128, 4, 256], F32, tag="bufB", name="scrA")
                scrB = work.tile([128, 4, 256], F32, tag="bufA", name="scrB")
                nb = nblk
                nc.vector.tensor_mul(out=scrA[:, 0:nb, :], in0=wsn[:, 0:nb, :],
                                     in1=wsn[:, 0:nb, :])
                nc.vector.tensor_reduce(out=g[:, 0, 0:nb, 0:1], in_=scrA[:, 0:nb, :],
                                        axis=mybir.AxisListType.X, op=ALU.add)
                nc.vector.tensor_mul(out=scrB[:, 0:nb, :], in0=wtn[:, 0:nb, :],
                                     in1=wtn[:, 0:nb, :])
                nc.vector.tensor_reduce(out=g[:, 0, 0:nb, 1:2], in_=scrB[:, 0:nb, :],
                                        axis=mybir.AxisListType.X, op=ALU.add)
                nc.vector.tensor_mul(out=scrA[:, 0:nb, :], in0=dnn[:, 0:nb, :],
                                     in1=dnn[:, 0:nb, :])
                nc.vector.tensor_reduce(out=g[:, 2, 0:nb, 0:1], in_=scrA[:, 0:nb, :],
                                        axis=mybir.AxisListType.X, op=ALU.add)
                nc.vector.tensor_copy(out=g[:, 2, 0:nb, 1:2], in_=g[:, 2, 0:nb, 0:1])
                nc.vector.tensor_mul(out=scrB[:, 0:nb, :], in0=dnn[:, 0:nb, :],
                                     in1=wsn[:, 0:nb, :])
                nc.vector.tensor_reduce(out=g[:, 1, 0:nb, 0:1], in_=scrB[:, 0:nb, :],
                                        axis=mybir.AxisListType.X, op=ALU.add)
                nc.vector.tensor_mul(out=scrA[:, 0:nb, :], in0=dnn[:, 0:nb, :],
                                     in1=wtn[:, 0:nb, :])
                nc.vector.tensor_reduce(out=g[:, 1, 0:nb, 1:2], in_=scrA[:, 0:nb, :],
                                        axis=mybir.AxisListType.X, op=ALU.add)
                yield
                for it in range(3):
                    if it == 0:
                        S, Dr = 0, 1
                    else:
                        # D = B + w*C ; S = A + w*(B + D)
                        nc.vector.tensor_mul(out=g[:, 4, 0:nb, :], in0=g[:, 15, 0:nb, :], in1=g[:, 2, 0:nb, :])
                        nc.vector.tensor_add(out=g[:, 4, 0:nb, :], in0=g[:, 1, 0:nb, :], in1=g[:, 4, 0:nb, :])
                        nc.vector.tensor_add(out=g[:, 5, 0:nb, :], in0=g[:, 1, 0:nb, :], in1=g[:, 4, 0:nb, :])
                        nc.vector.tensor_mul(out=g[:, 5, 0:nb, :], in0=g[:, 15, 0:nb, :], in1=g[:, 5, 0:nb, :])
                        nc.vector.tensor_add(out=g[:, 3, 0:nb, :], in0=g[:, 0, 0:nb, :], in1=g[:, 5, 0:nb, :])
                        S, Dr = 3, 4
                    # squash scale from S: g10 = 0.125*S / ((0.25*S+1)*(0.5*sqrt(S)+1e-9))
                    nc.vector.tensor_scalar_max(out=g[:, 6, 0:nb, :], in0=g[:, S, 0:nb, :],
                                                scalar1=1e-30)
                    emit_rsqrt(nc.vector, g[:, 7, 0:nb, :], g[:, 6, 0:nb, :], g[:, 8, 0:nb, :], niter=1)
                    nc.vector.tensor_mul(out=g[:, 8, 0:nb, :], in0=g[:, 6, 0:nb, :], in1=g[:, 7, 0:nb, :])
                    nc.vector.tensor_scalar(out=g[:, 9, 0:nb, :], in0=g[:, 6, 0:nb, :],
                                            scalar1=0.25, scalar2=1.0,
                                            op0=ALU.mult, op1=ALU.add)
                    nc.vector.tensor_scalar(out=g[:, 8, 0:nb, :], in0=g[:, 8, 0:nb, :],
                                            scalar1=0.5, scalar2=1e-9,
                                            op0=ALU.mult, op1=ALU.add)
                    nc.vector.tensor_mul(out=g[:, 9, 0:nb, :], in0=g[:, 9, 0:nb, :], in1=g[:, 8, 0:nb, :])
                    nc.vector.reciprocal_approx_fast(out=g[:, 13, 0:nb, :], in_=g[:, 9, 0:nb, :])
                    nc.vector.scalar_tensor_tensor(out=g[:, 10, 0:nb, :], in0=g[:, 6, 0:nb, :],
                                                   scalar=0.125, in1=g[:, 13, 0:nb, :],
                                                   op0=ALU.mult, op1=ALU.mult)
                    if it < 2:
                        # logit update u = D*scale ; diff += dsign*u ; w = dsign*tanh(diff/2)
                        nc.vector.tensor_mul(out=g[:, 5, 0:nb, :], in0=g[:, Dr, 0:nb, :], in1=g[:, 10, 0:nb, :])
                        if it == 0:
                            nc.vector.tensor_mul(out=diff[:, 0:nb, :], in0=g[:, 5, 0:nb, :], in1=dsign_sb[:, 0:nb, :])
                        else:
                            nc.vector.tensor_mul(out=g[:, 14, 0:nb, :], in0=g[:, 5, 0:nb, :], in1=dsign_sb[:, 0:nb, :])
                            nc.vector.tensor_add(out=diff[:, 0:nb, :], in0=diff[:, 0:nb, :], in1=g[:, 14, 0:nb, :])
                        nc.scalar.activation(g[:, 14, 0:nb, :], diff[:, 0:nb, :], AF.Tanh, scale=0.5)
                        nc.vector.tensor_mul(out=g[:, 15, 0:nb, :], in0=g[:, 14, 0:nb, :], in1=dsign_sb[:, 0:nb, :])
                        yield
                    else:
                        # vn = (wn + w*dnn)*scale = wn*a + dnn*b, b = w*a
                        nc.vector.tensor_mul(out=g[:, 5, 0:nb, :], in0=g[:, 15, 0:nb, :],
                                             in1=g[:, 10, 0:nb, :])
                        for bi in range(nblk):
                            nc.vector.tensor_scalar(
                                out=scrA[:, bi, :], in0=wsn[:, bi, :],
                                scalar1=g[:, 10, bi, 0:1], scalar2=None,
                                op0=ALU.mult)
                            nc.vector.scalar_tensor_tensor(
                                out=vn[0][:, blk6(bi), :], in0=dnn[:, bi, :],
                                scalar=g[:, 5, bi, 0:1], in1=scrA[:, bi, :],
                                op0=ALU.mult, op1=ALU.add)
                            nc.vector.tensor_scalar(
                                out=scrB[:, bi, :], in0=wtn[:, bi, :],
                                scalar1=g[:, 10, bi, 1:2], scalar2=None,
                                op0=ALU.mult)
                            nc.vector.scalar_tensor_tensor(
                                out=vn[1][:, blk6(bi), :], in0=dnn[:, bi, :],
                                scalar=g[:, 5, bi, 1:2], in1=scrB[:, bi, :],
                                op0=ALU.mult, op1=ALU.add)
                # ---- transpose v to vT (rolling 6-block window) ----
                for prob in range(2):
                    for bi in range(nblk):
                        pst = pspool.tile([128, 2, 128], F32R, tag="E1")
                        for o in range(2):
                            nc.tensor.transpose(pst[:, o, :], vn[prob][:, blk6(bi), o * 128:(o + 1) * 128], eye32[:])
                        m = blk6(bi)
                        if prob == 0:
                            nc.scalar.activation(vT[prob][:, :, m * 128:(m + 1) * 128],
                                                 pst[:], AF.Copy, bias=0.0, scale=1.0)
                        else:
                            nc.vector.tensor_copy(out=vT[prob][:, :, m * 128:(m + 1) * 128], in_=pst[:])
                yield
                # ---- banded time attention ----
                sfT = work.tile([128, 2, CW], F16, tag="bufA")
                tfT = work.tile([128, 2, CW], F16, tag="bufB")
                for prob in range(2):
                    vTt, vnt = vT[prob], vn[prob]
                    dstT = sfT if prob == 0 else tfT
                    q0 = 0
                    while q0 < w:
                        qw = min(256, w - q0)
                        Q0 = l0 + q0
                        mq = ((Q0 // 128) % 6) * 128
                        bbs = [bb for bb in range(1, 4)
                               if Q0 + 128 * (bb - 2) >= seg_start[0]
                               and 128 * (bb - 2) < qw]
                        Pt = wk3.tile([128, 4, 256], F32R, tag="Pbuf")
                        zones = {}
                        for bb in bbs:
                            zones.setdefault(bb // 2, []).append(bb)
                        for z, zbbs in zones.items():
                            base = zbbs[0]
                            Sps = pspool.tile([128, 2, 256], F32, tag="SbigZ",
                                              name=f"Sps_{z}")
                            for bb in zbbs:
                                K0 = Q0 + 128 * (bb - 2)
                                mk = ((K0 // 128) % 6) * 128
                                lo = max(0, 128 * (bb - 2))
                                nc.tensor.matmul(Sps[:, bb - base, 0:qw],
                                                 dbias16[0:1, bb * 128:(bb + 1) * 128],
                                                 ones16[0:1, 0:qw],
                                                 start=(bb == zbbs[0]), stop=False)
                                for o in range(2):
                                    nc.tensor.matmul(Sps[:, bb - base, lo:qw],
                                                     vTt[:, o, mk:mk + 128],
                                                     vTt[:, o, mq + lo:mq + qw],
                                                     start=False,
                                                     stop=(bb == zbbs[-1] and o == 1))
                            for bb in zbbs:
                                if bb >= 2:
                                    dlo = 128 * (bb - 2)
                                    dwdt = min(qw, dlo + 128) - dlo
                                    nc.vector.tensor_add(out=Sps[:, bb - base, dlo:dlo + dwdt],
                                                         in0=Sps[:, bb - base, dlo:dlo + dwdt],
                                                         in1=T32[:, 0:dwdt])
                            nc.scalar.activation(Pt[:, base:base + len(zbbs), 0:qw],
                                                 Sps[:, 0:len(zbbs), 0:qw], AF.Exp)
                        od = psO.tile([128, 3, 256], F32, tag="OD")
                        for bb in bbs:
                            K0 = Q0 + 128 * (bb - 2)
                            kb6 = (K0 // 128) % 6
                            lo = max(0, 128 * (bb - 2))
                            first, last = bb == bbs[0], bb == bbs[-1]
                            for o in range(2):
                                nc.tensor.matmul(od[:, o, lo:qw],
                                                 vnt[:, kb6, o * 128:(o + 1) * 128],
                                                 Pt[:, bb, lo:qw],
                                                 start=(first and o == 0),
                                                 stop=(last and o == 1))
                            nc.tensor.matmul(od[:, 2, lo:qw], ones32[:, 0:128],
                                             Pt[:, bb, lo:qw],
                                             start=first, stop=last)
                        rec = work.tile([128, 256], F32, tag="tmpB")
                        nc.vector.reciprocal_approx_fast(out=rec[:, 0:qw], in_=od[:, 2, 0:qw])
                        for o in range(2):
                            nc.vector.tensor_mul(out=dstT[:, o, q0:q0 + qw],
                                                 in0=od[:, o, 0:qw], in1=rec[:, 0:qw])
                        q0 += qw
                yield
                # ---- fusion + qkv ----
                fused = work.tile([128, 2, CW], F16, tag="bufC")
                for o in range(2):
                    psl = pspool.tile([128, 2, CW], F32, tag="L3", name="ps_fus")
                    ps = psl[:, 0]
                    for k in range(2):
                        nc.tensor.matmul(ps[:, 0:w],
                                         pk[:, C_FWT + (k * 2 + o) * 128:C_FWT + (k * 2 + o) * 128 + 128],
                                         sfT[:, k, 0:w],
                                         start=(k == 0), stop=False)
                        nc.tensor.matmul(ps[:, 0:w],
                                         pk[:, C_FWT + ((2 + k) * 2 + o) * 128:C_FWT + ((2 + k) * 2 + o) * 128 + 128],
                                         tfT[:, k, 0:w],
                                         start=False, stop=(k == 1))
                    nc.scalar.activation(fused[:, o, 0:w], ps[:, 0:w], AF.Identity,
                                         bias=pk32[:, P32_FB + o:P32_FB + o + 1], scale=1.0)
                for o in range(2):
                    psqk = pspool.tile([128, 2, CW], F32, tag="L3", name="psqk")
                    for k in range(2):
                        nc.tensor.matmul(psqk[:, 0, 0:w],
                                         pk[:, C_WQT + (k * 2 + o) * 128:C_WQT + (k * 2 + o) * 128 + 128],
                                         fused[:, k, 0:w],
                                         start=(k == 0), stop=(k == 1))
                        nc.tensor.matmul(psqk[:, 1, 0:w],
                                         pk[:, C_WKT + (k * 2 + o) * 128:C_WKT + (k * 2 + o) * 128 + 128],
                                         fused[:, k, 0:w],
                                         start=(k == 0), stop=(k == 1))
                    nc.scalar.activation(qT_all[:, o, l0:l0 + w], psqk[:, 0, 0:w], AF.Identity,
                                         bias=pk32[:, P32_BQ + o:P32_BQ + o + 1], scale=1.0)
                    nc.scalar.activation(kT_all[:, o, l0:l0 + w], psqk[:, 1, 0:w], AF.Identity,
                                         bias=pk32[:, P32_BK + o:P32_BK + o + 1], scale=1.0)
                for bi in range(nblk):
                    psvl = pspool.tile([128, 2, CW], F32, tag="L3", name="psv")
                    psv = psvl[:, 0, 0:256]
                    for k in range(2):
                        nc.tensor.matmul(psv, fused[:, k, bi * 128:(bi + 1) * 128],
                                         pk[:, C_WVT + k * 256:C_WVT + k * 256 + 256],
                                         start=(k == 0), stop=False)
                    nc.tensor.matmul(psv, ones16[0:1, 0:128], bv_row[0:1, 0:256],
                                     start=False, stop=True)
                    nc.vector.tensor_copy(out=v16[:, l0 // 128 + bi, :], in_=psv)

            def drive(chunks):
                # software-pipelined emission: chunk i+1's conv/LN instruction
                # stream is interleaved into chunk i's routing stream so PE/Act
                # have work while the serial routing chain runs on DVE.
                s1 = emit_s1f(*chunks[0])
                mv = emit_s1b1(*chunks[0], *s1)
                emit_s1b2(chunks[0][0], chunks[0][1], s1[0], *mv)
                for i, c in enumerate(chunks):
                    gen = emit_s2(*c)
                    nxt = chunks[i + 1] if i + 1 < len(chunks) else None
                    if nxt:
                        s1 = emit_s1f(*nxt)
                    next(gen)            # trend taps + transposes
                    if nxt:
                        mv = emit_s1b1(*nxt, *s1)
                    next(gen)            # invariants
                    if nxt:
                        emit_s1b2(nxt[0], nxt[1], s1[0], *mv)
                    next(gen)            # iteration 0
                    next(gen)            # iteration 1
                    for _ in gen:        # it2 + vn + vT, banded, fusion, qkv
                        pass

            def emit_mha(qlo, qhi):
                for Q0 in range(qlo, qhi, 512):
                    qw = min(512, qhi - Q0)
                    nkv = (Q0 + qw) // 128
                    ohall = work.tile([32, 4, 2, 512], F16, tag="bufA",
                                      name="ohall")
                    for hp in range(4):
                        hg = hp // 2
                        rows = [32 * ((2 * hp) % 4), 32 * ((2 * hp + 1) % 4)]
                        Oh = psO.tile([33, 2, 512], F32, tag="OD", name=f"Oh_{hp}")
                        sps = pspool.tile([128, 2, 512], F32, tag="L3", name="sps")
                        sps2 = pspool.tile([128, 2, 512], F32, tag="E2", name="sps2")
                        slots = [sps, sps2]
                        pend = None

                        def flush(p):
                            kb_, Pm_, lo_, fi_, la_ = p
                            for jj_ in range(2):
                                nc.tensor.matmul(Oh[:, jj_, lo_:qw],
                                                 v_all[:, kb_, 2 * hp + jj_, 0:33],
                                                 Pm_[:, jj_, lo_:qw],
                                                 start=fi_, stop=la_)

                        for kb in range(nkv):
                            K0 = kb * 128
                            dlt = K0 - Q0
                            lo = max(0, dlt)
                            dwdt = min(qw, dlt + 128) - dlt if dlt >= 0 else 0
                            first, last = kb == 0, kb == nkv - 1
                            sp = slots[kb % 2]
                            for jj in range(2):
                                nc.tensor.matmul(sp[:, jj, lo:qw],
                                                 kT_all[rows[jj]:rows[jj] + 32, hg, K0:K0 + 128],
                                                 qT_all[rows[jj]:rows[jj] + 32, hg, Q0 + lo:Q0 + qw],
                                                 start=True, stop=True,
                                                 tile_position=(rows[jj], 0))
                                if dlt >= 0:
                                    nc.vector.tensor_add(out=sp[:, jj, dlt:dlt + dwdt],
                                                         in0=sp[:, jj, dlt:dlt + dwdt],
                                                         in1=T32[:, 0:dwdt])
                            Pm = wk3.tile([128, 2, 512], F32R, tag="Pbuf", name="Pm")
                            nc.scalar.activation(Pm[:, :, lo:qw], sp[:, :, lo:qw], AF.Exp)
                            if pend is not None:
                                flush(pend)
                            pend = (kb, Pm, lo, first, last)
                        flush(pend)
                        # denominator row 32 -> partition 0, reciprocal there,
                        # broadcast to 32 partitions via a K=1 matmul
                        den = work.tile([33, 2, 512], F32, tag="tmpC", name="den_m")
                        nc.vector.tensor_copy(out=den[32:33, :, 0:qw],
                                              in_=Oh[32:33, :, 0:qw])
                        d0 = work.tile([1, 2, 512], F32, tag="r0row", name="d0row")
                        nc.sync.dma_start(d0[0:1, :, 0:qw], den[32:33, :, 0:qw])
                        r16 = work.tile([1, 2, 512], F16, tag="r1row", name="r16row")
                        nc.vector.reciprocal_approx_fast(out=d0[0:1, :, 0:qw],
                                                         in_=d0[0:1, :, 0:qw])
                        nc.vector.tensor_copy(out=r16[0:1, :, 0:qw],
                                              in_=d0[0:1, :, 0:qw])
                        recBp = pspool.tile([32, 2, 512], F32, tag="E2", name="recBp")
                        for jj in range(2):
                            nc.tensor.matmul(recBp[:, jj, 0:qw], ones16[0:1, 0:32],
                                             r16[0:1, jj, 0:qw], start=True, stop=True)
                        recB = work.tile([32, 2, 512], F32, tag="bufC", name="recB_sb")
                        nc.vector.tensor_copy(out=recB[:, :, 0:qw], in_=recBp[:, :, 0:qw])
                        nc.vector.tensor_mul(out=ohall[:, hp, :, 0:qw],
                                             in0=Oh[0:32, :, 0:qw],
                                             in1=recB[:, :, 0:qw])
                        if Q0 == 0:
                            nc.vector.tensor_copy(out=ohall[:, hp, :, 0:1],
                                                  in_=zeros8[0:32, 0:2].unsqueeze(-1))
                    # out projection: one 9-matmul PSUM accumulation per block
                    for bi in range(qw // 128):
                        psop = pspool.tile([128, 256], F32, tag="E1", name="psop")
                        for hp in range(4):
                            for jj in range(2):
                                nc.tensor.matmul(psop[:],
                                                 ohall[:, hp, jj, bi * 128:(bi + 1) * 128],
                                                 woT_sb[0:32, 2 * hp + jj, :],
                                                 start=(hp == 0 and jj == 0), stop=False)
                        nc.tensor.matmul(psop[:], ones16[0:1, 0:128], bo_row[0:1, 0:256],
                                         start=False, stop=True)
                        ot = work.tile([128, 256], F16, tag="tmpC", name="ot16")
                        nc.vector.tensor_copy(out=ot[:], in_=psop[:])
                        nc.sync.dma_start(outd[Q0 - qlo + bi * 128:Q0 - qlo + (bi + 1) * 128, :], ot[:])

            seg_start = [0]

            def emit_pipeline(vi):
                # position-local pipeline over [seg0, hi): own slab + one
                # 512-wide halo chunk (band reach 256 + conv/trend taps);
                # pack this slab's K and V for the cross-core AllGather.
                lo, hi = ABOUNDS[vi], ABOUNDS[vi + 1]
                seg0 = max(0, lo - 512)
                seg_start[0] = seg0
                chunks = []
                l0 = seg0
                while l0 < hi:
                    w = min(CW, hi - l0)
                    chunks.append((l0, w))
                    l0 += w
                drive(chunks)
                if force_variant is None:
                    nc.sync.dma_start(ag2_in[:, 0:1024], kT_all[:, :, lo:hi])
                    nc.sync.dma_start(ag2_in[:, 1024:2048],
                                      v16[:, 4 * vi:4 * vi + 4, :])

            def emit_variant(vi):
                # single-core path for TimelineSim: no collectives
                emit_pipeline(vi)
                for h in range(8):
                    nc.vector.tensor_copy(out=v_all[:, :, h, 0:32],
                                          in_=v16[:, :, h * 32:(h + 1) * 32])
                emit_mha(ABOUNDS[vi], ABOUNDS[vi + 1])

            if force_variant is not None:
                emit_variant(force_variant)
            else:
                with tc.If(pid < 2) as c0:
                    emit_pipeline(0)
                with c0.Else():
                    with tc.If(pid < 4) as c1:
                        emit_pipeline(1)
                    with c1.Else():
                        with tc.If(pid < 6) as c2:
                            emit_pipeline(2)
                        with c2.Else():
                            emit_pipeline(3)

                # cross-core K/V AllGather (top level: no control flow)
                nc.gpsimd.collective_compute(
                    "AllGather", ALU.bypass,
                    replica_groups=[list(range(8))],
                    ins=[ag2_in[:]], outs=[ag2_out[:]])

                # unpack the 4 slabs of my batch (batch = pid & 1)
                def unpack(b):
                    for c in (b, b + 2, b + 4, b + 6):
                        w0 = 512 * (c // 2)
                        nc.sync.dma_start(kT_all[:, :, w0:w0 + 512],
                                          ag2_out[c, :, 0:1024])
                        nc.sync.dma_start(v16[:, w0 // 128:w0 // 128 + 4, :],
                                          ag2_out[c, :, 1024:2048])

                def up_tree(lo_pid, hi_pid):
                    if hi_pid - lo_pid == 1:
                        unpack(lo_pid & 1)
                        return
                    mid = (lo_pid + hi_pid) // 2
                    with tc.If(pid < mid) as cc:
                        up_tree(lo_pid, mid)
                    with cc.Else():
                        up_tree(mid, hi_pid)

                up_tree(0, 8)
                for h in range(8):
                    nc.vector.tensor_copy(out=v_all[:, :, h, 0:32],
                                          in_=v16[:, :, h * 32:(h + 1) * 32])

                with tc.If(pid < 2) as d0:
                    emit_mha(ABOUNDS[0], ABOUNDS[1])
                with d0.Else():
                    with tc.If(pid < 4) as d1:
                        emit_mha(ABOUNDS[1], ABOUNDS[2])
                    with d1.Else():
                        with tc.If(pid < 6) as d2:
                            emit_mha(ABOUNDS[2], ABOUNDS[3])
                        with d2.Else():
                            emit_mha(ABOUNDS[3], ABOUNDS[4])
    nc.finalize()
    return nc


_CACHE = {}


def kernel(**inputs):
    from concourse.bass_utils import run_bass_kernel_spmd
    in_maps = build_in_maps(inputs)
    if "nc" not in _CACHE:
        _CACHE["nc"] = _build()
    nc = _CACHE["nc"]
    res = run_bass_kernel_spmd(nc, in_maps, core_ids=list(range(8)))
    out = np.zeros((B, L, D), np.float32)
    for core in range(8):
        b = core & 1
        vi = core // 2
        lo, hi = ABOUNDS[vi], ABOUNDS[vi + 1]
        out[b, lo:hi, :] = res.results[core]["out"].astype(np.float32)
    return out
